# revision 45
# baseline (speedup 1.0000x reference)
"""Trainium2 Bass kernel for nn_LlamaForSequenceRegression_14336600834254.

2-layer Llama (D=2048, H=16, HD=128, F=5632, LoRA r=16 on q/v) + regression
head, B=2, S=1024, fp32 reference.

Distribution (8 NeuronCores): DP2 x TP4.
  - cores 0-3 process batch 0, cores 4-7 batch 1 (data parallel).
  - within each group of 4: Megatron tensor parallel — Wq/Wk/Wv column
    shards (4 heads/core), Wo row shards, Wgate/Wup column shards
    (F/4=1408), Wdown row shards. AllReduce (bf16) after attn-out and
    after MLP-down, replica_groups=[[0,1,2,3],[4,5,6,7]].
  - embedding gather + norm-weight folding are done host-side; all
    device matmuls run in bf16 with fp32 PSUM accumulation; the
    residual stream / softmax / rmsnorm statistics are fp32.

Layout: activations are kept feature-major ("transposed"): h^T [D, T] as
SBUF tiles [128 part, 16 kchunk, 1024 tok] so every weight matmul uses the
natural [in, out] weight layout as lhsT and no transposes are needed.
Attention uses scores^T [Tk, Tq] so softmax needs no max-subtraction
(|scores| < ~6 with folded 1/sqrt(HD)) and probs feed the v-matmul
directly; the causal mask is an upload-once 0/1 strip multiplied into the
diagonal tiles, and the attention_mask rides the exp() per-partition bias.

Perf structure (v2):
  - attention / out_proj / AllReduce pipelined per token half so the attn
    AllReduce overlaps the other half's attention + the first MLP half.
  - partition reductions (rmsnorm sum-of-squares, softmax denominators)
    use an all-ones [128,128] stationary operand so the PSUM result is
    already broadcast across partitions: no M=1 matmuls, no
    single-partition reciprocals, no gpsimd broadcasts.
  - last layer runs in "tail" mode: only the last token flows through
    q/attention/Wo/MLP. Those matvecs are transposed — the activation
    vector is the stationary operand and the weights stream through the
    PE as the moving operand — so the tail is weight-DMA bound instead
    of LDWEIGHTS bound.
"""

import numpy as np
import ml_dtypes

import concourse.bacc as bacc
import concourse.tile as tile
from concourse import mybir
from concourse import bass_utils

BF16 = ml_dtypes.bfloat16
FP32 = np.float32

V, D, L, H, HD, F, R, ALPHA, B, S, OUT = 32000, 2048, 2, 16, 128, 5632, 16, 32, 2, 1024, 11
EPS = 1e-5
SCALING = ALPHA / R
N_CORES = 8
TP = 4
NH = H // TP          # 4 local heads
DL = NH * HD          # 512 local q/k/v cols
FL = F // TP          # 1408 local mlp cols
KC = D // 128         # 16 contraction chunks
FC = FL // 128        # 11 mlp chunks
TT = 512              # token tile (free dim per matmul)
NT = S // TT          # 2 token tiles
TC = S // 128         # 8 token chunks (128-wide)
DC = D // TT          # 4 output chunks of 512
FLCH = [(0, 512), (512, 512), (1024, 384)]  # FL split into <=512 chunks
REPLICA_GROUPS = [[0, 1, 2, 3], [4, 5, 6, 7]]

dt = mybir.dt


def build_program():
    """Build the SPMD Bass program (identical on all 8 cores; weights differ
    per core via the input maps)."""
    nc = bacc.Bacc(num_devices=N_CORES, debug=False)

    # ---- DRAM I/O ----
    xT = nc.dram_tensor("xT", [128, KC, S], dt.bfloat16, kind="ExternalInput")
    cosT = nc.dram_tensor("cosT", [128, S], dt.bfloat16, kind="ExternalInput")
    sinT = nc.dram_tensor("sinT", [128, S], dt.bfloat16, kind="ExternalInput")
    mstrip = nc.dram_tensor("mstrip", [128, 896], dt.bfloat16, kind="ExternalInput")
    maskbias = nc.dram_tensor("maskbias", [128, TC], dt.float32, kind="ExternalInput")
    mask01 = nc.dram_tensor("mask01", [128, TC], dt.float32, kind="ExternalInput")
    wreg = nc.dram_tensor("wreg", [KC, 128, OUT], dt.bfloat16, kind="ExternalInput")
    breg = nc.dram_tensor("breg", [OUT, 1], dt.float32, kind="ExternalInput")
    W = {}
    for l in range(L):
        W[f"wk{l}"] = nc.dram_tensor(f"wk{l}", [KC, 128, DL], dt.bfloat16, kind="ExternalInput")
        W[f"wv{l}"] = nc.dram_tensor(f"wv{l}", [KC, 128, DL], dt.bfloat16, kind="ExternalInput")
        W[f"aq{l}"] = nc.dram_tensor(f"aq{l}", [KC, 128, R], dt.bfloat16, kind="ExternalInput")
        W[f"av{l}"] = nc.dram_tensor(f"av{l}", [KC, 128, R], dt.bfloat16, kind="ExternalInput")
        W[f"bq{l}"] = nc.dram_tensor(f"bq{l}", [R, DL], dt.bfloat16, kind="ExternalInput")
        W[f"bv{l}"] = nc.dram_tensor(f"bv{l}", [R, DL], dt.bfloat16, kind="ExternalInput")
    # layer 0 (full-sequence Megatron TP layouts)
    W["wq0"] = nc.dram_tensor("wq0", [KC, 128, DL], dt.bfloat16, kind="ExternalInput")
    W["wo0"] = nc.dram_tensor("wo0", [NH, 128, D], dt.bfloat16, kind="ExternalInput")
    # gate|up interleaved per fc chunk: [fc][p][k*256 + (0:128 gate | 128:256 up)]
    W["wgu0"] = nc.dram_tensor("wgu0", [FC, 128, KC, 256], dt.bfloat16, kind="ExternalInput")
    W["wd0"] = nc.dram_tensor("wd0", [FC, 128, D], dt.bfloat16, kind="ExternalInput")
    # layer 1 (tail: weights stream as moving operand)
    W["wq1"] = nc.dram_tensor("wq1", [KC, 128, DL], dt.bfloat16, kind="ExternalInput")
    W["wo1"] = nc.dram_tensor("wo1", [NH, 128, D], dt.bfloat16, kind="ExternalInput")
    W["wgu1"] = nc.dram_tensor("wgu1", [KC, 128, 2 * FL], dt.bfloat16, kind="ExternalInput")
    W["wd1"] = nc.dram_tensor("wd1", [FC, 128, D], dt.bfloat16, kind="ExternalInput")
    out_dram = nc.dram_tensor("out", [OUT, 1], dt.float32, kind="ExternalOutput")

    with tile.TileContext(nc) as tc:
        with (
            tc.tile_pool(name="persist", bufs=1) as pp,
            tc.tile_pool(name="wts", bufs=3) as wp,
            tc.tile_pool(name="colw", bufs=4) as cwp,
            tc.tile_pool(name="tails", bufs=2) as tsp,
            tc.tile_pool(name="tmp", bufs=3) as tp_,
            tc.tile_pool(name="stage", bufs=2) as stp,
            tc.tile_pool(name="psum", bufs=8, space="PSUM") as ps,
            tc.tile_pool(name="dram", bufs=1, space="DRAM") as dram,
        ):
            f32, bf = dt.float32, dt.bfloat16
            # ---- persistent tiles ----
            h = pp.tile([128, KC, S], f32, tag="h")
            hn = pp.tile([128, KC, S], bf, tag="hn")
            cos_sb = pp.tile([128, S], bf, tag="cos")
            sin_sb = pp.tile([128, S], bf, tag="sin")
            mstrip_sb = pp.tile([128, 896], bf, tag="mstrip")
            mb_sb = pp.tile([128, TC], f32, tag="mb")
            m01_sb = pp.tile([128, TC], f32, tag="m01")
            cos_last = pp.tile([128, 1], f32, tag="cos_last")
            sin_last = pp.tile([128, 1], f32, tag="sin_last")
            ones_bf = pp.tile([128, 1], bf, tag="onesbf")
            allones = pp.tile([128, 128], bf, tag="allones")
            oneD128 = pp.tile([128, 128], bf, tag="oneD128")
            one1_bf = pp.tile([1, 1], bf, tag="one1")
            one64_bf = pp.tile([1, 1], bf, tag="one64")
            eps128 = pp.tile([128, 1], f32, tag="eps128")
            eps1 = pp.tile([1, 1], f32, tag="eps1")
            qT = pp.tile([128, NH, S], bf, tag="qT")     # q, then reused for ctx
            kT = pp.tile([128, NH, S], bf, tag="kT")
            vN = pp.tile([128, TC, DL], bf, tag="vN")
            expT = pp.tile([128, TC, TT], bf, tag="expT")
            mT = pp.tile([128, FC, TT], bf, tag="mT")    # per-half mlp act
            aqw = pp.tile([128, KC, R], bf, tag="aqw")
            avw = pp.tile([128, KC, R], bf, tag="avw")
            bq_sb = pp.tile([R, DL], bf, tag="bq")
            bv_sb = pp.tile([R, DL], bf, tag="bv")
            aq_sb = pp.tile([R, S], bf, tag="aq")
            av_sb = pp.tile([R, S], bf, tag="av")
            wreg_sb = pp.tile([128, KC, OUT], bf, tag="wreg")
            breg_sb = pp.tile([OUT, 1], f32, tag="breg")
            # tail smalls
            hn_last = pp.tile([128, KC, 1], bf, tag="hn_last")
            q_last = pp.tile([128, NH], bf, tag="q_last")
            ctx_n = pp.tile([128, NH], bf, tag="ctx_n")
            exp_tail = pp.tile([128, NH, TC], bf, tag="exp_tail")
            m_pm = pp.tile([128, FC], bf, tag="m_pm")

            # ---- constants in ----
            nc.vector.memset(ones_bf[:], 1.0)
            nc.vector.memset(allones[:], 1.0)
            nc.vector.memset(oneD128[:], 1.0 / D)
            nc.vector.memset(one1_bf[:], 1.0)
            nc.vector.memset(one64_bf[:], 1.0 / 64.0)
            nc.vector.memset(eps128[:], EPS)
            nc.vector.memset(eps1[:], EPS)
            nc.sync.dma_start(cos_sb[:], cosT[:])
            nc.sync.dma_start(sin_sb[:], sinT[:])
            nc.sync.dma_start(mstrip_sb[:], mstrip[:])
            nc.sync.dma_start(mb_sb[:], maskbias[:])
            nc.sync.dma_start(m01_sb[:], mask01[:])
            nc.scalar.copy(cos_last[:], cos_sb[:, S - 1:S])
            nc.scalar.copy(sin_last[:], sin_sb[:, S - 1:S])
            nc.sync.dma_start(breg_sb[:], breg[:])
            for k in range(KC):
                nc.sync.dma_start(wreg_sb[:, k, :], wreg[k])

            # ---- h init per half: bf16 upload -> fp32 residual ----
            for t in range(NT):
                ts_ = slice(t * TT, (t + 1) * TT)
                nc.sync.dma_start(hn[:, :, ts_], xT[:, :, ts_])
                nc.vector.tensor_copy(h[:, :, ts_], hn[:, :, ts_])

            # DRAM bounce buffers for collectives
            ar1h_in = [dram.tile([128, KC, TT], bf, name=f"ar1hi_{t}") for t in range(NT)]
            ar1h_out = [dram.tile([128, KC, TT], bf, name=f"ar1ho_{t}") for t in range(NT)]
            ar2h_in = [dram.tile([128, KC, TT], bf, name=f"ar2hi_{t}") for t in range(NT)]
            ar2h_out = [dram.tile([128, KC, TT], bf, name=f"ar2ho_{t}") for t in range(NT)]
            ar_in_s = dram.tile([128, KC, 1], bf)
            ar_out_s = dram.tile([128, KC, 1], bf)

            def norm_half(t):
                """hn[:, :, half t] = h / sqrt(mean(h^2) + eps), bf16."""
                ts_ = slice(t * TT, (t + 1) * TT)
                psb = ps.tile([128, TT], f32, tag="psum", name=f"nps_{t}")
                for k in range(KC):
                    sq = tp_.tile([128, TT], bf, tag="sq", bufs=2, name=f"nsq_{k}_{t}")
                    nc.scalar.activation(sq[:], h[:, k, ts_],
                                         mybir.ActivationFunctionType.Square)
                    # all-ones/D stationary => result broadcast to all partitions
                    nc.tensor.matmul(psb[:], oneD128[:], sq[:],
                                     start=(k == 0), stop=(k == KC - 1))
                rs = tp_.tile([128, TT], f32, tag="rsbc", bufs=1, name=f"nrs_{t}")
                nc.scalar.activation(rs[:], psb[:],
                                     mybir.ActivationFunctionType.Sqrt, bias=eps128[:])
                nc.vector.reciprocal_approx_fast(rs[:], rs[:])
                for k in range(KC):
                    nc.vector.tensor_mul(hn[:, k, ts_], h[:, k, ts_], rs[:])

            def lora_down_half(aw, dst, t):
                psa = ps.tile([128, TT], f32, tag="psum", name=f"ldh_{id(aw)}_{t}")
                for k in range(KC):
                    nc.tensor.matmul(
                        psa[0:R, :], aw[:, k, :], hn[:, k, t * TT:(t + 1) * TT],
                        start=(k == 0), stop=(k == KC - 1),
                    )
                nc.scalar.copy(dst[:, t * TT:(t + 1) * TT], psa[0:R, :])

            def rope_from_psum(psq, dst, hc, t):
                """Apply RoPE to psum [128,TT] (one head, token tile t) and
                write bf16 into dst[:, hc, t*TT:...]."""
                ts_ = slice(t * TT, (t + 1) * TT)
                t2 = tp_.tile([128, TT], bf, tag="ropetB", bufs=1)
                t4 = tp_.tile([128, TT], bf, tag="ropetB", bufs=1)
                nc.vector.tensor_mul(dst[0:64, hc, ts_], psq[0:64, :], cos_sb[0:64, ts_])
                nc.vector.tensor_mul(t2[0:64, :], psq[64:128, :], sin_sb[0:64, ts_])
                nc.vector.tensor_sub(dst[0:64, hc, ts_], dst[0:64, hc, ts_], t2[0:64, :])
                nc.vector.tensor_mul(dst[64:128, hc, ts_], psq[64:128, :], cos_sb[64:128, ts_])
                nc.vector.tensor_mul(t4[64:128, :], psq[0:64, :], sin_sb[64:128, ts_])
                nc.vector.tensor_add(dst[64:128, hc, ts_], dst[64:128, hc, ts_], t4[64:128, :])

            def qk_proj(wname, dst, lora_bw, lora_act):
                """dst[:, hc, :] (bf16, roped) = rope(W.T @ hn [+ lora])."""
                psq = [[ps.tile([128, TT], f32, tag="psum", name=f"psq_{wname}_{hc}_{t}")
                        for t in range(NT)] for hc in range(NH)]
                for kk in range(KC // 2):
                    wt = wp.tile([128, 2, DL], bf, tag="wqkv", name=f"w_{wname}_{kk}")
                    nc.sync.dma_start(wt[:], W[wname][2 * kk:2 * kk + 2].rearrange("i p c -> p i c"))
                    for i in range(2):
                        k = 2 * kk + i
                        for hc in range(NH):
                            for t in range(NT):
                                nc.tensor.matmul(
                                    psq[hc][t][:], wt[:, i, hc * HD:(hc + 1) * HD],
                                    hn[:, k, t * TT:(t + 1) * TT],
                                    start=(k == 0),
                                    stop=(lora_bw is None and k == KC - 1),
                                )
                for hc in range(NH):
                    for t in range(NT):
                        if lora_bw is not None:
                            nc.tensor.matmul(
                                psq[hc][t][:], lora_bw[:, hc * HD:(hc + 1) * HD],
                                lora_act[:, t * TT:(t + 1) * TT],
                                start=False, stop=True,
                            )
                        rope_from_psum(psq[hc][t], dst, hc, t)

            def v_proj(l):
                """vN [128(tok), TC, DL] bf16 = hn.T @ Wv + lora."""
                psv = [ps.tile([128, DL], f32, tag="psum", name=f"psv_{c}")
                       for c in range(TC)]
                for kk in range(KC // 2):
                    wt = wp.tile([128, 2, DL], bf, tag="wqkv", name=f"wv_t_{kk}")
                    nc.sync.dma_start(wt[:], W[f"wv{l}"][2 * kk:2 * kk + 2].rearrange("i p c -> p i c"))
                    for i in range(2):
                        k = 2 * kk + i
                        for c in range(TC):
                            nc.tensor.matmul(
                                psv[c][:], hn[:, k, c * 128:(c + 1) * 128], wt[:, i, :],
                                start=(k == 0), stop=False,
                            )
                for c in range(TC):
                    nc.tensor.matmul(
                        psv[c][:], av_sb[:, c * 128:(c + 1) * 128], bv_sb[:],
                        start=False, stop=True,
                    )
                    nc.scalar.copy(vN[:, c, :], psv[c][:])

            def q_proj_half(wname, dst, t):
                """dst[:, hc, half t] = rope(Wq.T @ hn + lora)."""
                psq = [ps.tile([128, TT], f32, tag="psum", name=f"psqq_{hc}_{t}")
                       for hc in range(NH)]
                for kk in range(KC // 2):
                    wt = wp.tile([128, 2, DL], bf, tag="wqkv", name=f"wq_{kk}_{t}")
                    nc.sync.dma_start(wt[:], W[wname][2 * kk:2 * kk + 2].rearrange("i p c -> p i c"))
                    for i in range(2):
                        k = 2 * kk + i
                        for hc in range(NH):
                            nc.tensor.matmul(
                                psq[hc][:], wt[:, i, hc * HD:(hc + 1) * HD],
                                hn[:, k, t * TT:(t + 1) * TT],
                                start=(k == 0), stop=False,
                            )
                for hc in range(NH):
                    nc.tensor.matmul(
                        psq[hc][:], bq_sb[:, hc * HD:(hc + 1) * HD],
                        aq_sb[:, t * TT:(t + 1) * TT],
                        start=False, stop=True,
                    )
                    rope_from_psum(psq[hc], dst, hc, t)

            def attention_half(t):
                """qT,kT,vN -> ctx (written into qT) for token half t.

                The denominator/ctx accumulation matmuls for chunk j are
                emitted two chunks behind the QK+exp chain, so by the time
                the PE (strict in-order queue) reaches them their exp input
                has drained from the ACT engine and nothing head-of-line
                blocks."""
                ts_ = slice(t * TT, (t + 1) * TT)
                jmax = (t + 1) * (TT // 128)
                LAG = 2
                for hh in range(NH):
                    psb = ps.tile([128, TT], f32, tag="psum", name=f"psd_{hh}_{t}")
                    psc = ps.tile([128, TT], f32, tag="psum", name=f"psc_{hh}_{t}")

                    def emit_reduce_j(j):
                        nc.tensor.matmul(
                            psb[:], allones[:], expT[:, j, :],
                            start=(j == 0), stop=(j == jmax - 1),
                        )
                        nc.tensor.matmul(
                            psc[:], vN[:, j, hh * HD:(hh + 1) * HD],
                            expT[:, j, :],
                            start=(j == 0), stop=(j == jmax - 1),
                        )

                    for j in range(jmax):
                        pss = ps.tile([128, TT], f32, tag="psum",
                                      name=f"pss_{hh}_{t}_{j}")
                        nc.tensor.matmul(
                            pss[:], kT[:, hh, j * 128:(j + 1) * 128],
                            qT[:, hh, ts_], start=True, stop=True,
                        )
                        nc.scalar.activation(
                            expT[:, j, :], pss[:],
                            mybir.ActivationFunctionType.Exp,
                            bias=mb_sb[:, j:j + 1], scale=1.0,
                        )
                        off = t * TT - j * 128
                        if off < 128:
                            nc.vector.tensor_mul(
                                expT[:, j, :], expT[:, j, :],
                                mstrip_sb[:, 384 + off:896 + off],
                            )
                        if j >= LAG:
                            emit_reduce_j(j - LAG)
                    for j in range(max(0, jmax - LAG), jmax):
                        emit_reduce_j(j)
                    rden = tp_.tile([128, TT], f32, tag="rsbc", bufs=1,
                                    name=f"rden_{hh}_{t}")
                    nc.vector.reciprocal_approx_fast(rden[:], psb[:])
                    nc.vector.tensor_mul(qT[:, hh, ts_], psc[:], rden[:])

            def out_proj_half(l, t):
                """attn partial for token half t -> ar1h_in[t]."""
                for og in range(4):  # groups of 4 output chunks
                    pso = [ps.tile([128, TT], f32, tag="psum", name=f"pso_{og}_{oi}_{t}")
                           for oi in range(4)]
                    for hp in range(NH // 2):
                        wt = wp.tile([128, 2, TT], bf, tag="wqkv",
                                     name=f"wo_t_{og}_{hp}_{t}")
                        nc.sync.dma_start(
                            wt[:], W[f"wo{l}"][2 * hp:2 * hp + 2,
                                               :, og * 512:(og + 1) * 512]
                            .rearrange("i p c -> p i c"))
                        for i in range(2):
                            hc = 2 * hp + i
                            for oi in range(4):
                                nc.tensor.matmul(
                                    pso[oi][:], wt[:, i, oi * 128:(oi + 1) * 128],
                                    qT[:, hc, t * TT:(t + 1) * TT],
                                    start=(hc == 0), stop=(hc == NH - 1),
                                )
                    for oi in range(4):
                        st = stp.tile([128, TT], bf, tag="stage")
                        nc.vector.tensor_copy(st[:], pso[oi][:])
                        nc.sync.dma_start(ar1h_in[t][:, og * 4 + oi, :], st[:])

            def allreduce1_half(t):
                nc.gpsimd.collective_compute(
                    "AllReduce", mybir.AluOpType.add,
                    replica_groups=REPLICA_GROUPS,
                    ins=[ar1h_in[t].opt()], outs=[ar1h_out[t].opt()],
                )

            def add1_half(t):
                # landing DMA rides the scalar HWDGE ring so its AR wait can't
                # head-of-line-block weight loads on the sync ring
                ts_ = slice(t * TT, (t + 1) * TT)
                nc.scalar.dma_start(hn[:, :, ts_], ar1h_out[t][:])
                for k in range(KC):
                    nc.vector.tensor_add(h[:, k, ts_], h[:, k, ts_], hn[:, k, ts_])

            def allreduce2_half(t):
                nc.gpsimd.collective_compute(
                    "AllReduce", mybir.AluOpType.add,
                    replica_groups=REPLICA_GROUPS,
                    ins=[ar2h_in[t].opt()], outs=[ar2h_out[t].opt()],
                )

            def add2_half(t):
                ts_ = slice(t * TT, (t + 1) * TT)
                nc.scalar.dma_start(hn[:, :, ts_], ar2h_out[t][:])
                for k in range(KC):
                    nc.vector.tensor_add(h[:, k, ts_], h[:, k, ts_], hn[:, k, ts_])

            def k_proj_half(wname, dst, t, tag="wqkv"):
                psq = [ps.tile([128, TT], f32, tag="psum", name=f"psqh_{hc}_{t}")
                       for hc in range(NH)]
                for kk in range(KC // 2):
                    wt = wp.tile([128, 2, DL], bf, tag=tag, name=f"wkh_{wname}_{kk}_{t}")
                    nc.sync.dma_start(wt[:], W[wname][2 * kk:2 * kk + 2].rearrange("i p c -> p i c"))
                    for i in range(2):
                        k = 2 * kk + i
                        for hc in range(NH):
                            nc.tensor.matmul(
                                psq[hc][:], wt[:, i, hc * HD:(hc + 1) * HD],
                                hn[:, k, t * TT:(t + 1) * TT],
                                start=(k == 0), stop=(k == KC - 1),
                            )
                for hc in range(NH):
                    rope_from_psum(psq[hc], dst, hc, t)

            def v_proj_half(l, t, tag="wqkv"):
                psv = [ps.tile([128, DL], f32, tag="psum", name=f"psvh_{c}_{t}")
                       for c in range(4)]
                for kk in range(KC // 2):
                    wt = wp.tile([128, 2, DL], bf, tag=tag, name=f"wvh_{l}_{kk}_{t}")
                    nc.sync.dma_start(wt[:], W[f"wv{l}"][2 * kk:2 * kk + 2].rearrange("i p c -> p i c"))
                    for i in range(2):
                        k = 2 * kk + i
                        for ci in range(4):
                            c = t * 4 + ci
                            nc.tensor.matmul(
                                psv[ci][:], hn[:, k, c * 128:(c + 1) * 128], wt[:, i, :],
                                start=(k == 0), stop=False,
                            )
                for ci in range(4):
                    c = t * 4 + ci
                    nc.tensor.matmul(
                        psv[ci][:], av_sb[:, c * 128:(c + 1) * 128], bv_sb[:],
                        start=False, stop=True,
                    )
                    nc.scalar.copy(vN[:, c, :], psv[ci][:])

            def mlp_gate_up_half(l, t):
                ts_ = slice(t * TT, (t + 1) * TT)
                for fc in range(FC):
                    psg = ps.tile([128, TT], f32, tag="psum", name=f"psg_{fc}_{t}")
                    psu = ps.tile([128, TT], f32, tag="psum", name=f"psu_{fc}_{t}")
                    for kh in range(2):
                        wgu = cwp.tile([128, KC // 2, 256], bf, tag="wgu",
                                       name=f"wgu_{fc}_{kh}_{t}")
                        nc.sync.dma_start(
                            wgu[:], W[f"wgu{l}"][fc][:, kh * (KC // 2):(kh + 1) * (KC // 2), :])
                        for ki in range(KC // 2):
                            k = kh * (KC // 2) + ki
                            nc.tensor.matmul(psg[:], wgu[:, ki, 0:128], hn[:, k, ts_],
                                             start=(k == 0), stop=(k == KC - 1))
                            nc.tensor.matmul(psu[:], wgu[:, ki, 128:256], hn[:, k, ts_],
                                             start=(k == 0), stop=(k == KC - 1))
                    sg = tp_.tile([128, TT], bf, tag="silu", bufs=1, name=f"sg_{fc}_{t}")
                    nc.scalar.activation(sg[:], psg[:], mybir.ActivationFunctionType.Silu)
                    nc.vector.tensor_mul(mT[:, fc, :], sg[:], psu[:])

            def mlp_down_half(l, t):
                for og in range(4):
                    pso = [ps.tile([128, TT], f32, tag="psum", name=f"psd_{og}_{oi}_{t}")
                           for oi in range(4)]
                    for kp in range((FC + 1) // 2):
                        nk = min(2, FC - 2 * kp)
                        wt = wp.tile([128, 2, TT], bf, tag="wqkv",
                                     name=f"wd_t_{og}_{kp}_{t}")
                        nc.sync.dma_start(
                            wt[:, 0:nk, :],
                            W[f"wd{l}"][2 * kp:2 * kp + nk,
                                        :, og * 512:(og + 1) * 512]
                            .rearrange("i p c -> p i c"))
                        for i in range(nk):
                            kc = 2 * kp + i
                            for oi in range(4):
                                nc.tensor.matmul(
                                    pso[oi][:], wt[:, i, oi * 128:(oi + 1) * 128],
                                    mT[:, kc, :],
                                    start=(kc == 0), stop=(kc == FC - 1),
                                )
                    for oi in range(4):
                        st = stp.tile([128, TT], bf, tag="stage")
                        nc.vector.tensor_copy(st[:], pso[oi][:])
                        nc.sync.dma_start(ar2h_in[t][:, og * 4 + oi, :], st[:])

            # ---------- tail (last layer): only the last token flows through
            # q/attention/Wo/MLP. Matvecs are transposed: the activation
            # column is the stationary operand, weights stream as rhs.

            def row_to_pm(row_sb, psum_pm, ncols):
                """psum_pm[:, c] = row_sb[0, c*128:(c+1)*128].T via K=1 matmuls."""
                for c in range(ncols):
                    nc.tensor.matmul(
                        psum_pm[:, c:c + 1],
                        row_sb[0:1, c * 128:(c + 1) * 128], one1_bf[:],
                        start=True, stop=True,
                    )

            def q_tail():
                psq = ps.tile([128, TT], f32, tag="psum", name="psq_tail")
                for kk in range(KC // 2):
                    wt = wp.tile([128, 2, DL], bf, tag="wqkv", name=f"wq1s_{kk}")
                    nc.sync.dma_start(wt[:], W["wq1"][2 * kk:2 * kk + 2].rearrange("i p c -> p i c"))
                    for i in range(2):
                        k = 2 * kk + i
                        nc.tensor.matmul(psq[0:1, :], hn[:, k, S - 1:S], wt[:, i, :],
                                         start=(k == 0), stop=False)
                nc.tensor.matmul(psq[0:1, :], aq_sb[:, S - 1:S], bq_sb[:],
                                 start=False, stop=True)
                qrow = tp_.tile([1, DL], bf, tag="qrow", bufs=1)
                nc.scalar.copy(qrow[:], psq[0:1, :])
                psqpm = ps.tile([128, NH], f32, tag="psum", name="psqpm")
                row_to_pm(qrow, psqpm, NH)
                # rope, all heads at once (per-partition cos/sin scalars)
                t2 = tp_.tile([128, NH], bf, tag="ropeS", bufs=2)
                t4 = tp_.tile([128, NH], bf, tag="ropeS", bufs=2)
                nc.vector.tensor_scalar_mul(q_last[0:64, :], psqpm[0:64, :],
                                            cos_last[0:64, :])
                nc.vector.tensor_scalar_mul(t2[0:64, :], psqpm[64:128, :],
                                            sin_last[0:64, :])
                nc.vector.tensor_sub(q_last[0:64, :], q_last[0:64, :], t2[0:64, :])
                nc.vector.tensor_scalar_mul(q_last[64:128, :], psqpm[64:128, :],
                                            cos_last[64:128, :])
                nc.vector.tensor_scalar_mul(t4[64:128, :], psqpm[0:64, :],
                                            sin_last[64:128, :])
                nc.vector.tensor_add(q_last[64:128, :], q_last[64:128, :], t4[64:128, :])

            def attention_tail():
                # scores for all heads/key-chunks in one psum tile
                pss = ps.tile([128, NH, TC], f32, tag="psum", name="pss_tail")
                for hh in range(NH):
                    for j in range(TC):
                        nc.tensor.matmul(
                            pss[:, hh, j:j + 1],
                            kT[:, hh, j * 128:(j + 1) * 128], q_last[:, hh:hh + 1],
                            start=True, stop=True,
                        )
                nc.scalar.activation(exp_tail[:], pss[:],
                                     mybir.ActivationFunctionType.Exp)
                for j in range(TC):
                    nc.vector.tensor_scalar_mul(exp_tail[:, :, j], exp_tail[:, :, j],
                                                m01_sb[:, j:j + 1])
                # denominators: all-partition sums then free-dim reduce per head
                psdb = ps.tile([128, NH, TC], f32, tag="psum", name="psdb_tail")
                nc.tensor.matmul(psdb[:], allones[:], exp_tail[:],
                                 start=True, stop=True)
                rdt = tp_.tile([128, NH], f32, tag="rdt")
                nc.vector.tensor_reduce(
                    rdt[:], psdb[:],
                    axis=mybir.AxisListType.X, op=mybir.AluOpType.add,
                )
                nc.vector.reciprocal_approx_fast(rdt[:], rdt[:])
                psc = ps.tile([128, NH], f32, tag="psum", name="psc_tail")
                for hh in range(NH):
                    for j in range(TC):
                        nc.tensor.matmul(
                            psc[:, hh:hh + 1], vN[:, j, hh * HD:(hh + 1) * HD],
                            exp_tail[:, hh, j:j + 1],
                            start=(j == 0), stop=(j == TC - 1),
                        )
                nc.vector.tensor_mul(ctx_n[:], psc[:], rdt[:])

            def wo_tail():
                pso = [ps.tile([128, TT], f32, tag="psum", name=f"psot_{c}")
                       for c in range(DC)]
                for hc in range(NH):
                    for dh in range(2):
                        wt = tsp.tile([128, 1024], bf, tag="wod1s",
                                      name=f"wo1s_{hc}_{dh}")
                        nc.sync.dma_start(wt[:], W["wo1"][hc][:, dh * 1024:(dh + 1) * 1024])
                        for ci in range(2):
                            c = dh * 2 + ci
                            nc.tensor.matmul(
                                pso[c][0:1, :], ctx_n[:, hc:hc + 1],
                                wt[:, ci * 512:(ci + 1) * 512],
                                start=(hc == 0), stop=(hc == NH - 1),
                            )
                orow = tp_.tile([1, D], bf, tag="orow", bufs=1)
                for c in range(DC):
                    nc.scalar.copy(orow[0:1, c * 512:(c + 1) * 512], pso[c][0:1, :])
                pst = ps.tile([128, KC], f32, tag="psum", name="pst_wo")
                row_to_pm(orow, pst, KC)
                stpm = tp_.tile([128, KC], bf, tag="stpm", bufs=2, name="stpm_wo")
                nc.vector.tensor_copy(stpm[:], pst[:])
                nc.sync.dma_start(ar_in_s[:, :, 0], stpm[:])

            def allreduce_and_add_tail():
                nc.gpsimd.collective_compute(
                    "AllReduce", mybir.AluOpType.add,
                    replica_groups=REPLICA_GROUPS,
                    ins=[ar_in_s.opt()], outs=[ar_out_s.opt()],
                )
                lb = tp_.tile([128, KC, 1], bf, tag="ar_land", bufs=2)
                nc.scalar.dma_start(lb[:], ar_out_s[:])
                nc.vector.tensor_add(h[:, :, S - 1:S], h[:, :, S - 1:S], lb[:])

            def norm_tail_to_hn_last():
                sqt = tp_.tile([128, KC, 1], bf, tag="sqlast")
                nc.scalar.activation(sqt[:], h[:, :, S - 1:S],
                                     mybir.ActivationFunctionType.Square)
                psl = ps.tile([128, 1], f32, tag="psum", name="psl_normt")
                for k in range(KC):
                    nc.tensor.matmul(psl[:], oneD128[:], sqt[:, k, :],
                                     start=(k == 0), stop=(k == KC - 1))
                rsb = tp_.tile([128, 1], f32, tag="rsb_tail", bufs=2)
                nc.scalar.activation(rsb[:], psl[:],
                                     mybir.ActivationFunctionType.Sqrt, bias=eps128[:])
                nc.vector.reciprocal_approx_fast(rsb[:], rsb[:])
                nc.vector.tensor_scalar_mul(hn_last[:], h[:, :, S - 1:S], rsb[:])

            def mlp_tail():
                norm_tail_to_hn_last()
                # gate/up: stream combined gate|up weights as rhs
                psg = [ps.tile([128, TT], f32, tag="psum", name=f"psgt_{c}")
                       for c in range(len(FLCH))]
                psu = [ps.tile([128, TT], f32, tag="psum", name=f"psut_{c}")
                       for c in range(len(FLCH))]
                for k in range(KC):
                    wgt = tsp.tile([128, FL], bf, tag="wg1s", name=f"wg1s_{k}")
                    wut = tsp.tile([128, FL], bf, tag="wu1s", name=f"wu1s_{k}")
                    nc.sync.dma_start(wgt[:], W["wgu1"][k][:, 0:FL])
                    nc.sync.dma_start(wut[:], W["wgu1"][k][:, FL:2 * FL])
                    for c, (off, sz) in enumerate(FLCH):
                        nc.tensor.matmul(psg[c][0:1, 0:sz], hn_last[:, k, :],
                                         wgt[:, off:off + sz],
                                         start=(k == 0), stop=(k == KC - 1))
                        nc.tensor.matmul(psu[c][0:1, 0:sz], hn_last[:, k, :],
                                         wut[:, off:off + sz],
                                         start=(k == 0), stop=(k == KC - 1))
                mrow = tp_.tile([1, FL], bf, tag="mrow", bufs=1)
                for c, (off, sz) in enumerate(FLCH):
                    grow = tp_.tile([1, TT], bf, tag="grow", bufs=1, name=f"grow_{c}")
                    nc.scalar.activation(grow[0:1, 0:sz], psg[c][0:1, 0:sz],
                                         mybir.ActivationFunctionType.Silu)
                    nc.vector.tensor_mul(mrow[0:1, off:off + sz], grow[0:1, 0:sz],
                                         psu[c][0:1, 0:sz])
                psm = ps.tile([128, FC], f32, tag="psum", name="psm_pm")
                row_to_pm(mrow, psm, FC)
                nc.vector.tensor_copy(m_pm[:], psm[:])
                # down
                psd = [ps.tile([128, TT], f32, tag="psum", name=f"psdt2_{c}")
                       for c in range(DC)]
                for fc in range(FC):
                    for dh in range(2):
                        wdt = tsp.tile([128, 1024], bf, tag="wod1s",
                                       name=f"wd1s_{fc}_{dh}")
                        nc.sync.dma_start(wdt[:],
                                          W["wd1"][fc][:, dh * 1024:(dh + 1) * 1024])
                        for ci in range(2):
                            c = dh * 2 + ci
                            nc.tensor.matmul(
                                psd[c][0:1, :], m_pm[:, fc:fc + 1],
                                wdt[:, ci * 512:(ci + 1) * 512],
                                start=(fc == 0), stop=(fc == FC - 1),
                            )
                orow = tp_.tile([1, D], bf, tag="orow", bufs=1)
                for c in range(DC):
                    nc.scalar.copy(orow[0:1, c * 512:(c + 1) * 512], psd[c][0:1, :])
                pst = ps.tile([128, KC], f32, tag="psum", name="pst_mlp")
                row_to_pm(orow, pst, KC)
                stpm = tp_.tile([128, KC], bf, tag="stpm", bufs=2, name="stpm_mlp")
                nc.vector.tensor_copy(stpm[:], pst[:])
                nc.sync.dma_start(ar_in_s[:, :, 0], stpm[:])

            # ================= layer 0 (all per token half, so the first
            # attn AllReduce fires as early as possible) =================
            for k in range(KC):
                nc.sync.dma_start(aqw[:, k, :], W["aq0"][k])
                nc.sync.dma_start(avw[:, k, :], W["av0"][k])
            nc.sync.dma_start(bq_sb[:], W["bq0"][:])
            nc.sync.dma_start(bv_sb[:], W["bv0"][:])
            # all projection weight traffic lands before any AllReduce is
            # on the wire; the per-half attention (DMA-free) then overlaps
            # the attn AllReduces without SDMA contention
            for t in range(NT):
                norm_half(t)
                lora_down_half(avw, av_sb, t)
                lora_down_half(aqw, aq_sb, t)
                k_proj_half("wk0", kT, t)
                v_proj_half(0, t)
                q_proj_half("wq0", qT, t)
            for t in range(NT):
                attention_half(t)
                out_proj_half(0, t)
                allreduce1_half(t)
            for t in range(NT):
                add1_half(t)
                norm_half(t)
                mlp_gate_up_half(0, t)
                mlp_down_half(0, t)
                allreduce2_half(t)

            # ================= layer 1 (tail) =================
            for k in range(KC):
                nc.sync.dma_start(aqw[:, k, :], W["aq1"][k])
                nc.sync.dma_start(avw[:, k, :], W["av1"][k])
            nc.sync.dma_start(bq_sb[:], W["bq1"][:])
            nc.sync.dma_start(bv_sb[:], W["bv1"][:])
            for t in range(NT):
                add2_half(t)
                norm_half(t)
                lora_down_half(avw, av_sb, t)
                if t == NT - 1:
                    lora_down_half(aqw, aq_sb, t)
                k_proj_half("wk1", kT, t)
                v_proj_half(1, t)
            q_tail()
            attention_tail()
            wo_tail()
            allreduce_and_add_tail()
            mlp_tail()
            allreduce_and_add_tail()

            # ================= final norm + head (last token only) ========
            sq = tp_.tile([128, KC, 1], bf, tag="sqlast")
            nc.scalar.activation(sq[:], h[:, :, S - 1:S],
                                 mybir.ActivationFunctionType.Square)
            psl = ps.tile([128, 1], f32, tag="psum", name="psl_final")
            for k in range(KC):
                nc.tensor.matmul(psl[:], oneD128[:], sq[:, k, :],
                                 start=(k == 0), stop=(k == KC - 1))
            rsl = tp_.tile([128, 1], f32, tag="rsl_final")
            nc.scalar.activation(rsl[:], psl[:],
                                 mybir.ActivationFunctionType.Sqrt, bias=eps128[:])
            nc.vector.reciprocal_approx_fast(rsl[:], rsl[:])
            hl = tp_.tile([128, KC, 1], bf, tag="hlast")
            nc.vector.tensor_scalar_mul(hl[:], h[:, :, S - 1:S], rsl[:])
            pso = ps.tile([128, TT], f32, tag="psum", name="ps_head")
            for k in range(KC):
                nc.tensor.matmul(pso[0:OUT, 0:1], wreg_sb[:, k, :], hl[:, k, :],
                                 start=(k == 0), stop=(k == KC - 1))
            ot = tp_.tile([OUT, 1], f32, tag="outt")
            nc.vector.tensor_add(ot[:], pso[0:OUT, 0:1], breg_sb[:])
            nc.sync.dma_start(out_dram[:], ot[:])

    nc.finalize()
    return nc


_CACHED = {}


def _get_program():
    if "nc" not in _CACHED:
        _CACHED["nc"] = build_program()
    return _CACHED["nc"]


def _host_prepare(inputs):
    """Fold norms/scales into weights, gather embeddings, build the 8
    per-core input maps."""
    ids = np.asarray(inputs["input_ids"]).astype(np.int64)        # [B,S]
    amask = np.asarray(inputs["attention_mask"]).astype(np.int64)  # [B,S]
    embed = np.asarray(inputs["embed"], FP32)

    inv_sqrt_hd = FP32(1.0 / np.sqrt(HD))

    # rope tables (half: both halves identical)
    inv = 1.0 / (10000.0 ** (np.arange(0, HD, 2, dtype=np.float64) / HD))
    ang = (np.arange(S, dtype=np.float64)[:, None] * inv[None, :])  # [S,64]
    cos64 = np.cos(ang).T.astype(BF16)  # [64,S]
    sin64 = np.sin(ang).T.astype(BF16)
    cosT = np.concatenate([cos64, cos64], axis=0).copy()  # [128,S]
    sinT = np.concatenate([sin64, sin64], axis=0).copy()

    # causal strip [128, 896]: strip[p,u] = 1 if (u-384) >= p else 0
    u = np.arange(896)[None, :]
    p = np.arange(128)[:, None]
    mstrip = ((u - 384) >= p).astype(BF16)

    common = {}

    def fold(l):
        g1 = np.asarray(inputs["norm1"], FP32)[l][:, None]
        g2 = np.asarray(inputs["norm2"], FP32)[l][:, None]
        wq = np.asarray(inputs["Wq"], FP32)[l] * g1 * inv_sqrt_hd
        wk = np.asarray(inputs["Wk"], FP32)[l] * g1
        wv = np.asarray(inputs["Wv"], FP32)[l] * g1
        aq = np.asarray(inputs["Aq"], FP32)[l] * g1
        av = np.asarray(inputs["Av"], FP32)[l] * g1
        bq = np.asarray(inputs["Bq"], FP32)[l] * (SCALING * inv_sqrt_hd)
        bv = np.asarray(inputs["Bv"], FP32)[l] * SCALING
        wo = np.asarray(inputs["Wo"], FP32)[l]
        wg = np.asarray(inputs["Wgate"], FP32)[l] * g2
        wu = np.asarray(inputs["Wup"], FP32)[l] * g2
        wd = np.asarray(inputs["Wdown"], FP32)[l]
        return wq, wk, wv, aq, av, bq, bv, wo, wg, wu, wd

    folded = [fold(l) for l in range(L)]
    wregf = (np.asarray(inputs["Wreg"], FP32) * np.asarray(inputs["norm_f"], FP32)[:, None])
    common["wreg"] = wregf.reshape(KC, 128, OUT).astype(BF16)
    common["breg"] = np.asarray(inputs["breg"], FP32).reshape(OUT, 1)
    common["cosT"] = cosT
    common["sinT"] = sinT
    common["mstrip"] = mstrip

    in_maps = []
    for c in range(N_CORES):
        b = c // TP      # batch index (DP group)
        r = c % TP       # TP rank within group
        m = dict(common)
        # embedding gather, transposed, bf16: [D,S] -> [16,128,S] -> [128,16,S]
        xt = embed[ids[b]].T.reshape(KC, 128, S).transpose(1, 0, 2)
        m["xT"] = np.ascontiguousarray(xt).astype(BF16)
        # attention_mask bias [128, TC]: col j, part p -> key token 128j+p
        mb = np.where(amask[b] > 0, FP32(0), FP32(-1e9)).reshape(TC, 128).T
        m["maskbias"] = np.ascontiguousarray(mb)
        m["mask01"] = np.ascontiguousarray((amask[b] > 0).reshape(TC, 128).T).astype(FP32)
        for l in range(L):
            wq, wk, wv, aq, av, bq, bv, wo, wg, wu, wd = folded[l]
            dsl = slice(r * DL, (r + 1) * DL)
            fsl = slice(r * FL, (r + 1) * FL)
            m[f"wk{l}"] = np.ascontiguousarray(wk[:, dsl].reshape(KC, 128, DL)).astype(BF16)
            m[f"wv{l}"] = np.ascontiguousarray(wv[:, dsl].reshape(KC, 128, DL)).astype(BF16)
            m[f"aq{l}"] = np.ascontiguousarray(aq.reshape(KC, 128, R)).astype(BF16)
            m[f"av{l}"] = np.ascontiguousarray(av.reshape(KC, 128, R)).astype(BF16)
            m[f"bq{l}"] = np.ascontiguousarray(bq[:, dsl]).astype(BF16)
            m[f"bv{l}"] = np.ascontiguousarray(bv[:, dsl]).astype(BF16)
            m[f"wq{l}"] = np.ascontiguousarray(wq[:, dsl].reshape(KC, 128, DL)).astype(BF16)
            m[f"wo{l}"] = np.ascontiguousarray(wo[dsl].reshape(NH, 128, D)).astype(BF16)
            if l == 0:
                m["wd0"] = np.ascontiguousarray(wd[fsl].reshape(FC, 128, D)).astype(BF16)
                # gate|up interleaved, fc-major, contiguous per partition
                wg4 = wg[:, fsl].reshape(KC, 128, FC, 128).transpose(2, 1, 0, 3)
                wu4 = wu[:, fsl].reshape(KC, 128, FC, 128).transpose(2, 1, 0, 3)
                wgu = np.concatenate([wg4, wu4], axis=-1)  # [FC,128,KC,256]
                m["wgu0"] = np.ascontiguousarray(wgu).astype(BF16)
            else:
                wg_r = wg[:, fsl].reshape(KC, 128, FL)
                wu_r = wu[:, fsl].reshape(KC, 128, FL)
                m["wgu1"] = np.ascontiguousarray(
                    np.concatenate([wg_r, wu_r], axis=-1)).astype(BF16)
                m["wd1"] = np.ascontiguousarray(
                    wd[fsl].reshape(FC, 128, D)).astype(BF16)
        in_maps.append(m)
    return in_maps


def run_on_device(in_maps, trace=False):
    nc = _get_program()
    return bass_utils.run_bass_kernel_spmd(
        nc, in_maps, core_ids=list(range(N_CORES)), trace=trace,
    )


def kernel(**inputs):
    in_maps = _host_prepare(inputs)
    res = run_on_device(in_maps, trace=False)
    out = np.stack([
        res.results[0]["out"].reshape(OUT),
        res.results[TP]["out"].reshape(OUT),
    ]).astype(FP32)
    return out


# revision 47
# speedup vs baseline: 1.0031x; 1.0031x over previous
"""Trainium2 Bass kernel for nn_LlamaForSequenceRegression_14336600834254.

2-layer Llama (D=2048, H=16, HD=128, F=5632, LoRA r=16 on q/v) + regression
head, B=2, S=1024, fp32 reference.

Distribution (8 NeuronCores): DP2 x TP4.
  - cores 0-3 process batch 0, cores 4-7 batch 1 (data parallel).
  - within each group of 4: Megatron tensor parallel — Wq/Wk/Wv column
    shards (4 heads/core), Wo row shards, Wgate/Wup column shards
    (F/4=1408), Wdown row shards. AllReduce (bf16) after attn-out and
    after MLP-down, replica_groups=[[0,1,2,3],[4,5,6,7]].
  - embedding gather + norm-weight folding are done host-side; all
    device matmuls run in bf16 with fp32 PSUM accumulation; the
    residual stream / softmax / rmsnorm statistics are fp32.

Layout: activations are kept feature-major ("transposed"): h^T [D, T] as
SBUF tiles [128 part, 16 kchunk, 1024 tok] so every weight matmul uses the
natural [in, out] weight layout as lhsT and no transposes are needed.
Attention uses scores^T [Tk, Tq] so softmax needs no max-subtraction
(|scores| < ~6 with folded 1/sqrt(HD)) and probs feed the v-matmul
directly; the causal mask is an upload-once 0/1 strip multiplied into the
diagonal tiles, and the attention_mask rides the exp() per-partition bias.

Perf structure (v2):
  - attention / out_proj / AllReduce pipelined per token half so the attn
    AllReduce overlaps the other half's attention + the first MLP half.
  - partition reductions (rmsnorm sum-of-squares, softmax denominators)
    use an all-ones [128,128] stationary operand so the PSUM result is
    already broadcast across partitions: no M=1 matmuls, no
    single-partition reciprocals, no gpsimd broadcasts.
  - last layer runs in "tail" mode: only the last token flows through
    q/attention/Wo/MLP. Those matvecs are transposed — the activation
    vector is the stationary operand and the weights stream through the
    PE as the moving operand — so the tail is weight-DMA bound instead
    of LDWEIGHTS bound.
"""

import numpy as np
import ml_dtypes

import concourse.bacc as bacc
import concourse.tile as tile
from concourse import mybir
from concourse import bass_utils

BF16 = ml_dtypes.bfloat16
FP32 = np.float32

V, D, L, H, HD, F, R, ALPHA, B, S, OUT = 32000, 2048, 2, 16, 128, 5632, 16, 32, 2, 1024, 11
EPS = 1e-5
SCALING = ALPHA / R
N_CORES = 8
TP = 4
NH = H // TP          # 4 local heads
DL = NH * HD          # 512 local q/k/v cols
FL = F // TP          # 1408 local mlp cols
KC = D // 128         # 16 contraction chunks
FC = FL // 128        # 11 mlp chunks
TT = 512              # token tile (free dim per matmul)
NT = S // TT          # 2 token tiles
TC = S // 128         # 8 token chunks (128-wide)
DC = D // TT          # 4 output chunks of 512
FLCH = [(0, 512), (512, 512), (1024, 384)]  # FL split into <=512 chunks
REPLICA_GROUPS = [[0, 1, 2, 3], [4, 5, 6, 7]]

dt = mybir.dt


def build_program():
    """Build the SPMD Bass program (identical on all 8 cores; weights differ
    per core via the input maps)."""
    nc = bacc.Bacc(num_devices=N_CORES, debug=False)

    # ---- DRAM I/O ----
    xT = nc.dram_tensor("xT", [128, KC, S], dt.bfloat16, kind="ExternalInput")
    cosT = nc.dram_tensor("cosT", [128, S], dt.bfloat16, kind="ExternalInput")
    sinT = nc.dram_tensor("sinT", [128, S], dt.bfloat16, kind="ExternalInput")
    mstrip = nc.dram_tensor("mstrip", [128, 896], dt.bfloat16, kind="ExternalInput")
    maskbias = nc.dram_tensor("maskbias", [128, TC], dt.float32, kind="ExternalInput")
    mask01 = nc.dram_tensor("mask01", [128, TC], dt.float32, kind="ExternalInput")
    wreg = nc.dram_tensor("wreg", [KC, 128, OUT], dt.bfloat16, kind="ExternalInput")
    breg = nc.dram_tensor("breg", [OUT, 1], dt.float32, kind="ExternalInput")
    W = {}
    for l in range(L):
        W[f"wk{l}"] = nc.dram_tensor(f"wk{l}", [KC, 128, DL], dt.bfloat16, kind="ExternalInput")
        W[f"wv{l}"] = nc.dram_tensor(f"wv{l}", [KC, 128, DL], dt.bfloat16, kind="ExternalInput")
        W[f"aq{l}"] = nc.dram_tensor(f"aq{l}", [KC, 128, R], dt.bfloat16, kind="ExternalInput")
        W[f"av{l}"] = nc.dram_tensor(f"av{l}", [KC, 128, R], dt.bfloat16, kind="ExternalInput")
        W[f"bq{l}"] = nc.dram_tensor(f"bq{l}", [R, DL], dt.bfloat16, kind="ExternalInput")
        W[f"bv{l}"] = nc.dram_tensor(f"bv{l}", [R, DL], dt.bfloat16, kind="ExternalInput")
    # layer 0 (full-sequence Megatron TP layouts)
    W["wq0"] = nc.dram_tensor("wq0", [KC, 128, DL], dt.bfloat16, kind="ExternalInput")
    W["wo0"] = nc.dram_tensor("wo0", [NH, 128, D], dt.bfloat16, kind="ExternalInput")
    # gate|up interleaved per fc chunk: [fc][p][k*256 + (0:128 gate | 128:256 up)]
    W["wgu0"] = nc.dram_tensor("wgu0", [FC, 128, KC, 256], dt.bfloat16, kind="ExternalInput")
    W["wd0"] = nc.dram_tensor("wd0", [FC, 128, D], dt.bfloat16, kind="ExternalInput")
    # layer 1 (tail: weights stream as moving operand)
    W["wq1"] = nc.dram_tensor("wq1", [KC, 128, DL], dt.bfloat16, kind="ExternalInput")
    W["wo1"] = nc.dram_tensor("wo1", [NH, 128, D], dt.bfloat16, kind="ExternalInput")
    W["wgu1"] = nc.dram_tensor("wgu1", [KC, 128, 2 * FL], dt.bfloat16, kind="ExternalInput")
    W["wd1"] = nc.dram_tensor("wd1", [FC, 128, D], dt.bfloat16, kind="ExternalInput")
    out_dram = nc.dram_tensor("out", [OUT, 1], dt.float32, kind="ExternalOutput")

    with tile.TileContext(nc) as tc:
        with (
            tc.tile_pool(name="persist", bufs=1) as pp,
            tc.tile_pool(name="wts", bufs=3) as wp,
            tc.tile_pool(name="colw", bufs=4) as cwp,
            tc.tile_pool(name="tails", bufs=2) as tsp,
            tc.tile_pool(name="tmp", bufs=3) as tp_,
            tc.tile_pool(name="stage", bufs=2) as stp,
            tc.tile_pool(name="psum", bufs=8, space="PSUM") as ps,
            tc.tile_pool(name="dram", bufs=1, space="DRAM") as dram,
        ):
            f32, bf = dt.float32, dt.bfloat16
            # ---- persistent tiles ----
            h = pp.tile([128, KC, S], f32, tag="h")
            hn = pp.tile([128, KC, S], bf, tag="hn")
            cos_sb = pp.tile([128, S], bf, tag="cos")
            sin_sb = pp.tile([128, S], bf, tag="sin")
            mstrip_sb = pp.tile([128, 896], bf, tag="mstrip")
            mb_sb = pp.tile([128, TC], f32, tag="mb")
            m01_sb = pp.tile([128, TC], f32, tag="m01")
            cos_last = pp.tile([128, 1], f32, tag="cos_last")
            sin_last = pp.tile([128, 1], f32, tag="sin_last")
            ones_bf = pp.tile([128, 1], bf, tag="onesbf")
            allones = pp.tile([128, 128], bf, tag="allones")
            oneD128 = pp.tile([128, 128], bf, tag="oneD128")
            one1_bf = pp.tile([1, 1], bf, tag="one1")
            one64_bf = pp.tile([1, 1], bf, tag="one64")
            eps128 = pp.tile([128, 1], f32, tag="eps128")
            eps1 = pp.tile([1, 1], f32, tag="eps1")
            qT = pp.tile([128, NH, S], bf, tag="qT")     # q, then reused for ctx
            kT = pp.tile([128, NH, S], bf, tag="kT")
            vN = pp.tile([128, TC, DL], bf, tag="vN")
            expT = pp.tile([128, TC, TT], bf, tag="expT")
            mT = pp.tile([128, FC, TT], bf, tag="mT")    # per-half mlp act
            aqw = pp.tile([128, KC, R], bf, tag="aqw")
            avw = pp.tile([128, KC, R], bf, tag="avw")
            bq_sb = pp.tile([R, DL], bf, tag="bq")
            bv_sb = pp.tile([R, DL], bf, tag="bv")
            aq_sb = pp.tile([R, S], bf, tag="aq")
            av_sb = pp.tile([R, S], bf, tag="av")
            wreg_sb = pp.tile([128, KC, OUT], bf, tag="wreg")
            breg_sb = pp.tile([OUT, 1], f32, tag="breg")
            # tail smalls
            hn_last = pp.tile([128, KC, 1], bf, tag="hn_last")
            q_last = pp.tile([128, NH], bf, tag="q_last")
            ctx_n = pp.tile([128, NH], bf, tag="ctx_n")
            exp_tail = pp.tile([128, NH, TC], bf, tag="exp_tail")
            m_pm = pp.tile([128, FC], bf, tag="m_pm")

            # ---- constants in ----
            nc.vector.memset(ones_bf[:], 1.0)
            nc.vector.memset(allones[:], 1.0)
            nc.vector.memset(oneD128[:], 1.0 / D)
            nc.vector.memset(one1_bf[:], 1.0)
            nc.vector.memset(one64_bf[:], 1.0 / 64.0)
            nc.vector.memset(eps128[:], EPS)
            nc.vector.memset(eps1[:], EPS)
            nc.sync.dma_start(cos_sb[:], cosT[:])
            nc.sync.dma_start(sin_sb[:], sinT[:])
            nc.sync.dma_start(mstrip_sb[:], mstrip[:])
            nc.sync.dma_start(mb_sb[:], maskbias[:])
            nc.sync.dma_start(m01_sb[:], mask01[:])
            nc.scalar.copy(cos_last[:], cos_sb[:, S - 1:S])
            nc.scalar.copy(sin_last[:], sin_sb[:, S - 1:S])
            nc.sync.dma_start(breg_sb[:], breg[:])
            for k in range(KC):
                nc.sync.dma_start(wreg_sb[:, k, :], wreg[k])

            # ---- h init per half: bf16 upload -> fp32 residual ----
            for t in range(NT):
                ts_ = slice(t * TT, (t + 1) * TT)
                nc.sync.dma_start(hn[:, :, ts_], xT[:, :, ts_])
                nc.vector.tensor_copy(h[:, :, ts_], hn[:, :, ts_])

            # DRAM bounce buffers for collectives
            ar1h_in = [dram.tile([128, KC, TT], bf, name=f"ar1hi_{t}") for t in range(NT)]
            ar1h_out = [dram.tile([128, KC, TT], bf, name=f"ar1ho_{t}") for t in range(NT)]
            ar2h_in = [dram.tile([128, KC, TT], bf, name=f"ar2hi_{t}") for t in range(NT)]
            ar2h_out = [dram.tile([128, KC, TT], bf, name=f"ar2ho_{t}") for t in range(NT)]
            ar_in_s = dram.tile([128, KC, 1], bf)
            ar_out_s = dram.tile([128, KC, 1], bf)

            def norm_half(t):
                """hn[:, :, half t] = h / sqrt(mean(h^2) + eps), bf16."""
                ts_ = slice(t * TT, (t + 1) * TT)
                psb = ps.tile([128, TT], f32, tag="psum", name=f"nps_{t}")
                for k in range(KC):
                    sq = tp_.tile([128, TT], bf, tag="sq", bufs=2, name=f"nsq_{k}_{t}")
                    nc.scalar.activation(sq[:], h[:, k, ts_],
                                         mybir.ActivationFunctionType.Square)
                    # all-ones/D stationary => result broadcast to all partitions
                    nc.tensor.matmul(psb[:], oneD128[:], sq[:],
                                     start=(k == 0), stop=(k == KC - 1))
                rs = tp_.tile([128, TT], f32, tag="rsbc", bufs=1, name=f"nrs_{t}")
                nc.scalar.activation(rs[:], psb[:],
                                     mybir.ActivationFunctionType.Sqrt, bias=eps128[:])
                nc.vector.reciprocal_approx_fast(rs[:], rs[:])
                for k in range(KC):
                    nc.vector.tensor_mul(hn[:, k, ts_], h[:, k, ts_], rs[:])

            def lora_down_half(aw, dst, t):
                psa = ps.tile([128, TT], f32, tag="psum", name=f"ldh_{id(aw)}_{t}")
                for k in range(KC):
                    nc.tensor.matmul(
                        psa[0:R, :], aw[:, k, :], hn[:, k, t * TT:(t + 1) * TT],
                        start=(k == 0), stop=(k == KC - 1),
                    )
                nc.scalar.copy(dst[:, t * TT:(t + 1) * TT], psa[0:R, :])

            def rope_from_psum(psq, dst, hc, t):
                """Apply RoPE to psum [128,TT] (one head, token tile t) and
                write bf16 into dst[:, hc, t*TT:...]."""
                ts_ = slice(t * TT, (t + 1) * TT)
                t2 = tp_.tile([128, TT], bf, tag="ropetB", bufs=1)
                t4 = tp_.tile([128, TT], bf, tag="ropetB", bufs=1)
                nc.vector.tensor_mul(dst[0:64, hc, ts_], psq[0:64, :], cos_sb[0:64, ts_])
                nc.vector.tensor_mul(t2[0:64, :], psq[64:128, :], sin_sb[0:64, ts_])
                nc.vector.tensor_sub(dst[0:64, hc, ts_], dst[0:64, hc, ts_], t2[0:64, :])
                nc.vector.tensor_mul(dst[64:128, hc, ts_], psq[64:128, :], cos_sb[64:128, ts_])
                nc.vector.tensor_mul(t4[64:128, :], psq[0:64, :], sin_sb[64:128, ts_])
                nc.vector.tensor_add(dst[64:128, hc, ts_], dst[64:128, hc, ts_], t4[64:128, :])

            def qk_proj(wname, dst, lora_bw, lora_act):
                """dst[:, hc, :] (bf16, roped) = rope(W.T @ hn [+ lora])."""
                psq = [[ps.tile([128, TT], f32, tag="psum", name=f"psq_{wname}_{hc}_{t}")
                        for t in range(NT)] for hc in range(NH)]
                for kk in range(KC // 2):
                    wt = wp.tile([128, 2, DL], bf, tag="wqkv", name=f"w_{wname}_{kk}")
                    nc.sync.dma_start(wt[:], W[wname][2 * kk:2 * kk + 2].rearrange("i p c -> p i c"))
                    for i in range(2):
                        k = 2 * kk + i
                        for hc in range(NH):
                            for t in range(NT):
                                nc.tensor.matmul(
                                    psq[hc][t][:], wt[:, i, hc * HD:(hc + 1) * HD],
                                    hn[:, k, t * TT:(t + 1) * TT],
                                    start=(k == 0),
                                    stop=(lora_bw is None and k == KC - 1),
                                )
                for hc in range(NH):
                    for t in range(NT):
                        if lora_bw is not None:
                            nc.tensor.matmul(
                                psq[hc][t][:], lora_bw[:, hc * HD:(hc + 1) * HD],
                                lora_act[:, t * TT:(t + 1) * TT],
                                start=False, stop=True,
                            )
                        rope_from_psum(psq[hc][t], dst, hc, t)

            def v_proj(l):
                """vN [128(tok), TC, DL] bf16 = hn.T @ Wv + lora."""
                psv = [ps.tile([128, DL], f32, tag="psum", name=f"psv_{c}")
                       for c in range(TC)]
                for kk in range(KC // 2):
                    wt = wp.tile([128, 2, DL], bf, tag="wqkv", name=f"wv_t_{kk}")
                    nc.sync.dma_start(wt[:], W[f"wv{l}"][2 * kk:2 * kk + 2].rearrange("i p c -> p i c"))
                    for i in range(2):
                        k = 2 * kk + i
                        for c in range(TC):
                            nc.tensor.matmul(
                                psv[c][:], hn[:, k, c * 128:(c + 1) * 128], wt[:, i, :],
                                start=(k == 0), stop=False,
                            )
                for c in range(TC):
                    nc.tensor.matmul(
                        psv[c][:], av_sb[:, c * 128:(c + 1) * 128], bv_sb[:],
                        start=False, stop=True,
                    )
                    nc.scalar.copy(vN[:, c, :], psv[c][:])

            def q_proj_half(wname, dst, t, pool=None, tag="wqkv"):
                """dst[:, hc, half t] = rope(Wq.T @ hn + lora)."""
                pool = pool or wp
                psq = [ps.tile([128, TT], f32, tag="psum", name=f"psqq_{hc}_{t}")
                       for hc in range(NH)]
                for kk in range(KC // 2):
                    wt = pool.tile([128, 2, DL], bf, tag=tag, name=f"wq_{kk}_{t}")
                    nc.sync.dma_start(wt[:], W[wname][2 * kk:2 * kk + 2].rearrange("i p c -> p i c"))
                    for i in range(2):
                        k = 2 * kk + i
                        for hc in range(NH):
                            nc.tensor.matmul(
                                psq[hc][:], wt[:, i, hc * HD:(hc + 1) * HD],
                                hn[:, k, t * TT:(t + 1) * TT],
                                start=(k == 0), stop=False,
                            )
                for hc in range(NH):
                    nc.tensor.matmul(
                        psq[hc][:], bq_sb[:, hc * HD:(hc + 1) * HD],
                        aq_sb[:, t * TT:(t + 1) * TT],
                        start=False, stop=True,
                    )
                    rope_from_psum(psq[hc], dst, hc, t)

            def attention_half(t):
                """qT,kT,vN -> ctx (written into qT) for token half t.

                The denominator/ctx accumulation matmuls for chunk j are
                emitted two chunks behind the QK+exp chain, so by the time
                the PE (strict in-order queue) reaches them their exp input
                has drained from the ACT engine and nothing head-of-line
                blocks."""
                ts_ = slice(t * TT, (t + 1) * TT)
                jmax = (t + 1) * (TT // 128)
                LAG = 2
                for hh in range(NH):
                    psb = ps.tile([128, TT], f32, tag="psum", name=f"psd_{hh}_{t}")
                    psc = ps.tile([128, TT], f32, tag="psum", name=f"psc_{hh}_{t}")

                    def emit_reduce_j(j):
                        nc.tensor.matmul(
                            psb[:], allones[:], expT[:, j, :],
                            start=(j == 0), stop=(j == jmax - 1),
                        )
                        nc.tensor.matmul(
                            psc[:], vN[:, j, hh * HD:(hh + 1) * HD],
                            expT[:, j, :],
                            start=(j == 0), stop=(j == jmax - 1),
                        )

                    for j in range(jmax):
                        pss = ps.tile([128, TT], f32, tag="psum",
                                      name=f"pss_{hh}_{t}_{j}")
                        nc.tensor.matmul(
                            pss[:], kT[:, hh, j * 128:(j + 1) * 128],
                            qT[:, hh, ts_], start=True, stop=True,
                        )
                        nc.scalar.activation(
                            expT[:, j, :], pss[:],
                            mybir.ActivationFunctionType.Exp,
                            bias=mb_sb[:, j:j + 1], scale=1.0,
                        )
                        off = t * TT - j * 128
                        if off < 128:
                            nc.vector.tensor_mul(
                                expT[:, j, :], expT[:, j, :],
                                mstrip_sb[:, 384 + off:896 + off],
                            )
                        if j >= LAG:
                            emit_reduce_j(j - LAG)
                    for j in range(max(0, jmax - LAG), jmax):
                        emit_reduce_j(j)
                    rden = tp_.tile([128, TT], f32, tag="rsbc", bufs=1,
                                    name=f"rden_{hh}_{t}")
                    nc.vector.reciprocal_approx_fast(rden[:], psb[:])
                    nc.vector.tensor_mul(qT[:, hh, ts_], psc[:], rden[:])

            def out_proj_half(l, t):
                """attn partial for token half t -> ar1h_in[t]."""
                for og in range(4):  # groups of 4 output chunks
                    pso = [ps.tile([128, TT], f32, tag="psum", name=f"pso_{og}_{oi}_{t}")
                           for oi in range(4)]
                    for hp in range(NH // 2):
                        wt = wp.tile([128, 2, TT], bf, tag="wqkv",
                                     name=f"wo_t_{og}_{hp}_{t}")
                        nc.sync.dma_start(
                            wt[:], W[f"wo{l}"][2 * hp:2 * hp + 2,
                                               :, og * 512:(og + 1) * 512]
                            .rearrange("i p c -> p i c"))
                        for i in range(2):
                            hc = 2 * hp + i
                            for oi in range(4):
                                nc.tensor.matmul(
                                    pso[oi][:], wt[:, i, oi * 128:(oi + 1) * 128],
                                    qT[:, hc, t * TT:(t + 1) * TT],
                                    start=(hc == 0), stop=(hc == NH - 1),
                                )
                    for oi in range(4):
                        st = stp.tile([128, TT], bf, tag="stage")
                        nc.vector.tensor_copy(st[:], pso[oi][:])
                        nc.sync.dma_start(ar1h_in[t][:, og * 4 + oi, :], st[:])

            def allreduce1_half(t):
                nc.gpsimd.collective_compute(
                    "AllReduce", mybir.AluOpType.add,
                    replica_groups=REPLICA_GROUPS,
                    ins=[ar1h_in[t].opt()], outs=[ar1h_out[t].opt()],
                )

            def add1_half(t):
                # landing DMA rides the scalar HWDGE ring so its AR wait can't
                # head-of-line-block weight loads on the sync ring
                ts_ = slice(t * TT, (t + 1) * TT)
                nc.scalar.dma_start(hn[:, :, ts_], ar1h_out[t][:])
                for k in range(KC):
                    nc.vector.tensor_add(h[:, k, ts_], h[:, k, ts_], hn[:, k, ts_])

            def allreduce2_half(t):
                nc.gpsimd.collective_compute(
                    "AllReduce", mybir.AluOpType.add,
                    replica_groups=REPLICA_GROUPS,
                    ins=[ar2h_in[t].opt()], outs=[ar2h_out[t].opt()],
                )

            def add2_half(t):
                ts_ = slice(t * TT, (t + 1) * TT)
                nc.scalar.dma_start(hn[:, :, ts_], ar2h_out[t][:])
                for k in range(KC):
                    nc.vector.tensor_add(h[:, k, ts_], h[:, k, ts_], hn[:, k, ts_])

            def k_proj_half(wname, dst, t, pool=None, tag="wqkv"):
                pool = pool or wp
                psq = [ps.tile([128, TT], f32, tag="psum", name=f"psqh_{hc}_{t}")
                       for hc in range(NH)]
                for kk in range(KC // 2):
                    wt = pool.tile([128, 2, DL], bf, tag=tag, name=f"wkh_{wname}_{kk}_{t}")
                    nc.sync.dma_start(wt[:], W[wname][2 * kk:2 * kk + 2].rearrange("i p c -> p i c"))
                    for i in range(2):
                        k = 2 * kk + i
                        for hc in range(NH):
                            nc.tensor.matmul(
                                psq[hc][:], wt[:, i, hc * HD:(hc + 1) * HD],
                                hn[:, k, t * TT:(t + 1) * TT],
                                start=(k == 0), stop=(k == KC - 1),
                            )
                for hc in range(NH):
                    rope_from_psum(psq[hc], dst, hc, t)

            def v_proj_half(l, t, pool=None, tag="wqkv"):
                pool = pool or wp
                psv = [ps.tile([128, DL], f32, tag="psum", name=f"psvh_{c}_{t}")
                       for c in range(4)]
                for kk in range(KC // 2):
                    wt = pool.tile([128, 2, DL], bf, tag=tag, name=f"wvh_{l}_{kk}_{t}")
                    nc.sync.dma_start(wt[:], W[f"wv{l}"][2 * kk:2 * kk + 2].rearrange("i p c -> p i c"))
                    for i in range(2):
                        k = 2 * kk + i
                        for ci in range(4):
                            c = t * 4 + ci
                            nc.tensor.matmul(
                                psv[ci][:], hn[:, k, c * 128:(c + 1) * 128], wt[:, i, :],
                                start=(k == 0), stop=False,
                            )
                for ci in range(4):
                    c = t * 4 + ci
                    nc.tensor.matmul(
                        psv[ci][:], av_sb[:, c * 128:(c + 1) * 128], bv_sb[:],
                        start=False, stop=True,
                    )
                    nc.scalar.copy(vN[:, c, :], psv[ci][:])

            def mlp_gate_up_half(l, t):
                ts_ = slice(t * TT, (t + 1) * TT)
                for fc in range(FC):
                    psg = ps.tile([128, TT], f32, tag="psum", name=f"psg_{fc}_{t}")
                    psu = ps.tile([128, TT], f32, tag="psum", name=f"psu_{fc}_{t}")
                    for kh in range(2):
                        wgu = cwp.tile([128, KC // 2, 256], bf, tag="wgu",
                                       name=f"wgu_{fc}_{kh}_{t}")
                        nc.sync.dma_start(
                            wgu[:], W[f"wgu{l}"][fc][:, kh * (KC // 2):(kh + 1) * (KC // 2), :])
                        for ki in range(KC // 2):
                            k = kh * (KC // 2) + ki
                            nc.tensor.matmul(psg[:], wgu[:, ki, 0:128], hn[:, k, ts_],
                                             start=(k == 0), stop=(k == KC - 1))
                            nc.tensor.matmul(psu[:], wgu[:, ki, 128:256], hn[:, k, ts_],
                                             start=(k == 0), stop=(k == KC - 1))
                    sg = tp_.tile([128, TT], bf, tag="silu", bufs=1, name=f"sg_{fc}_{t}")
                    nc.scalar.activation(sg[:], psg[:], mybir.ActivationFunctionType.Silu)
                    nc.vector.tensor_mul(mT[:, fc, :], sg[:], psu[:])

            def mlp_down_half(l, t):
                for og in range(4):
                    pso = [ps.tile([128, TT], f32, tag="psum", name=f"psd_{og}_{oi}_{t}")
                           for oi in range(4)]
                    for kp in range((FC + 1) // 2):
                        nk = min(2, FC - 2 * kp)
                        wt = wp.tile([128, 2, TT], bf, tag="wqkv",
                                     name=f"wd_t_{og}_{kp}_{t}")
                        nc.sync.dma_start(
                            wt[:, 0:nk, :],
                            W[f"wd{l}"][2 * kp:2 * kp + nk,
                                        :, og * 512:(og + 1) * 512]
                            .rearrange("i p c -> p i c"))
                        for i in range(nk):
                            kc = 2 * kp + i
                            for oi in range(4):
                                nc.tensor.matmul(
                                    pso[oi][:], wt[:, i, oi * 128:(oi + 1) * 128],
                                    mT[:, kc, :],
                                    start=(kc == 0), stop=(kc == FC - 1),
                                )
                    for oi in range(4):
                        st = stp.tile([128, TT], bf, tag="stage")
                        nc.vector.tensor_copy(st[:], pso[oi][:])
                        nc.sync.dma_start(ar2h_in[t][:, og * 4 + oi, :], st[:])

            # ---------- tail (last layer): only the last token flows through
            # q/attention/Wo/MLP. Matvecs are transposed: the activation
            # column is the stationary operand, weights stream as rhs.

            def row_to_pm(row_sb, psum_pm, ncols):
                """psum_pm[:, c] = row_sb[0, c*128:(c+1)*128].T via K=1 matmuls."""
                for c in range(ncols):
                    nc.tensor.matmul(
                        psum_pm[:, c:c + 1],
                        row_sb[0:1, c * 128:(c + 1) * 128], one1_bf[:],
                        start=True, stop=True,
                    )

            def q_tail():
                psq = ps.tile([128, TT], f32, tag="psum", name="psq_tail")
                for kk in range(KC // 2):
                    wt = wp.tile([128, 2, DL], bf, tag="wqkv", name=f"wq1s_{kk}")
                    nc.sync.dma_start(wt[:], W["wq1"][2 * kk:2 * kk + 2].rearrange("i p c -> p i c"))
                    for i in range(2):
                        k = 2 * kk + i
                        nc.tensor.matmul(psq[0:1, :], hn[:, k, S - 1:S], wt[:, i, :],
                                         start=(k == 0), stop=False)
                nc.tensor.matmul(psq[0:1, :], aq_sb[:, S - 1:S], bq_sb[:],
                                 start=False, stop=True)
                qrow = tp_.tile([1, DL], bf, tag="qrow", bufs=1)
                nc.scalar.copy(qrow[:], psq[0:1, :])
                psqpm = ps.tile([128, NH], f32, tag="psum", name="psqpm")
                row_to_pm(qrow, psqpm, NH)
                # rope, all heads at once (per-partition cos/sin scalars)
                t2 = tp_.tile([128, NH], bf, tag="ropeS", bufs=2)
                t4 = tp_.tile([128, NH], bf, tag="ropeS", bufs=2)
                nc.vector.tensor_scalar_mul(q_last[0:64, :], psqpm[0:64, :],
                                            cos_last[0:64, :])
                nc.vector.tensor_scalar_mul(t2[0:64, :], psqpm[64:128, :],
                                            sin_last[0:64, :])
                nc.vector.tensor_sub(q_last[0:64, :], q_last[0:64, :], t2[0:64, :])
                nc.vector.tensor_scalar_mul(q_last[64:128, :], psqpm[64:128, :],
                                            cos_last[64:128, :])
                nc.vector.tensor_scalar_mul(t4[64:128, :], psqpm[0:64, :],
                                            sin_last[64:128, :])
                nc.vector.tensor_add(q_last[64:128, :], q_last[64:128, :], t4[64:128, :])

            def attention_tail():
                # scores for all heads/key-chunks in one psum tile
                pss = ps.tile([128, NH, TC], f32, tag="psum", name="pss_tail")
                for hh in range(NH):
                    for j in range(TC):
                        nc.tensor.matmul(
                            pss[:, hh, j:j + 1],
                            kT[:, hh, j * 128:(j + 1) * 128], q_last[:, hh:hh + 1],
                            start=True, stop=True,
                        )
                nc.scalar.activation(exp_tail[:], pss[:],
                                     mybir.ActivationFunctionType.Exp)
                for j in range(TC):
                    nc.vector.tensor_scalar_mul(exp_tail[:, :, j], exp_tail[:, :, j],
                                                m01_sb[:, j:j + 1])
                # denominators: all-partition sums then free-dim reduce per head
                psdb = ps.tile([128, NH, TC], f32, tag="psum", name="psdb_tail")
                nc.tensor.matmul(psdb[:], allones[:], exp_tail[:],
                                 start=True, stop=True)
                rdt = tp_.tile([128, NH], f32, tag="rdt")
                nc.vector.tensor_reduce(
                    rdt[:], psdb[:],
                    axis=mybir.AxisListType.X, op=mybir.AluOpType.add,
                )
                nc.vector.reciprocal_approx_fast(rdt[:], rdt[:])
                psc = ps.tile([128, NH], f32, tag="psum", name="psc_tail")
                for hh in range(NH):
                    for j in range(TC):
                        nc.tensor.matmul(
                            psc[:, hh:hh + 1], vN[:, j, hh * HD:(hh + 1) * HD],
                            exp_tail[:, hh, j:j + 1],
                            start=(j == 0), stop=(j == TC - 1),
                        )
                nc.vector.tensor_mul(ctx_n[:], psc[:], rdt[:])

            def wo_tail():
                pso = [ps.tile([128, TT], f32, tag="psum", name=f"psot_{c}")
                       for c in range(DC)]
                for hc in range(NH):
                    for dh in range(2):
                        wt = tsp.tile([128, 1024], bf, tag="wod1s",
                                      name=f"wo1s_{hc}_{dh}")
                        nc.sync.dma_start(wt[:], W["wo1"][hc][:, dh * 1024:(dh + 1) * 1024])
                        for ci in range(2):
                            c = dh * 2 + ci
                            nc.tensor.matmul(
                                pso[c][0:1, :], ctx_n[:, hc:hc + 1],
                                wt[:, ci * 512:(ci + 1) * 512],
                                start=(hc == 0), stop=(hc == NH - 1),
                            )
                orow = tp_.tile([1, D], bf, tag="orow", bufs=1)
                for c in range(DC):
                    nc.scalar.copy(orow[0:1, c * 512:(c + 1) * 512], pso[c][0:1, :])
                pst = ps.tile([128, KC], f32, tag="psum", name="pst_wo")
                row_to_pm(orow, pst, KC)
                stpm = tp_.tile([128, KC], bf, tag="stpm", bufs=2, name="stpm_wo")
                nc.vector.tensor_copy(stpm[:], pst[:])
                nc.sync.dma_start(ar_in_s[:, :, 0], stpm[:])

            def allreduce_and_add_tail():
                nc.gpsimd.collective_compute(
                    "AllReduce", mybir.AluOpType.add,
                    replica_groups=REPLICA_GROUPS,
                    ins=[ar_in_s.opt()], outs=[ar_out_s.opt()],
                )
                lb = tp_.tile([128, KC, 1], bf, tag="ar_land", bufs=2)
                nc.scalar.dma_start(lb[:], ar_out_s[:])
                nc.vector.tensor_add(h[:, :, S - 1:S], h[:, :, S - 1:S], lb[:])

            def norm_tail_to_hn_last():
                sqt = tp_.tile([128, KC, 1], bf, tag="sqlast")
                nc.scalar.activation(sqt[:], h[:, :, S - 1:S],
                                     mybir.ActivationFunctionType.Square)
                psl = ps.tile([128, 1], f32, tag="psum", name="psl_normt")
                for k in range(KC):
                    nc.tensor.matmul(psl[:], oneD128[:], sqt[:, k, :],
                                     start=(k == 0), stop=(k == KC - 1))
                rsb = tp_.tile([128, 1], f32, tag="rsb_tail", bufs=2)
                nc.scalar.activation(rsb[:], psl[:],
                                     mybir.ActivationFunctionType.Sqrt, bias=eps128[:])
                nc.vector.reciprocal_approx_fast(rsb[:], rsb[:])
                nc.vector.tensor_scalar_mul(hn_last[:], h[:, :, S - 1:S], rsb[:])

            def mlp_tail():
                norm_tail_to_hn_last()
                # gate/up: stream combined gate|up weights as rhs
                psg = [ps.tile([128, TT], f32, tag="psum", name=f"psgt_{c}")
                       for c in range(len(FLCH))]
                psu = [ps.tile([128, TT], f32, tag="psum", name=f"psut_{c}")
                       for c in range(len(FLCH))]
                for k in range(KC):
                    wgt = tsp.tile([128, 2 * FL], bf, tag="wg1s", name=f"wg1s_{k}")
                    nc.sync.dma_start(wgt[:], W["wgu1"][k])
                    for c, (off, sz) in enumerate(FLCH):
                        nc.tensor.matmul(psg[c][0:1, 0:sz], hn_last[:, k, :],
                                         wgt[:, off:off + sz],
                                         start=(k == 0), stop=(k == KC - 1))
                        nc.tensor.matmul(psu[c][0:1, 0:sz], hn_last[:, k, :],
                                         wgt[:, FL + off:FL + off + sz],
                                         start=(k == 0), stop=(k == KC - 1))
                mrow = tp_.tile([1, FL], bf, tag="mrow", bufs=1)
                for c, (off, sz) in enumerate(FLCH):
                    grow = tp_.tile([1, TT], bf, tag="grow", bufs=1, name=f"grow_{c}")
                    nc.scalar.activation(grow[0:1, 0:sz], psg[c][0:1, 0:sz],
                                         mybir.ActivationFunctionType.Silu)
                    nc.vector.tensor_mul(mrow[0:1, off:off + sz], grow[0:1, 0:sz],
                                         psu[c][0:1, 0:sz])
                psm = ps.tile([128, FC], f32, tag="psum", name="psm_pm")
                row_to_pm(mrow, psm, FC)
                nc.vector.tensor_copy(m_pm[:], psm[:])
                # down
                psd = [ps.tile([128, TT], f32, tag="psum", name=f"psdt2_{c}")
                       for c in range(DC)]
                for fc in range(FC):
                    for dh in range(2):
                        wdt = tsp.tile([128, 1024], bf, tag="wod1s",
                                       name=f"wd1s_{fc}_{dh}")
                        nc.sync.dma_start(wdt[:],
                                          W["wd1"][fc][:, dh * 1024:(dh + 1) * 1024])
                        for ci in range(2):
                            c = dh * 2 + ci
                            nc.tensor.matmul(
                                psd[c][0:1, :], m_pm[:, fc:fc + 1],
                                wdt[:, ci * 512:(ci + 1) * 512],
                                start=(fc == 0), stop=(fc == FC - 1),
                            )
                orow = tp_.tile([1, D], bf, tag="orow", bufs=1)
                for c in range(DC):
                    nc.scalar.copy(orow[0:1, c * 512:(c + 1) * 512], psd[c][0:1, :])
                pst = ps.tile([128, KC], f32, tag="psum", name="pst_mlp")
                row_to_pm(orow, pst, KC)
                stpm = tp_.tile([128, KC], bf, tag="stpm", bufs=2, name="stpm_mlp")
                nc.vector.tensor_copy(stpm[:], pst[:])
                nc.sync.dma_start(ar_in_s[:, :, 0], stpm[:])

            # ================= layer 0 (all per token half, so the first
            # attn AllReduce fires as early as possible) =================
            for k in range(KC):
                nc.sync.dma_start(aqw[:, k, :], W["aq0"][k])
                nc.sync.dma_start(avw[:, k, :], W["av0"][k])
            nc.sync.dma_start(bq_sb[:], W["bq0"][:])
            nc.sync.dma_start(bv_sb[:], W["bv0"][:])
            for t in range(NT):
                norm_half(t)
                lora_down_half(avw, av_sb, t)
                lora_down_half(aqw, aq_sb, t)
                pl, tg = (cwp, "wgu") if t == 1 else (wp, "wqkv")
                k_proj_half("wk0", kT, t, pool=pl, tag=tg)
                v_proj_half(0, t, pool=pl, tag=tg)
                q_proj_half("wq0", qT, t, pool=pl, tag=tg)
                attention_half(t)
                out_proj_half(0, t)
                allreduce1_half(t)
            for t in range(NT):
                add1_half(t)
                norm_half(t)
                mlp_gate_up_half(0, t)
                mlp_down_half(0, t)
                allreduce2_half(t)

            # ================= layer 1 (tail) =================
            for k in range(KC):
                nc.sync.dma_start(aqw[:, k, :], W["aq1"][k])
                nc.sync.dma_start(avw[:, k, :], W["av1"][k])
            nc.sync.dma_start(bq_sb[:], W["bq1"][:])
            nc.sync.dma_start(bv_sb[:], W["bv1"][:])
            for t in range(NT):
                add2_half(t)
                norm_half(t)
                lora_down_half(avw, av_sb, t)
                if t == NT - 1:
                    lora_down_half(aqw, aq_sb, t)
                k_proj_half("wk1", kT, t)
                v_proj_half(1, t)
            q_tail()
            attention_tail()
            wo_tail()
            allreduce_and_add_tail()
            mlp_tail()
            allreduce_and_add_tail()

            # ================= final norm + head (last token only) ========
            sq = tp_.tile([128, KC, 1], bf, tag="sqlast")
            nc.scalar.activation(sq[:], h[:, :, S - 1:S],
                                 mybir.ActivationFunctionType.Square)
            psl = ps.tile([128, 1], f32, tag="psum", name="psl_final")
            for k in range(KC):
                nc.tensor.matmul(psl[:], oneD128[:], sq[:, k, :],
                                 start=(k == 0), stop=(k == KC - 1))
            rsl = tp_.tile([128, 1], f32, tag="rsl_final")
            nc.scalar.activation(rsl[:], psl[:],
                                 mybir.ActivationFunctionType.Sqrt, bias=eps128[:])
            nc.vector.reciprocal_approx_fast(rsl[:], rsl[:])
            hl = tp_.tile([128, KC, 1], bf, tag="hlast")
            nc.vector.tensor_scalar_mul(hl[:], h[:, :, S - 1:S], rsl[:])
            pso = ps.tile([128, TT], f32, tag="psum", name="ps_head")
            for k in range(KC):
                nc.tensor.matmul(pso[0:OUT, 0:1], wreg_sb[:, k, :], hl[:, k, :],
                                 start=(k == 0), stop=(k == KC - 1))
            ot = tp_.tile([OUT, 1], f32, tag="outt")
            nc.vector.tensor_add(ot[:], pso[0:OUT, 0:1], breg_sb[:])
            nc.sync.dma_start(out_dram[:], ot[:])

    nc.finalize()
    return nc


_CACHED = {}


def _get_program():
    if "nc" not in _CACHED:
        _CACHED["nc"] = build_program()
    return _CACHED["nc"]


def _host_prepare(inputs):
    """Fold norms/scales into weights, gather embeddings, build the 8
    per-core input maps."""
    ids = np.asarray(inputs["input_ids"]).astype(np.int64)        # [B,S]
    amask = np.asarray(inputs["attention_mask"]).astype(np.int64)  # [B,S]
    embed = np.asarray(inputs["embed"], FP32)

    inv_sqrt_hd = FP32(1.0 / np.sqrt(HD))

    # rope tables (half: both halves identical)
    inv = 1.0 / (10000.0 ** (np.arange(0, HD, 2, dtype=np.float64) / HD))
    ang = (np.arange(S, dtype=np.float64)[:, None] * inv[None, :])  # [S,64]
    cos64 = np.cos(ang).T.astype(BF16)  # [64,S]
    sin64 = np.sin(ang).T.astype(BF16)
    cosT = np.concatenate([cos64, cos64], axis=0).copy()  # [128,S]
    sinT = np.concatenate([sin64, sin64], axis=0).copy()

    # causal strip [128, 896]: strip[p,u] = 1 if (u-384) >= p else 0
    u = np.arange(896)[None, :]
    p = np.arange(128)[:, None]
    mstrip = ((u - 384) >= p).astype(BF16)

    common = {}

    def fold(l):
        g1 = np.asarray(inputs["norm1"], FP32)[l][:, None]
        g2 = np.asarray(inputs["norm2"], FP32)[l][:, None]
        wq = np.asarray(inputs["Wq"], FP32)[l] * g1 * inv_sqrt_hd
        wk = np.asarray(inputs["Wk"], FP32)[l] * g1
        wv = np.asarray(inputs["Wv"], FP32)[l] * g1
        aq = np.asarray(inputs["Aq"], FP32)[l] * g1
        av = np.asarray(inputs["Av"], FP32)[l] * g1
        bq = np.asarray(inputs["Bq"], FP32)[l] * (SCALING * inv_sqrt_hd)
        bv = np.asarray(inputs["Bv"], FP32)[l] * SCALING
        wo = np.asarray(inputs["Wo"], FP32)[l]
        wg = np.asarray(inputs["Wgate"], FP32)[l] * g2
        wu = np.asarray(inputs["Wup"], FP32)[l] * g2
        wd = np.asarray(inputs["Wdown"], FP32)[l]
        return wq, wk, wv, aq, av, bq, bv, wo, wg, wu, wd

    folded = [fold(l) for l in range(L)]
    wregf = (np.asarray(inputs["Wreg"], FP32) * np.asarray(inputs["norm_f"], FP32)[:, None])
    common["wreg"] = wregf.reshape(KC, 128, OUT).astype(BF16)
    common["breg"] = np.asarray(inputs["breg"], FP32).reshape(OUT, 1)
    common["cosT"] = cosT
    common["sinT"] = sinT
    common["mstrip"] = mstrip

    in_maps = []
    for c in range(N_CORES):
        b = c // TP      # batch index (DP group)
        r = c % TP       # TP rank within group
        m = dict(common)
        # embedding gather, transposed, bf16: [D,S] -> [16,128,S] -> [128,16,S]
        xt = embed[ids[b]].T.reshape(KC, 128, S).transpose(1, 0, 2)
        m["xT"] = np.ascontiguousarray(xt).astype(BF16)
        # attention_mask bias [128, TC]: col j, part p -> key token 128j+p
        mb = np.where(amask[b] > 0, FP32(0), FP32(-1e9)).reshape(TC, 128).T
        m["maskbias"] = np.ascontiguousarray(mb)
        m["mask01"] = np.ascontiguousarray((amask[b] > 0).reshape(TC, 128).T).astype(FP32)
        for l in range(L):
            wq, wk, wv, aq, av, bq, bv, wo, wg, wu, wd = folded[l]
            dsl = slice(r * DL, (r + 1) * DL)
            fsl = slice(r * FL, (r + 1) * FL)
            m[f"wk{l}"] = np.ascontiguousarray(wk[:, dsl].reshape(KC, 128, DL)).astype(BF16)
            m[f"wv{l}"] = np.ascontiguousarray(wv[:, dsl].reshape(KC, 128, DL)).astype(BF16)
            m[f"aq{l}"] = np.ascontiguousarray(aq.reshape(KC, 128, R)).astype(BF16)
            m[f"av{l}"] = np.ascontiguousarray(av.reshape(KC, 128, R)).astype(BF16)
            m[f"bq{l}"] = np.ascontiguousarray(bq[:, dsl]).astype(BF16)
            m[f"bv{l}"] = np.ascontiguousarray(bv[:, dsl]).astype(BF16)
            m[f"wq{l}"] = np.ascontiguousarray(wq[:, dsl].reshape(KC, 128, DL)).astype(BF16)
            m[f"wo{l}"] = np.ascontiguousarray(wo[dsl].reshape(NH, 128, D)).astype(BF16)
            if l == 0:
                m["wd0"] = np.ascontiguousarray(wd[fsl].reshape(FC, 128, D)).astype(BF16)
                # gate|up interleaved, fc-major, contiguous per partition
                wg4 = wg[:, fsl].reshape(KC, 128, FC, 128).transpose(2, 1, 0, 3)
                wu4 = wu[:, fsl].reshape(KC, 128, FC, 128).transpose(2, 1, 0, 3)
                wgu = np.concatenate([wg4, wu4], axis=-1)  # [FC,128,KC,256]
                m["wgu0"] = np.ascontiguousarray(wgu).astype(BF16)
            else:
                wg_r = wg[:, fsl].reshape(KC, 128, FL)
                wu_r = wu[:, fsl].reshape(KC, 128, FL)
                m["wgu1"] = np.ascontiguousarray(
                    np.concatenate([wg_r, wu_r], axis=-1)).astype(BF16)
                m["wd1"] = np.ascontiguousarray(
                    wd[fsl].reshape(FC, 128, D)).astype(BF16)
        in_maps.append(m)
    return in_maps


def run_on_device(in_maps, trace=False):
    nc = _get_program()
    return bass_utils.run_bass_kernel_spmd(
        nc, in_maps, core_ids=list(range(N_CORES)), trace=trace,
    )


def kernel(**inputs):
    in_maps = _host_prepare(inputs)
    res = run_on_device(in_maps, trace=False)
    out = np.stack([
        res.results[0]["out"].reshape(OUT),
        res.results[TP]["out"].reshape(OUT),
    ]).astype(FP32)
    return out


# revision 48
# speedup vs baseline: 1.0247x; 1.0215x over previous
"""Trainium2 Bass kernel for nn_LlamaForSequenceRegression_14336600834254.

2-layer Llama (D=2048, H=16, HD=128, F=5632, LoRA r=16 on q/v) + regression
head, B=2, S=1024, fp32 reference.

Distribution (8 NeuronCores): DP2 x TP4.
  - cores 0-3 process batch 0, cores 4-7 batch 1 (data parallel).
  - within each group of 4: Megatron tensor parallel — Wq/Wk/Wv column
    shards (4 heads/core), Wo row shards, Wgate/Wup column shards
    (F/4=1408), Wdown row shards. AllReduce (bf16) after attn-out and
    after MLP-down, replica_groups=[[0,1,2,3],[4,5,6,7]].
  - embedding gather + norm-weight folding are done host-side; all
    device matmuls run in bf16 with fp32 PSUM accumulation; the
    residual stream / softmax / rmsnorm statistics are fp32.

Layout: activations are kept feature-major ("transposed"): h^T [D, T] as
SBUF tiles [128 part, 16 kchunk, 1024 tok] so every weight matmul uses the
natural [in, out] weight layout as lhsT and no transposes are needed.
Attention uses scores^T [Tk, Tq] so softmax needs no max-subtraction
(|scores| < ~6 with folded 1/sqrt(HD)) and probs feed the v-matmul
directly; the causal mask is an upload-once 0/1 strip multiplied into the
diagonal tiles, and the attention_mask rides the exp() per-partition bias.

Perf structure (v2):
  - attention / out_proj / AllReduce pipelined per token half so the attn
    AllReduce overlaps the other half's attention + the first MLP half.
  - partition reductions (rmsnorm sum-of-squares, softmax denominators)
    use an all-ones [128,128] stationary operand so the PSUM result is
    already broadcast across partitions: no M=1 matmuls, no
    single-partition reciprocals, no gpsimd broadcasts.
  - last layer runs in "tail" mode: only the last token flows through
    q/attention/Wo/MLP. Those matvecs are transposed — the activation
    vector is the stationary operand and the weights stream through the
    PE as the moving operand — so the tail is weight-DMA bound instead
    of LDWEIGHTS bound.
"""

import numpy as np
import ml_dtypes

import concourse.bacc as bacc
import concourse.tile as tile
from concourse import mybir
from concourse import bass_utils

BF16 = ml_dtypes.bfloat16
FP32 = np.float32

V, D, L, H, HD, F, R, ALPHA, B, S, OUT = 32000, 2048, 2, 16, 128, 5632, 16, 32, 2, 1024, 11
EPS = 1e-5
SCALING = ALPHA / R
N_CORES = 8
TP = 4
NH = H // TP          # 4 local heads
DL = NH * HD          # 512 local q/k/v cols
FL = F // TP          # 1408 local mlp cols
KC = D // 128         # 16 contraction chunks
FC = FL // 128        # 11 mlp chunks
TT = 512              # token tile (free dim per matmul)
NT = S // TT          # 2 token tiles
TC = S // 128         # 8 token chunks (128-wide)
DC = D // TT          # 4 output chunks of 512
FLCH = [(0, 512), (512, 512), (1024, 384)]  # FL split into <=512 chunks
REPLICA_GROUPS = [[0, 1, 2, 3], [4, 5, 6, 7]]

dt = mybir.dt


def build_program():
    """Build the SPMD Bass program (identical on all 8 cores; weights differ
    per core via the input maps)."""
    nc = bacc.Bacc(num_devices=N_CORES, debug=False)

    # ---- DRAM I/O ----
    xT = nc.dram_tensor("xT", [128, KC, S], dt.bfloat16, kind="ExternalInput")
    cosT = nc.dram_tensor("cosT", [128, S], dt.bfloat16, kind="ExternalInput")
    sinT = nc.dram_tensor("sinT", [128, S], dt.bfloat16, kind="ExternalInput")
    mstrip = nc.dram_tensor("mstrip", [128, 896], dt.bfloat16, kind="ExternalInput")
    maskbias = nc.dram_tensor("maskbias", [128, TC], dt.float32, kind="ExternalInput")
    mask01 = nc.dram_tensor("mask01", [128, TC], dt.float32, kind="ExternalInput")
    wreg = nc.dram_tensor("wreg", [KC, 128, OUT], dt.bfloat16, kind="ExternalInput")
    breg = nc.dram_tensor("breg", [OUT, 1], dt.float32, kind="ExternalInput")
    W = {}
    for l in range(L):
        W[f"wk{l}"] = nc.dram_tensor(f"wk{l}", [KC, 128, DL], dt.bfloat16, kind="ExternalInput")
        W[f"wv{l}"] = nc.dram_tensor(f"wv{l}", [KC, 128, DL], dt.bfloat16, kind="ExternalInput")
        W[f"aq{l}"] = nc.dram_tensor(f"aq{l}", [KC, 128, R], dt.bfloat16, kind="ExternalInput")
        W[f"av{l}"] = nc.dram_tensor(f"av{l}", [KC, 128, R], dt.bfloat16, kind="ExternalInput")
        W[f"bq{l}"] = nc.dram_tensor(f"bq{l}", [R, DL], dt.bfloat16, kind="ExternalInput")
        W[f"bv{l}"] = nc.dram_tensor(f"bv{l}", [R, DL], dt.bfloat16, kind="ExternalInput")
    # layer 0 (full-sequence Megatron TP layouts)
    W["wq0"] = nc.dram_tensor("wq0", [KC, 128, DL], dt.bfloat16, kind="ExternalInput")
    W["wo0"] = nc.dram_tensor("wo0", [NH, 128, D], dt.bfloat16, kind="ExternalInput")
    # gate|up interleaved per fc chunk: [fc][p][k*256 + (0:128 gate | 128:256 up)]
    W["wgu0"] = nc.dram_tensor("wgu0", [FC, 128, KC, 256], dt.bfloat16, kind="ExternalInput")
    W["wd0"] = nc.dram_tensor("wd0", [FC, 128, D], dt.bfloat16, kind="ExternalInput")
    # layer 1 (tail: weights stream as moving operand)
    W["wq1"] = nc.dram_tensor("wq1", [KC, 128, DL], dt.bfloat16, kind="ExternalInput")
    W["wo1"] = nc.dram_tensor("wo1", [NH, 128, D], dt.bfloat16, kind="ExternalInput")
    W["wgu1"] = nc.dram_tensor("wgu1", [KC, 128, 2 * FL], dt.bfloat16, kind="ExternalInput")
    W["wd1"] = nc.dram_tensor("wd1", [FC, 128, D], dt.bfloat16, kind="ExternalInput")
    out_dram = nc.dram_tensor("out", [OUT, 1], dt.float32, kind="ExternalOutput")

    with tile.TileContext(nc) as tc:
        with (
            tc.tile_pool(name="persist", bufs=1) as pp,
            tc.tile_pool(name="wts", bufs=3) as wp,
            tc.tile_pool(name="colw", bufs=4) as cwp,
            tc.tile_pool(name="tails", bufs=2) as tsp,
            tc.tile_pool(name="tmp", bufs=3) as tp_,
            tc.tile_pool(name="stage", bufs=2) as stp,
            tc.tile_pool(name="psum", bufs=8, space="PSUM") as ps,
            tc.tile_pool(name="dram", bufs=1, space="DRAM") as dram,
        ):
            f32, bf = dt.float32, dt.bfloat16
            # ---- persistent tiles ----
            h = pp.tile([128, KC, S], f32, tag="h")
            hn = pp.tile([128, KC, S], bf, tag="hn")
            cos_sb = pp.tile([128, S], bf, tag="cos")
            sin_sb = pp.tile([128, S], bf, tag="sin")
            mstrip_sb = pp.tile([128, 896], bf, tag="mstrip")
            mb_sb = pp.tile([128, TC], f32, tag="mb")
            m01_sb = pp.tile([128, TC], f32, tag="m01")
            cos_last = pp.tile([128, 1], f32, tag="cos_last")
            sin_last = pp.tile([128, 1], f32, tag="sin_last")
            ones_bf = pp.tile([128, 1], bf, tag="onesbf")
            allones = pp.tile([128, 128], bf, tag="allones")
            oneD128 = pp.tile([128, 128], bf, tag="oneD128")
            one1_bf = pp.tile([1, 1], bf, tag="one1")
            one64_bf = pp.tile([1, 1], bf, tag="one64")
            eps128 = pp.tile([128, 1], f32, tag="eps128")
            eps1 = pp.tile([1, 1], f32, tag="eps1")
            qT = pp.tile([128, NH, S], bf, tag="qT")     # q, then reused for ctx
            kT = pp.tile([128, NH, S], bf, tag="kT")
            vN = pp.tile([128, TC, DL], bf, tag="vN")
            expT = pp.tile([128, TC, TT], bf, tag="expT")
            mT = pp.tile([128, FC, TT], bf, tag="mT")    # per-half mlp act
            aqw = pp.tile([128, KC, R], bf, tag="aqw")
            avw = pp.tile([128, KC, R], bf, tag="avw")
            bq_sb = pp.tile([R, DL], bf, tag="bq")
            bv_sb = pp.tile([R, DL], bf, tag="bv")
            aq_sb = pp.tile([R, S], bf, tag="aq")
            av_sb = pp.tile([R, S], bf, tag="av")
            wreg_sb = pp.tile([128, KC, OUT], bf, tag="wreg")
            breg_sb = pp.tile([OUT, 1], f32, tag="breg")
            # tail smalls
            hn_last = pp.tile([128, KC, 1], bf, tag="hn_last")
            q_last = pp.tile([128, NH], bf, tag="q_last")
            ctx_n = pp.tile([128, NH], bf, tag="ctx_n")
            exp_tail = pp.tile([128, NH, TC], bf, tag="exp_tail")
            m_pm = pp.tile([128, FC], bf, tag="m_pm")

            # ---- constants in ----
            nc.vector.memset(ones_bf[:], 1.0)
            nc.vector.memset(allones[:], 1.0)
            nc.vector.memset(oneD128[:], 1.0 / D)
            nc.vector.memset(one1_bf[:], 1.0)
            nc.vector.memset(one64_bf[:], 1.0 / 64.0)
            nc.vector.memset(eps128[:], EPS)
            nc.vector.memset(eps1[:], EPS)
            nc.sync.dma_start(cos_sb[:], cosT[:])
            nc.sync.dma_start(sin_sb[:], sinT[:])
            nc.sync.dma_start(mstrip_sb[:], mstrip[:])
            nc.sync.dma_start(mb_sb[:], maskbias[:])
            nc.sync.dma_start(m01_sb[:], mask01[:])
            nc.scalar.copy(cos_last[:], cos_sb[:, S - 1:S])
            nc.scalar.copy(sin_last[:], sin_sb[:, S - 1:S])
            nc.sync.dma_start(breg_sb[:], breg[:])
            for k in range(KC):
                nc.sync.dma_start(wreg_sb[:, k, :], wreg[k])

            # ---- h init per half: bf16 upload -> fp32 residual ----
            for t in range(NT):
                ts_ = slice(t * TT, (t + 1) * TT)
                nc.sync.dma_start(hn[:, :, ts_], xT[:, :, ts_])
                nc.vector.tensor_copy(h[:, :, ts_], hn[:, :, ts_])

            # DRAM bounce buffers for collectives
            ar1h_in = [dram.tile([128, KC, TT], bf, name=f"ar1hi_{t}") for t in range(NT)]
            ar1h_out = [dram.tile([128, KC, TT], bf, name=f"ar1ho_{t}") for t in range(NT)]
            ar2h_in = [dram.tile([128, KC, TT], bf, name=f"ar2hi_{t}") for t in range(NT)]
            ar2h_out = [dram.tile([128, KC, TT], bf, name=f"ar2ho_{t}") for t in range(NT)]
            ar_in_s = dram.tile([128, KC, 1], bf)
            ar_out_s = dram.tile([128, KC, 1], bf)

            def norm_half(t):
                """hn[:, :, half t] = h / sqrt(mean(h^2) + eps), bf16."""
                ts_ = slice(t * TT, (t + 1) * TT)
                psb = ps.tile([128, TT], f32, tag="psum", name=f"nps_{t}")
                for k in range(KC):
                    sq = tp_.tile([128, TT], bf, tag="sq", bufs=2, name=f"nsq_{k}_{t}")
                    nc.scalar.activation(sq[:], h[:, k, ts_],
                                         mybir.ActivationFunctionType.Square)
                    # all-ones/D stationary => result broadcast to all partitions
                    nc.tensor.matmul(psb[:], oneD128[:], sq[:],
                                     start=(k == 0), stop=(k == KC - 1))
                rs = tp_.tile([128, TT], f32, tag="rsbc", bufs=1, name=f"nrs_{t}")
                nc.scalar.activation(rs[:], psb[:],
                                     mybir.ActivationFunctionType.Sqrt, bias=eps128[:])
                nc.vector.reciprocal_approx_fast(rs[:], rs[:])
                for k in range(KC):
                    nc.vector.tensor_mul(hn[:, k, ts_], h[:, k, ts_], rs[:])

            def lora_down_half(aw, dst, t):
                psa = ps.tile([128, TT], f32, tag="psum", name=f"ldh_{id(aw)}_{t}")
                for k in range(KC):
                    nc.tensor.matmul(
                        psa[0:R, :], aw[:, k, :], hn[:, k, t * TT:(t + 1) * TT],
                        start=(k == 0), stop=(k == KC - 1),
                    )
                nc.scalar.copy(dst[:, t * TT:(t + 1) * TT], psa[0:R, :])

            def rope_from_psum(psq, dst, hc, t):
                """Apply RoPE to psum [128,TT] (one head, token tile t) and
                write bf16 into dst[:, hc, t*TT:...]."""
                ts_ = slice(t * TT, (t + 1) * TT)
                t2 = tp_.tile([128, TT], bf, tag="ropetB", bufs=1)
                t4 = tp_.tile([128, TT], bf, tag="ropetB", bufs=1)
                nc.vector.tensor_mul(dst[0:64, hc, ts_], psq[0:64, :], cos_sb[0:64, ts_])
                nc.vector.tensor_mul(t2[0:64, :], psq[64:128, :], sin_sb[0:64, ts_])
                nc.vector.tensor_sub(dst[0:64, hc, ts_], dst[0:64, hc, ts_], t2[0:64, :])
                nc.vector.tensor_mul(dst[64:128, hc, ts_], psq[64:128, :], cos_sb[64:128, ts_])
                nc.vector.tensor_mul(t4[64:128, :], psq[0:64, :], sin_sb[64:128, ts_])
                nc.vector.tensor_add(dst[64:128, hc, ts_], dst[64:128, hc, ts_], t4[64:128, :])

            def qk_proj(wname, dst, lora_bw, lora_act):
                """dst[:, hc, :] (bf16, roped) = rope(W.T @ hn [+ lora])."""
                psq = [[ps.tile([128, TT], f32, tag="psum", name=f"psq_{wname}_{hc}_{t}")
                        for t in range(NT)] for hc in range(NH)]
                for kk in range(KC // 2):
                    wt = wp.tile([128, 2, DL], bf, tag="wqkv", name=f"w_{wname}_{kk}")
                    nc.sync.dma_start(wt[:], W[wname][2 * kk:2 * kk + 2].rearrange("i p c -> p i c"))
                    for i in range(2):
                        k = 2 * kk + i
                        for hc in range(NH):
                            for t in range(NT):
                                nc.tensor.matmul(
                                    psq[hc][t][:], wt[:, i, hc * HD:(hc + 1) * HD],
                                    hn[:, k, t * TT:(t + 1) * TT],
                                    start=(k == 0),
                                    stop=(lora_bw is None and k == KC - 1),
                                )
                for hc in range(NH):
                    for t in range(NT):
                        if lora_bw is not None:
                            nc.tensor.matmul(
                                psq[hc][t][:], lora_bw[:, hc * HD:(hc + 1) * HD],
                                lora_act[:, t * TT:(t + 1) * TT],
                                start=False, stop=True,
                            )
                        rope_from_psum(psq[hc][t], dst, hc, t)

            def v_proj(l):
                """vN [128(tok), TC, DL] bf16 = hn.T @ Wv + lora."""
                psv = [ps.tile([128, DL], f32, tag="psum", name=f"psv_{c}")
                       for c in range(TC)]
                for kk in range(KC // 2):
                    wt = wp.tile([128, 2, DL], bf, tag="wqkv", name=f"wv_t_{kk}")
                    nc.sync.dma_start(wt[:], W[f"wv{l}"][2 * kk:2 * kk + 2].rearrange("i p c -> p i c"))
                    for i in range(2):
                        k = 2 * kk + i
                        for c in range(TC):
                            nc.tensor.matmul(
                                psv[c][:], hn[:, k, c * 128:(c + 1) * 128], wt[:, i, :],
                                start=(k == 0), stop=False,
                            )
                for c in range(TC):
                    nc.tensor.matmul(
                        psv[c][:], av_sb[:, c * 128:(c + 1) * 128], bv_sb[:],
                        start=False, stop=True,
                    )
                    nc.scalar.copy(vN[:, c, :], psv[c][:])

            def q_proj_half(wname, dst, t):
                """dst[:, hc, half t] = rope(Wq.T @ hn + lora)."""
                psq = [ps.tile([128, TT], f32, tag="psum", name=f"psqq_{hc}_{t}")
                       for hc in range(NH)]
                for kk in range(KC // 2):
                    wt = wp.tile([128, 2, DL], bf, tag="wqkv", name=f"wq_{kk}_{t}")
                    nc.sync.dma_start(wt[:], W[wname][2 * kk:2 * kk + 2].rearrange("i p c -> p i c"))
                    for i in range(2):
                        k = 2 * kk + i
                        for hc in range(NH):
                            nc.tensor.matmul(
                                psq[hc][:], wt[:, i, hc * HD:(hc + 1) * HD],
                                hn[:, k, t * TT:(t + 1) * TT],
                                start=(k == 0), stop=False,
                            )
                for hc in range(NH):
                    nc.tensor.matmul(
                        psq[hc][:], bq_sb[:, hc * HD:(hc + 1) * HD],
                        aq_sb[:, t * TT:(t + 1) * TT],
                        start=False, stop=True,
                    )
                    rope_from_psum(psq[hc], dst, hc, t)

            def attention_half(t):
                """qT,kT,vN -> ctx (written into qT) for token half t.

                The denominator/ctx accumulation matmuls for chunk j are
                emitted two chunks behind the QK+exp chain, so by the time
                the PE (strict in-order queue) reaches them their exp input
                has drained from the ACT engine and nothing head-of-line
                blocks."""
                ts_ = slice(t * TT, (t + 1) * TT)
                jmax = (t + 1) * (TT // 128)
                LAG = 2
                for hh in range(NH):
                    psb = ps.tile([128, TT], f32, tag="psum", name=f"psd_{hh}_{t}")
                    psc = ps.tile([128, TT], f32, tag="psum", name=f"psc_{hh}_{t}")

                    def emit_reduce_j(j):
                        nc.tensor.matmul(
                            psb[:], allones[:], expT[:, j, :],
                            start=(j == 0), stop=(j == jmax - 1),
                        )
                        nc.tensor.matmul(
                            psc[:], vN[:, j, hh * HD:(hh + 1) * HD],
                            expT[:, j, :],
                            start=(j == 0), stop=(j == jmax - 1),
                        )

                    for j in range(jmax):
                        pss = ps.tile([128, TT], f32, tag="psum",
                                      name=f"pss_{hh}_{t}_{j}")
                        nc.tensor.matmul(
                            pss[:], kT[:, hh, j * 128:(j + 1) * 128],
                            qT[:, hh, ts_], start=True, stop=True,
                        )
                        nc.scalar.activation(
                            expT[:, j, :], pss[:],
                            mybir.ActivationFunctionType.Exp,
                            bias=mb_sb[:, j:j + 1], scale=1.0,
                        )
                        off = t * TT - j * 128
                        if off < 128:
                            nc.vector.tensor_mul(
                                expT[:, j, :], expT[:, j, :],
                                mstrip_sb[:, 384 + off:896 + off],
                            )
                        if j >= LAG:
                            emit_reduce_j(j - LAG)
                    for j in range(max(0, jmax - LAG), jmax):
                        emit_reduce_j(j)
                    rden = tp_.tile([128, TT], f32, tag="rsbc", bufs=1,
                                    name=f"rden_{hh}_{t}")
                    nc.vector.reciprocal_approx_fast(rden[:], psb[:])
                    nc.vector.tensor_mul(qT[:, hh, ts_], psc[:], rden[:])

            def out_proj_half(l, t):
                """attn partial for token half t -> ar1h_in[t]."""
                for og in range(4):  # groups of 4 output chunks
                    pso = [ps.tile([128, TT], f32, tag="psum", name=f"pso_{og}_{oi}_{t}")
                           for oi in range(4)]
                    for hp in range(NH // 2):
                        wt = wp.tile([128, 2, TT], bf, tag="wqkv",
                                     name=f"wo_t_{og}_{hp}_{t}")
                        nc.sync.dma_start(
                            wt[:], W[f"wo{l}"][2 * hp:2 * hp + 2,
                                               :, og * 512:(og + 1) * 512]
                            .rearrange("i p c -> p i c"))
                        for i in range(2):
                            hc = 2 * hp + i
                            for oi in range(4):
                                nc.tensor.matmul(
                                    pso[oi][:], wt[:, i, oi * 128:(oi + 1) * 128],
                                    qT[:, hc, t * TT:(t + 1) * TT],
                                    start=(hc == 0), stop=(hc == NH - 1),
                                )
                    for oi in range(4):
                        st = stp.tile([128, TT], bf, tag="stage")
                        nc.vector.tensor_copy(st[:], pso[oi][:])
                        nc.sync.dma_start(ar1h_in[t][:, og * 4 + oi, :], st[:])

            def allreduce1_half(t):
                nc.gpsimd.collective_compute(
                    "AllReduce", mybir.AluOpType.add,
                    replica_groups=REPLICA_GROUPS,
                    ins=[ar1h_in[t].opt()], outs=[ar1h_out[t].opt()],
                )

            def add1_half(t):
                # landing DMA rides the scalar HWDGE ring so its AR wait can't
                # head-of-line-block weight loads on the sync ring
                ts_ = slice(t * TT, (t + 1) * TT)
                nc.scalar.dma_start(hn[:, :, ts_], ar1h_out[t][:])
                for k in range(KC):
                    nc.vector.tensor_add(h[:, k, ts_], h[:, k, ts_], hn[:, k, ts_])

            def allreduce2_half(t):
                nc.gpsimd.collective_compute(
                    "AllReduce", mybir.AluOpType.add,
                    replica_groups=REPLICA_GROUPS,
                    ins=[ar2h_in[t].opt()], outs=[ar2h_out[t].opt()],
                )

            def add2_half(t):
                ts_ = slice(t * TT, (t + 1) * TT)
                nc.scalar.dma_start(hn[:, :, ts_], ar2h_out[t][:])
                for k in range(KC):
                    nc.vector.tensor_add(h[:, k, ts_], h[:, k, ts_], hn[:, k, ts_])

            def k_proj_half(wname, dst, t, tag="wqkv"):
                psq = [ps.tile([128, TT], f32, tag="psum", name=f"psqh_{hc}_{t}")
                       for hc in range(NH)]
                for kk in range(KC // 2):
                    wt = wp.tile([128, 2, DL], bf, tag=tag, name=f"wkh_{wname}_{kk}_{t}")
                    nc.sync.dma_start(wt[:], W[wname][2 * kk:2 * kk + 2].rearrange("i p c -> p i c"))
                    for i in range(2):
                        k = 2 * kk + i
                        for hc in range(NH):
                            nc.tensor.matmul(
                                psq[hc][:], wt[:, i, hc * HD:(hc + 1) * HD],
                                hn[:, k, t * TT:(t + 1) * TT],
                                start=(k == 0), stop=(k == KC - 1),
                            )
                for hc in range(NH):
                    rope_from_psum(psq[hc], dst, hc, t)

            def v_proj_half(l, t, tag="wqkv"):
                psv = [ps.tile([128, DL], f32, tag="psum", name=f"psvh_{c}_{t}")
                       for c in range(4)]
                for kk in range(KC // 2):
                    wt = wp.tile([128, 2, DL], bf, tag=tag, name=f"wvh_{l}_{kk}_{t}")
                    nc.sync.dma_start(wt[:], W[f"wv{l}"][2 * kk:2 * kk + 2].rearrange("i p c -> p i c"))
                    for i in range(2):
                        k = 2 * kk + i
                        for ci in range(4):
                            c = t * 4 + ci
                            nc.tensor.matmul(
                                psv[ci][:], hn[:, k, c * 128:(c + 1) * 128], wt[:, i, :],
                                start=(k == 0), stop=False,
                            )
                for ci in range(4):
                    c = t * 4 + ci
                    nc.tensor.matmul(
                        psv[ci][:], av_sb[:, c * 128:(c + 1) * 128], bv_sb[:],
                        start=False, stop=True,
                    )
                    nc.scalar.copy(vN[:, c, :], psv[ci][:])

            def mlp_gate_up_half(l, t):
                ts_ = slice(t * TT, (t + 1) * TT)
                for fc in range(FC):
                    psg = ps.tile([128, TT], f32, tag="psum", name=f"psg_{fc}_{t}")
                    psu = ps.tile([128, TT], f32, tag="psum", name=f"psu_{fc}_{t}")
                    for kh in range(2):
                        wgu = cwp.tile([128, KC // 2, 256], bf, tag="wgu",
                                       name=f"wgu_{fc}_{kh}_{t}")
                        nc.sync.dma_start(
                            wgu[:], W[f"wgu{l}"][fc][:, kh * (KC // 2):(kh + 1) * (KC // 2), :])
                        for ki in range(KC // 2):
                            k = kh * (KC // 2) + ki
                            nc.tensor.matmul(psg[:], wgu[:, ki, 0:128], hn[:, k, ts_],
                                             start=(k == 0), stop=(k == KC - 1))
                            nc.tensor.matmul(psu[:], wgu[:, ki, 128:256], hn[:, k, ts_],
                                             start=(k == 0), stop=(k == KC - 1))
                    sg = tp_.tile([128, TT], bf, tag="silu", bufs=1, name=f"sg_{fc}_{t}")
                    nc.scalar.activation(sg[:], psg[:], mybir.ActivationFunctionType.Silu)
                    nc.vector.tensor_mul(mT[:, fc, :], sg[:], psu[:])

            def mlp_down_half(l, t):
                for og in range(4):
                    pso = [ps.tile([128, TT], f32, tag="psum", name=f"psd_{og}_{oi}_{t}")
                           for oi in range(4)]
                    for kp in range((FC + 1) // 2):
                        nk = min(2, FC - 2 * kp)
                        wt = wp.tile([128, 2, TT], bf, tag="wqkv",
                                     name=f"wd_t_{og}_{kp}_{t}")
                        nc.sync.dma_start(
                            wt[:, 0:nk, :],
                            W[f"wd{l}"][2 * kp:2 * kp + nk,
                                        :, og * 512:(og + 1) * 512]
                            .rearrange("i p c -> p i c"))
                        for i in range(nk):
                            kc = 2 * kp + i
                            for oi in range(4):
                                nc.tensor.matmul(
                                    pso[oi][:], wt[:, i, oi * 128:(oi + 1) * 128],
                                    mT[:, kc, :],
                                    start=(kc == 0), stop=(kc == FC - 1),
                                )
                    for oi in range(4):
                        st = stp.tile([128, TT], bf, tag="stage")
                        nc.vector.tensor_copy(st[:], pso[oi][:])
                        nc.sync.dma_start(ar2h_in[t][:, og * 4 + oi, :], st[:])

            # ---------- tail (last layer): only the last token flows through
            # q/attention/Wo/MLP. Matvecs are transposed: the activation
            # column is the stationary operand, weights stream as rhs.

            def row_to_pm(row_sb, psum_pm, ncols):
                """psum_pm[:, c] = row_sb[0, c*128:(c+1)*128].T via K=1 matmuls."""
                for c in range(ncols):
                    nc.tensor.matmul(
                        psum_pm[:, c:c + 1],
                        row_sb[0:1, c * 128:(c + 1) * 128], one1_bf[:],
                        start=True, stop=True,
                    )

            def q_tail():
                psq = ps.tile([128, TT], f32, tag="psum", name="psq_tail")
                for kk in range(KC // 2):
                    wt = wp.tile([128, 2, DL], bf, tag="wqkv", name=f"wq1s_{kk}")
                    nc.sync.dma_start(wt[:], W["wq1"][2 * kk:2 * kk + 2].rearrange("i p c -> p i c"))
                    for i in range(2):
                        k = 2 * kk + i
                        nc.tensor.matmul(psq[0:1, :], hn[:, k, S - 1:S], wt[:, i, :],
                                         start=(k == 0), stop=False)
                nc.tensor.matmul(psq[0:1, :], aq_sb[:, S - 1:S], bq_sb[:],
                                 start=False, stop=True)
                qrow = tp_.tile([1, DL], bf, tag="qrow", bufs=1)
                nc.scalar.copy(qrow[:], psq[0:1, :])
                psqpm = ps.tile([128, NH], f32, tag="psum", name="psqpm")
                row_to_pm(qrow, psqpm, NH)
                # rope, all heads at once (per-partition cos/sin scalars)
                t2 = tp_.tile([128, NH], bf, tag="ropeS", bufs=2)
                t4 = tp_.tile([128, NH], bf, tag="ropeS", bufs=2)
                nc.vector.tensor_scalar_mul(q_last[0:64, :], psqpm[0:64, :],
                                            cos_last[0:64, :])
                nc.vector.tensor_scalar_mul(t2[0:64, :], psqpm[64:128, :],
                                            sin_last[0:64, :])
                nc.vector.tensor_sub(q_last[0:64, :], q_last[0:64, :], t2[0:64, :])
                nc.vector.tensor_scalar_mul(q_last[64:128, :], psqpm[64:128, :],
                                            cos_last[64:128, :])
                nc.vector.tensor_scalar_mul(t4[64:128, :], psqpm[0:64, :],
                                            sin_last[64:128, :])
                nc.vector.tensor_add(q_last[64:128, :], q_last[64:128, :], t4[64:128, :])

            def attention_tail():
                # scores for all heads/key-chunks in one psum tile
                pss = ps.tile([128, NH, TC], f32, tag="psum", name="pss_tail")
                for hh in range(NH):
                    for j in range(TC):
                        nc.tensor.matmul(
                            pss[:, hh, j:j + 1],
                            kT[:, hh, j * 128:(j + 1) * 128], q_last[:, hh:hh + 1],
                            start=True, stop=True,
                        )
                nc.scalar.activation(exp_tail[:], pss[:],
                                     mybir.ActivationFunctionType.Exp)
                for j in range(TC):
                    nc.vector.tensor_scalar_mul(exp_tail[:, :, j], exp_tail[:, :, j],
                                                m01_sb[:, j:j + 1])
                # denominators: all-partition sums then free-dim reduce per head
                psdb = ps.tile([128, NH, TC], f32, tag="psum", name="psdb_tail")
                nc.tensor.matmul(psdb[:], allones[:], exp_tail[:],
                                 start=True, stop=True)
                rdt = tp_.tile([128, NH], f32, tag="rdt")
                nc.vector.tensor_reduce(
                    rdt[:], psdb[:],
                    axis=mybir.AxisListType.X, op=mybir.AluOpType.add,
                )
                nc.vector.reciprocal_approx_fast(rdt[:], rdt[:])
                psc = ps.tile([128, NH], f32, tag="psum", name="psc_tail")
                for hh in range(NH):
                    for j in range(TC):
                        nc.tensor.matmul(
                            psc[:, hh:hh + 1], vN[:, j, hh * HD:(hh + 1) * HD],
                            exp_tail[:, hh, j:j + 1],
                            start=(j == 0), stop=(j == TC - 1),
                        )
                nc.vector.tensor_mul(ctx_n[:], psc[:], rdt[:])

            def wo_tail():
                pso = [ps.tile([128, TT], f32, tag="psum", name=f"psot_{c}")
                       for c in range(DC)]
                for hc in range(NH):
                    for dh in range(2):
                        wt = tsp.tile([128, 1024], bf, tag="wod1s",
                                      name=f"wo1s_{hc}_{dh}")
                        nc.sync.dma_start(wt[:], W["wo1"][hc][:, dh * 1024:(dh + 1) * 1024])
                        for ci in range(2):
                            c = dh * 2 + ci
                            nc.tensor.matmul(
                                pso[c][0:1, :], ctx_n[:, hc:hc + 1],
                                wt[:, ci * 512:(ci + 1) * 512],
                                start=(hc == 0), stop=(hc == NH - 1),
                            )
                orow = tp_.tile([1, D], bf, tag="orow", bufs=1)
                for c in range(DC):
                    nc.scalar.copy(orow[0:1, c * 512:(c + 1) * 512], pso[c][0:1, :])
                pst = ps.tile([128, KC], f32, tag="psum", name="pst_wo")
                row_to_pm(orow, pst, KC)
                stpm = tp_.tile([128, KC], bf, tag="stpm", bufs=2, name="stpm_wo")
                nc.vector.tensor_copy(stpm[:], pst[:])
                nc.sync.dma_start(ar_in_s[:, :, 0], stpm[:])

            def allreduce_and_add_tail():
                nc.gpsimd.collective_compute(
                    "AllReduce", mybir.AluOpType.add,
                    replica_groups=REPLICA_GROUPS,
                    ins=[ar_in_s.opt()], outs=[ar_out_s.opt()],
                )
                lb = tp_.tile([128, KC, 1], bf, tag="ar_land", bufs=2)
                nc.scalar.dma_start(lb[:], ar_out_s[:])
                nc.vector.tensor_add(h[:, :, S - 1:S], h[:, :, S - 1:S], lb[:])

            def norm_tail_to_hn_last():
                sqt = tp_.tile([128, KC, 1], bf, tag="sqlast")
                nc.scalar.activation(sqt[:], h[:, :, S - 1:S],
                                     mybir.ActivationFunctionType.Square)
                psl = ps.tile([128, 1], f32, tag="psum", name="psl_normt")
                for k in range(KC):
                    nc.tensor.matmul(psl[:], oneD128[:], sqt[:, k, :],
                                     start=(k == 0), stop=(k == KC - 1))
                rsb = tp_.tile([128, 1], f32, tag="rsb_tail", bufs=2)
                nc.scalar.activation(rsb[:], psl[:],
                                     mybir.ActivationFunctionType.Sqrt, bias=eps128[:])
                nc.vector.reciprocal_approx_fast(rsb[:], rsb[:])
                nc.vector.tensor_scalar_mul(hn_last[:], h[:, :, S - 1:S], rsb[:])

            def mlp_tail():
                norm_tail_to_hn_last()
                # gate/up: stream combined gate|up weights as rhs
                psg = [ps.tile([128, TT], f32, tag="psum", name=f"psgt_{c}")
                       for c in range(len(FLCH))]
                psu = [ps.tile([128, TT], f32, tag="psum", name=f"psut_{c}")
                       for c in range(len(FLCH))]
                for k in range(KC):
                    wgt = tsp.tile([128, FL], bf, tag="wg1s", name=f"wg1s_{k}")
                    wut = tsp.tile([128, FL], bf, tag="wu1s", name=f"wu1s_{k}")
                    nc.sync.dma_start(wgt[:], W["wgu1"][k][:, 0:FL])
                    nc.sync.dma_start(wut[:], W["wgu1"][k][:, FL:2 * FL])
                    for c, (off, sz) in enumerate(FLCH):
                        nc.tensor.matmul(psg[c][0:1, 0:sz], hn_last[:, k, :],
                                         wgt[:, off:off + sz],
                                         start=(k == 0), stop=(k == KC - 1))
                        nc.tensor.matmul(psu[c][0:1, 0:sz], hn_last[:, k, :],
                                         wut[:, off:off + sz],
                                         start=(k == 0), stop=(k == KC - 1))
                mrow = tp_.tile([1, FL], bf, tag="mrow", bufs=1)
                for c, (off, sz) in enumerate(FLCH):
                    grow = tp_.tile([1, TT], bf, tag="grow", bufs=1, name=f"grow_{c}")
                    nc.scalar.activation(grow[0:1, 0:sz], psg[c][0:1, 0:sz],
                                         mybir.ActivationFunctionType.Silu)
                    nc.vector.tensor_mul(mrow[0:1, off:off + sz], grow[0:1, 0:sz],
                                         psu[c][0:1, 0:sz])
                psm = ps.tile([128, FC], f32, tag="psum", name="psm_pm")
                row_to_pm(mrow, psm, FC)
                nc.vector.tensor_copy(m_pm[:], psm[:])
                # down
                psd = [ps.tile([128, TT], f32, tag="psum", name=f"psdt2_{c}")
                       for c in range(DC)]
                for fc in range(FC):
                    for dh in range(2):
                        wdt = tsp.tile([128, 1024], bf, tag="wod1s",
                                       name=f"wd1s_{fc}_{dh}")
                        nc.sync.dma_start(wdt[:],
                                          W["wd1"][fc][:, dh * 1024:(dh + 1) * 1024])
                        for ci in range(2):
                            c = dh * 2 + ci
                            nc.tensor.matmul(
                                psd[c][0:1, :], m_pm[:, fc:fc + 1],
                                wdt[:, ci * 512:(ci + 1) * 512],
                                start=(fc == 0), stop=(fc == FC - 1),
                            )
                orow = tp_.tile([1, D], bf, tag="orow", bufs=1)
                for c in range(DC):
                    nc.scalar.copy(orow[0:1, c * 512:(c + 1) * 512], psd[c][0:1, :])
                pst = ps.tile([128, KC], f32, tag="psum", name="pst_mlp")
                row_to_pm(orow, pst, KC)
                stpm = tp_.tile([128, KC], bf, tag="stpm", bufs=2, name="stpm_mlp")
                nc.vector.tensor_copy(stpm[:], pst[:])
                nc.sync.dma_start(ar_in_s[:, :, 0], stpm[:])

            # ================= layer 0 (all per token half, so the first
            # attn AllReduce fires as early as possible) =================
            for k in range(KC):
                nc.sync.dma_start(aqw[:, k, :], W["aq0"][k])
                nc.sync.dma_start(avw[:, k, :], W["av0"][k])
            nc.sync.dma_start(bq_sb[:], W["bq0"][:])
            nc.sync.dma_start(bv_sb[:], W["bv0"][:])
            for t in range(NT):
                norm_half(t)
                lora_down_half(avw, av_sb, t)
                lora_down_half(aqw, aq_sb, t)
                k_proj_half("wk0", kT, t)
                v_proj_half(0, t)
                q_proj_half("wq0", qT, t)
                attention_half(t)
                out_proj_half(0, t)
                allreduce1_half(t)
            for t in range(NT):
                add1_half(t)
                norm_half(t)
                mlp_gate_up_half(0, t)
                mlp_down_half(0, t)
                allreduce2_half(t)

            # ================= layer 1 (tail) =================
            for k in range(KC):
                nc.sync.dma_start(aqw[:, k, :], W["aq1"][k])
                nc.sync.dma_start(avw[:, k, :], W["av1"][k])
            nc.sync.dma_start(bq_sb[:], W["bq1"][:])
            nc.sync.dma_start(bv_sb[:], W["bv1"][:])
            for t in range(NT):
                add2_half(t)
                norm_half(t)
                lora_down_half(avw, av_sb, t)
                if t == NT - 1:
                    lora_down_half(aqw, aq_sb, t)
                k_proj_half("wk1", kT, t)
                v_proj_half(1, t)
            q_tail()
            attention_tail()
            wo_tail()
            allreduce_and_add_tail()
            mlp_tail()
            allreduce_and_add_tail()

            # ================= final norm + head (last token only) ========
            sq = tp_.tile([128, KC, 1], bf, tag="sqlast")
            nc.scalar.activation(sq[:], h[:, :, S - 1:S],
                                 mybir.ActivationFunctionType.Square)
            psl = ps.tile([128, 1], f32, tag="psum", name="psl_final")
            for k in range(KC):
                nc.tensor.matmul(psl[:], oneD128[:], sq[:, k, :],
                                 start=(k == 0), stop=(k == KC - 1))
            rsl = tp_.tile([128, 1], f32, tag="rsl_final")
            nc.scalar.activation(rsl[:], psl[:],
                                 mybir.ActivationFunctionType.Sqrt, bias=eps128[:])
            nc.vector.reciprocal_approx_fast(rsl[:], rsl[:])
            hl = tp_.tile([128, KC, 1], bf, tag="hlast")
            nc.vector.tensor_scalar_mul(hl[:], h[:, :, S - 1:S], rsl[:])
            pso = ps.tile([128, TT], f32, tag="psum", name="ps_head")
            for k in range(KC):
                nc.tensor.matmul(pso[0:OUT, 0:1], wreg_sb[:, k, :], hl[:, k, :],
                                 start=(k == 0), stop=(k == KC - 1))
            ot = tp_.tile([OUT, 1], f32, tag="outt")
            nc.vector.tensor_add(ot[:], pso[0:OUT, 0:1], breg_sb[:])
            nc.sync.dma_start(out_dram[:], ot[:])

    nc.finalize()
    return nc


_CACHED = {}


def _get_program():
    if "nc" not in _CACHED:
        _CACHED["nc"] = build_program()
    return _CACHED["nc"]


def _host_prepare(inputs):
    """Fold norms/scales into weights, gather embeddings, build the 8
    per-core input maps."""
    ids = np.asarray(inputs["input_ids"]).astype(np.int64)        # [B,S]
    amask = np.asarray(inputs["attention_mask"]).astype(np.int64)  # [B,S]
    embed = np.asarray(inputs["embed"], FP32)

    inv_sqrt_hd = FP32(1.0 / np.sqrt(HD))

    # rope tables (half: both halves identical)
    inv = 1.0 / (10000.0 ** (np.arange(0, HD, 2, dtype=np.float64) / HD))
    ang = (np.arange(S, dtype=np.float64)[:, None] * inv[None, :])  # [S,64]
    cos64 = np.cos(ang).T.astype(BF16)  # [64,S]
    sin64 = np.sin(ang).T.astype(BF16)
    cosT = np.concatenate([cos64, cos64], axis=0).copy()  # [128,S]
    sinT = np.concatenate([sin64, sin64], axis=0).copy()

    # causal strip [128, 896]: strip[p,u] = 1 if (u-384) >= p else 0
    u = np.arange(896)[None, :]
    p = np.arange(128)[:, None]
    mstrip = ((u - 384) >= p).astype(BF16)

    common = {}

    def fold(l):
        g1 = np.asarray(inputs["norm1"], FP32)[l][:, None]
        g2 = np.asarray(inputs["norm2"], FP32)[l][:, None]
        wq = np.asarray(inputs["Wq"], FP32)[l] * g1 * inv_sqrt_hd
        wk = np.asarray(inputs["Wk"], FP32)[l] * g1
        wv = np.asarray(inputs["Wv"], FP32)[l] * g1
        aq = np.asarray(inputs["Aq"], FP32)[l] * g1
        av = np.asarray(inputs["Av"], FP32)[l] * g1
        bq = np.asarray(inputs["Bq"], FP32)[l] * (SCALING * inv_sqrt_hd)
        bv = np.asarray(inputs["Bv"], FP32)[l] * SCALING
        wo = np.asarray(inputs["Wo"], FP32)[l]
        wg = np.asarray(inputs["Wgate"], FP32)[l] * g2
        wu = np.asarray(inputs["Wup"], FP32)[l] * g2
        wd = np.asarray(inputs["Wdown"], FP32)[l]
        return wq, wk, wv, aq, av, bq, bv, wo, wg, wu, wd

    folded = [fold(l) for l in range(L)]
    wregf = (np.asarray(inputs["Wreg"], FP32) * np.asarray(inputs["norm_f"], FP32)[:, None])
    common["wreg"] = wregf.reshape(KC, 128, OUT).astype(BF16)
    common["breg"] = np.asarray(inputs["breg"], FP32).reshape(OUT, 1)
    common["cosT"] = cosT
    common["sinT"] = sinT
    common["mstrip"] = mstrip

    in_maps = []
    for c in range(N_CORES):
        b = c // TP      # batch index (DP group)
        r = c % TP       # TP rank within group
        m = dict(common)
        # embedding gather, transposed, bf16: [D,S] -> [16,128,S] -> [128,16,S]
        xt = embed[ids[b]].T.reshape(KC, 128, S).transpose(1, 0, 2)
        m["xT"] = np.ascontiguousarray(xt).astype(BF16)
        # attention_mask bias [128, TC]: col j, part p -> key token 128j+p
        mb = np.where(amask[b] > 0, FP32(0), FP32(-1e9)).reshape(TC, 128).T
        m["maskbias"] = np.ascontiguousarray(mb)
        m["mask01"] = np.ascontiguousarray((amask[b] > 0).reshape(TC, 128).T).astype(FP32)
        for l in range(L):
            wq, wk, wv, aq, av, bq, bv, wo, wg, wu, wd = folded[l]
            dsl = slice(r * DL, (r + 1) * DL)
            fsl = slice(r * FL, (r + 1) * FL)
            m[f"wk{l}"] = np.ascontiguousarray(wk[:, dsl].reshape(KC, 128, DL)).astype(BF16)
            m[f"wv{l}"] = np.ascontiguousarray(wv[:, dsl].reshape(KC, 128, DL)).astype(BF16)
            m[f"aq{l}"] = np.ascontiguousarray(aq.reshape(KC, 128, R)).astype(BF16)
            m[f"av{l}"] = np.ascontiguousarray(av.reshape(KC, 128, R)).astype(BF16)
            m[f"bq{l}"] = np.ascontiguousarray(bq[:, dsl]).astype(BF16)
            m[f"bv{l}"] = np.ascontiguousarray(bv[:, dsl]).astype(BF16)
            m[f"wq{l}"] = np.ascontiguousarray(wq[:, dsl].reshape(KC, 128, DL)).astype(BF16)
            m[f"wo{l}"] = np.ascontiguousarray(wo[dsl].reshape(NH, 128, D)).astype(BF16)
            if l == 0:
                m["wd0"] = np.ascontiguousarray(wd[fsl].reshape(FC, 128, D)).astype(BF16)
                # gate|up interleaved, fc-major, contiguous per partition
                wg4 = wg[:, fsl].reshape(KC, 128, FC, 128).transpose(2, 1, 0, 3)
                wu4 = wu[:, fsl].reshape(KC, 128, FC, 128).transpose(2, 1, 0, 3)
                wgu = np.concatenate([wg4, wu4], axis=-1)  # [FC,128,KC,256]
                m["wgu0"] = np.ascontiguousarray(wgu).astype(BF16)
            else:
                wg_r = wg[:, fsl].reshape(KC, 128, FL)
                wu_r = wu[:, fsl].reshape(KC, 128, FL)
                m["wgu1"] = np.ascontiguousarray(
                    np.concatenate([wg_r, wu_r], axis=-1)).astype(BF16)
                m["wd1"] = np.ascontiguousarray(
                    wd[fsl].reshape(FC, 128, D)).astype(BF16)
        in_maps.append(m)
    return in_maps


def run_on_device(in_maps, trace=False):
    nc = _get_program()
    return bass_utils.run_bass_kernel_spmd(
        nc, in_maps, core_ids=list(range(N_CORES)), trace=trace,
    )


def kernel(**inputs):
    in_maps = _host_prepare(inputs)
    res = run_on_device(in_maps, trace=False)
    out = np.stack([
        res.results[0]["out"].reshape(OUT),
        res.results[TP]["out"].reshape(OUT),
    ]).astype(FP32)
    return out


# revision 49
# speedup vs baseline: 1.0952x; 1.0689x over previous
"""Trainium2 Bass kernel for nn_LlamaForSequenceRegression_14336600834254.

2-layer Llama (D=2048, H=16, HD=128, F=5632, LoRA r=16 on q/v) + regression
head, B=2, S=1024, fp32 reference.

Distribution (8 NeuronCores): DP2 x TP4.
  - cores 0-3 process batch 0, cores 4-7 batch 1 (data parallel).
  - within each group of 4: Megatron tensor parallel — Wq/Wk/Wv column
    shards (4 heads/core), Wo row shards, Wgate/Wup column shards
    (F/4=1408), Wdown row shards. AllReduce (bf16) after attn-out and
    after MLP-down, replica_groups=[[0,1,2,3],[4,5,6,7]].
  - embedding gather + norm-weight folding are done host-side; all
    device matmuls run in bf16 with fp32 PSUM accumulation; the
    residual stream / softmax / rmsnorm statistics are fp32.

Layout: activations are kept feature-major ("transposed"): h^T [D, T] as
SBUF tiles [128 part, 16 kchunk, 1024 tok] so every weight matmul uses the
natural [in, out] weight layout as lhsT and no transposes are needed.
Attention uses scores^T [Tk, Tq] so softmax needs no max-subtraction
(|scores| < ~6 with folded 1/sqrt(HD)) and probs feed the v-matmul
directly; the causal mask is an upload-once 0/1 strip multiplied into the
diagonal tiles, and the attention_mask rides the exp() per-partition bias.

Perf structure (v2):
  - attention / out_proj / AllReduce pipelined per token half so the attn
    AllReduce overlaps the other half's attention + the first MLP half.
  - partition reductions (rmsnorm sum-of-squares, softmax denominators)
    use an all-ones [128,128] stationary operand so the PSUM result is
    already broadcast across partitions: no M=1 matmuls, no
    single-partition reciprocals, no gpsimd broadcasts.
  - last layer runs in "tail" mode: only the last token flows through
    q/attention/Wo/MLP. Those matvecs are transposed — the activation
    vector is the stationary operand and the weights stream through the
    PE as the moving operand — so the tail is weight-DMA bound instead
    of LDWEIGHTS bound.
"""

import numpy as np
import ml_dtypes

import concourse.bacc as bacc
import concourse.tile as tile
from concourse import mybir
from concourse import bass_utils

BF16 = ml_dtypes.bfloat16
FP32 = np.float32

V, D, L, H, HD, F, R, ALPHA, B, S, OUT = 32000, 2048, 2, 16, 128, 5632, 16, 32, 2, 1024, 11
EPS = 1e-5
SCALING = ALPHA / R
N_CORES = 8
TP = 4
NH = H // TP          # 4 local heads
DL = NH * HD          # 512 local q/k/v cols
FL = F // TP          # 1408 local mlp cols
KC = D // 128         # 16 contraction chunks
FC = FL // 128        # 11 mlp chunks
TT = 512              # token tile (free dim per matmul)
NT = S // TT          # 2 token tiles
TC = S // 128         # 8 token chunks (128-wide)
DC = D // TT          # 4 output chunks of 512
FLCH = [(0, 512), (512, 512), (1024, 384)]  # FL split into <=512 chunks
REPLICA_GROUPS = [[0, 1, 2, 3], [4, 5, 6, 7]]

dt = mybir.dt


def build_program():
    """Build the SPMD Bass program (identical on all 8 cores; weights differ
    per core via the input maps)."""
    nc = bacc.Bacc(num_devices=N_CORES, debug=False)

    # ---- DRAM I/O ----
    xT = nc.dram_tensor("xT", [128, KC, S], dt.bfloat16, kind="ExternalInput")
    cosT = nc.dram_tensor("cosT", [128, S], dt.bfloat16, kind="ExternalInput")
    sinT = nc.dram_tensor("sinT", [128, S], dt.bfloat16, kind="ExternalInput")
    mstrip = nc.dram_tensor("mstrip", [128, 896], dt.bfloat16, kind="ExternalInput")
    maskbias = nc.dram_tensor("maskbias", [128, TC], dt.float32, kind="ExternalInput")
    mask01 = nc.dram_tensor("mask01", [128, TC], dt.float32, kind="ExternalInput")
    wreg = nc.dram_tensor("wreg", [KC, 128, OUT], dt.bfloat16, kind="ExternalInput")
    breg = nc.dram_tensor("breg", [OUT, 1], dt.float32, kind="ExternalInput")
    W = {}
    for l in range(L):
        W[f"wk{l}"] = nc.dram_tensor(f"wk{l}", [KC, 128, DL], dt.bfloat16, kind="ExternalInput")
        W[f"wv{l}"] = nc.dram_tensor(f"wv{l}", [KC, 128, DL], dt.bfloat16, kind="ExternalInput")
        W[f"aq{l}"] = nc.dram_tensor(f"aq{l}", [KC, 128, R], dt.bfloat16, kind="ExternalInput")
        W[f"av{l}"] = nc.dram_tensor(f"av{l}", [KC, 128, R], dt.bfloat16, kind="ExternalInput")
        W[f"bq{l}"] = nc.dram_tensor(f"bq{l}", [R, DL], dt.bfloat16, kind="ExternalInput")
        W[f"bv{l}"] = nc.dram_tensor(f"bv{l}", [R, DL], dt.bfloat16, kind="ExternalInput")
    # layer 0 (full-sequence Megatron TP layouts)
    W["wq0"] = nc.dram_tensor("wq0", [KC, 128, DL], dt.bfloat16, kind="ExternalInput")
    W["wo0"] = nc.dram_tensor("wo0", [NH, 128, D], dt.bfloat16, kind="ExternalInput")
    # gate|up interleaved per fc chunk: [fc][p][k*256 + (0:128 gate | 128:256 up)]
    W["wgu0"] = nc.dram_tensor("wgu0", [FC, 128, KC, 256], dt.bfloat16, kind="ExternalInput")
    W["wd0"] = nc.dram_tensor("wd0", [FC, 128, D], dt.bfloat16, kind="ExternalInput")
    # layer 1 (tail: weights stream as moving operand)
    W["wq1"] = nc.dram_tensor("wq1", [KC, 128, DL], dt.bfloat16, kind="ExternalInput")
    W["wo1"] = nc.dram_tensor("wo1", [NH, 128, D], dt.bfloat16, kind="ExternalInput")
    W["wgu1"] = nc.dram_tensor("wgu1", [KC, 128, 2 * FL], dt.bfloat16, kind="ExternalInput")
    W["wd1"] = nc.dram_tensor("wd1", [FC, 128, D], dt.bfloat16, kind="ExternalInput")
    out_dram = nc.dram_tensor("out", [OUT, 1], dt.float32, kind="ExternalOutput")

    with tile.TileContext(nc) as tc:
        with (
            tc.tile_pool(name="persist", bufs=1) as pp,
            tc.tile_pool(name="wts", bufs=5) as wp,
            tc.tile_pool(name="colw", bufs=8) as cwp,
            tc.tile_pool(name="tails", bufs=3) as tsp,
            tc.tile_pool(name="tmp", bufs=3) as tp_,
            tc.tile_pool(name="stage", bufs=3) as stp,
            tc.tile_pool(name="psum", bufs=8, space="PSUM") as ps,
            tc.tile_pool(name="dram", bufs=1, space="DRAM") as dram,
        ):
            f32, bf = dt.float32, dt.bfloat16
            # ---- persistent tiles ----
            h = pp.tile([128, KC, S], bf, tag="h")
            hn = pp.tile([128, KC, S], bf, tag="hn")
            cos_sb = pp.tile([128, S], bf, tag="cos")
            sin_sb = pp.tile([128, S], bf, tag="sin")
            mstrip_sb = pp.tile([128, 896], bf, tag="mstrip")
            mb_sb = pp.tile([128, TC], f32, tag="mb")
            m01_sb = pp.tile([128, TC], f32, tag="m01")
            cos_last = pp.tile([128, 1], f32, tag="cos_last")
            sin_last = pp.tile([128, 1], f32, tag="sin_last")
            ones_bf = pp.tile([128, 1], bf, tag="onesbf")
            allones = pp.tile([128, 128], bf, tag="allones")
            oneD128 = pp.tile([128, 128], bf, tag="oneD128")
            one1_bf = pp.tile([1, 1], bf, tag="one1")
            one64_bf = pp.tile([1, 1], bf, tag="one64")
            eps128 = pp.tile([128, 1], f32, tag="eps128")
            eps1 = pp.tile([1, 1], f32, tag="eps1")
            qT = pp.tile([128, NH, S], bf, tag="qT")     # q, then reused for ctx
            kT = pp.tile([128, NH, S], bf, tag="kT")
            vN = pp.tile([128, TC, DL], bf, tag="vN")
            expT = pp.tile([128, TC, TT], bf, tag="expT")
            mT = pp.tile([128, FC, TT], bf, tag="mT")    # per-half mlp act
            aqw = pp.tile([128, KC, R], bf, tag="aqw")
            avw = pp.tile([128, KC, R], bf, tag="avw")
            bq_sb = pp.tile([R, DL], bf, tag="bq")
            bv_sb = pp.tile([R, DL], bf, tag="bv")
            aq_sb = pp.tile([R, S], bf, tag="aq")
            av_sb = pp.tile([R, S], bf, tag="av")
            wreg_sb = pp.tile([128, KC, OUT], bf, tag="wreg")
            breg_sb = pp.tile([OUT, 1], f32, tag="breg")
            # tail smalls
            hn_last = pp.tile([128, KC, 1], bf, tag="hn_last")
            q_last = pp.tile([128, NH], bf, tag="q_last")
            ctx_n = pp.tile([128, NH], bf, tag="ctx_n")
            exp_tail = pp.tile([128, NH, TC], bf, tag="exp_tail")
            m_pm = pp.tile([128, FC], bf, tag="m_pm")

            # ---- constants in ----
            nc.vector.memset(ones_bf[:], 1.0)
            nc.vector.memset(allones[:], 1.0)
            nc.vector.memset(oneD128[:], 1.0 / D)
            nc.vector.memset(one1_bf[:], 1.0)
            nc.vector.memset(one64_bf[:], 1.0 / 64.0)
            nc.vector.memset(eps128[:], EPS)
            nc.vector.memset(eps1[:], EPS)
            nc.sync.dma_start(cos_sb[:], cosT[:])
            nc.sync.dma_start(sin_sb[:], sinT[:])
            nc.sync.dma_start(mstrip_sb[:], mstrip[:])
            nc.sync.dma_start(mb_sb[:], maskbias[:])
            nc.sync.dma_start(m01_sb[:], mask01[:])
            nc.scalar.copy(cos_last[:], cos_sb[:, S - 1:S])
            nc.scalar.copy(sin_last[:], sin_sb[:, S - 1:S])
            nc.sync.dma_start(breg_sb[:], breg[:])
            for k in range(KC):
                nc.sync.dma_start(wreg_sb[:, k, :], wreg[k])

            # ---- h init per half: bf16 upload -> fp32 residual ----
            for t in range(NT):
                ts_ = slice(t * TT, (t + 1) * TT)
                nc.sync.dma_start(h[:, :, ts_], xT[:, :, ts_])

            # DRAM bounce buffers for collectives
            ar1h_in = [dram.tile([128, KC, TT], bf, name=f"ar1hi_{t}") for t in range(NT)]
            ar1h_out = [dram.tile([128, KC, TT], bf, name=f"ar1ho_{t}") for t in range(NT)]
            ar2h_in = [dram.tile([128, KC, TT], bf, name=f"ar2hi_{t}") for t in range(NT)]
            ar2h_out = [dram.tile([128, KC, TT], bf, name=f"ar2ho_{t}") for t in range(NT)]
            ar_in_s = dram.tile([128, KC, 1], bf)
            ar_out_s = dram.tile([128, KC, 1], bf)

            def norm_half(t):
                """hn[:, :, half t] = h / sqrt(mean(h^2) + eps), bf16."""
                ts_ = slice(t * TT, (t + 1) * TT)
                psb = ps.tile([128, TT], f32, tag="psum", name=f"nps_{t}")
                for k in range(KC):
                    sq = tp_.tile([128, TT], bf, tag="sq", bufs=2, name=f"nsq_{k}_{t}")
                    nc.scalar.activation(sq[:], h[:, k, ts_],
                                         mybir.ActivationFunctionType.Square)
                    # all-ones/D stationary => result broadcast to all partitions
                    nc.tensor.matmul(psb[:], oneD128[:], sq[:],
                                     start=(k == 0), stop=(k == KC - 1))
                rs = tp_.tile([128, TT], f32, tag="rsbc", bufs=1, name=f"nrs_{t}")
                nc.scalar.activation(rs[:], psb[:],
                                     mybir.ActivationFunctionType.Sqrt, bias=eps128[:])
                nc.vector.reciprocal_approx_fast(rs[:], rs[:])
                for k in range(KC):
                    nc.vector.tensor_mul(hn[:, k, ts_], h[:, k, ts_], rs[:])

            def lora_down_half(aw, dst, t):
                psa = ps.tile([128, TT], f32, tag="psum", name=f"ldh_{id(aw)}_{t}")
                for k in range(KC):
                    nc.tensor.matmul(
                        psa[0:R, :], aw[:, k, :], hn[:, k, t * TT:(t + 1) * TT],
                        start=(k == 0), stop=(k == KC - 1),
                    )
                nc.scalar.copy(dst[:, t * TT:(t + 1) * TT], psa[0:R, :])

            def rope_from_psum(psq, dst, hc, t):
                """Apply RoPE to psum [128,TT] (one head, token tile t) and
                write bf16 into dst[:, hc, t*TT:...]."""
                ts_ = slice(t * TT, (t + 1) * TT)
                t2 = tp_.tile([128, TT], bf, tag="ropetB", bufs=1)
                t4 = tp_.tile([128, TT], bf, tag="ropetB", bufs=1)
                nc.vector.tensor_mul(dst[0:64, hc, ts_], psq[0:64, :], cos_sb[0:64, ts_])
                nc.vector.tensor_mul(t2[0:64, :], psq[64:128, :], sin_sb[0:64, ts_])
                nc.vector.tensor_sub(dst[0:64, hc, ts_], dst[0:64, hc, ts_], t2[0:64, :])
                nc.vector.tensor_mul(dst[64:128, hc, ts_], psq[64:128, :], cos_sb[64:128, ts_])
                nc.vector.tensor_mul(t4[64:128, :], psq[0:64, :], sin_sb[64:128, ts_])
                nc.vector.tensor_add(dst[64:128, hc, ts_], dst[64:128, hc, ts_], t4[64:128, :])

            def qk_proj(wname, dst, lora_bw, lora_act):
                """dst[:, hc, :] (bf16, roped) = rope(W.T @ hn [+ lora])."""
                psq = [[ps.tile([128, TT], f32, tag="psum", name=f"psq_{wname}_{hc}_{t}")
                        for t in range(NT)] for hc in range(NH)]
                for kk in range(KC // 2):
                    wt = wp.tile([128, 2, DL], bf, tag="wqkv", name=f"w_{wname}_{kk}")
                    nc.sync.dma_start(wt[:], W[wname][2 * kk:2 * kk + 2].rearrange("i p c -> p i c"))
                    for i in range(2):
                        k = 2 * kk + i
                        for hc in range(NH):
                            for t in range(NT):
                                nc.tensor.matmul(
                                    psq[hc][t][:], wt[:, i, hc * HD:(hc + 1) * HD],
                                    hn[:, k, t * TT:(t + 1) * TT],
                                    start=(k == 0),
                                    stop=(lora_bw is None and k == KC - 1),
                                )
                for hc in range(NH):
                    for t in range(NT):
                        if lora_bw is not None:
                            nc.tensor.matmul(
                                psq[hc][t][:], lora_bw[:, hc * HD:(hc + 1) * HD],
                                lora_act[:, t * TT:(t + 1) * TT],
                                start=False, stop=True,
                            )
                        rope_from_psum(psq[hc][t], dst, hc, t)

            def v_proj(l):
                """vN [128(tok), TC, DL] bf16 = hn.T @ Wv + lora."""
                psv = [ps.tile([128, DL], f32, tag="psum", name=f"psv_{c}")
                       for c in range(TC)]
                for kk in range(KC // 2):
                    wt = wp.tile([128, 2, DL], bf, tag="wqkv", name=f"wv_t_{kk}")
                    nc.sync.dma_start(wt[:], W[f"wv{l}"][2 * kk:2 * kk + 2].rearrange("i p c -> p i c"))
                    for i in range(2):
                        k = 2 * kk + i
                        for c in range(TC):
                            nc.tensor.matmul(
                                psv[c][:], hn[:, k, c * 128:(c + 1) * 128], wt[:, i, :],
                                start=(k == 0), stop=False,
                            )
                for c in range(TC):
                    nc.tensor.matmul(
                        psv[c][:], av_sb[:, c * 128:(c + 1) * 128], bv_sb[:],
                        start=False, stop=True,
                    )
                    nc.scalar.copy(vN[:, c, :], psv[c][:])

            def q_proj_half(wname, dst, t):
                """dst[:, hc, half t] = rope(Wq.T @ hn + lora)."""
                psq = [ps.tile([128, TT], f32, tag="psum", name=f"psqq_{hc}_{t}")
                       for hc in range(NH)]
                for kk in range(KC // 2):
                    wt = wp.tile([128, 2, DL], bf, tag="wqkv", name=f"wq_{kk}_{t}")
                    nc.sync.dma_start(wt[:], W[wname][2 * kk:2 * kk + 2].rearrange("i p c -> p i c"))
                    for i in range(2):
                        k = 2 * kk + i
                        for hc in range(NH):
                            nc.tensor.matmul(
                                psq[hc][:], wt[:, i, hc * HD:(hc + 1) * HD],
                                hn[:, k, t * TT:(t + 1) * TT],
                                start=(k == 0), stop=False,
                            )
                for hc in range(NH):
                    nc.tensor.matmul(
                        psq[hc][:], bq_sb[:, hc * HD:(hc + 1) * HD],
                        aq_sb[:, t * TT:(t + 1) * TT],
                        start=False, stop=True,
                    )
                    rope_from_psum(psq[hc], dst, hc, t)

            def attention_half(t):
                """qT,kT,vN -> ctx (written into qT) for token half t.

                The denominator/ctx accumulation matmuls for chunk j are
                emitted two chunks behind the QK+exp chain, so by the time
                the PE (strict in-order queue) reaches them their exp input
                has drained from the ACT engine and nothing head-of-line
                blocks."""
                ts_ = slice(t * TT, (t + 1) * TT)
                jmax = (t + 1) * (TT // 128)
                LAG = 2
                for hh in range(NH):
                    psb = ps.tile([128, TT], f32, tag="psum", name=f"psd_{hh}_{t}")
                    psc = ps.tile([128, TT], f32, tag="psum", name=f"psc_{hh}_{t}")

                    def emit_reduce_j(j):
                        nc.tensor.matmul(
                            psb[:], allones[:], expT[:, j, :],
                            start=(j == 0), stop=(j == jmax - 1),
                        )
                        nc.tensor.matmul(
                            psc[:], vN[:, j, hh * HD:(hh + 1) * HD],
                            expT[:, j, :],
                            start=(j == 0), stop=(j == jmax - 1),
                        )

                    for j in range(jmax):
                        pss = ps.tile([128, TT], f32, tag="psum",
                                      name=f"pss_{hh}_{t}_{j}")
                        nc.tensor.matmul(
                            pss[:], kT[:, hh, j * 128:(j + 1) * 128],
                            qT[:, hh, ts_], start=True, stop=True,
                        )
                        nc.scalar.activation(
                            expT[:, j, :], pss[:],
                            mybir.ActivationFunctionType.Exp,
                            bias=mb_sb[:, j:j + 1], scale=1.0,
                        )
                        off = t * TT - j * 128
                        if off < 128:
                            nc.vector.tensor_mul(
                                expT[:, j, :], expT[:, j, :],
                                mstrip_sb[:, 384 + off:896 + off],
                            )
                        if j >= LAG:
                            emit_reduce_j(j - LAG)
                    for j in range(max(0, jmax - LAG), jmax):
                        emit_reduce_j(j)
                    rden = tp_.tile([128, TT], f32, tag="rsbc", bufs=1,
                                    name=f"rden_{hh}_{t}")
                    nc.vector.reciprocal_approx_fast(rden[:], psb[:])
                    nc.vector.tensor_mul(qT[:, hh, ts_], psc[:], rden[:])

            def out_proj_half(l, t):
                """attn partial for token half t -> ar1h_in[t]."""
                for og in range(4):  # groups of 4 output chunks
                    pso = [ps.tile([128, TT], f32, tag="psum", name=f"pso_{og}_{oi}_{t}")
                           for oi in range(4)]
                    for hp in range(NH // 2):
                        wt = wp.tile([128, 2, TT], bf, tag="wqkv",
                                     name=f"wo_t_{og}_{hp}_{t}")
                        nc.sync.dma_start(
                            wt[:], W[f"wo{l}"][2 * hp:2 * hp + 2,
                                               :, og * 512:(og + 1) * 512]
                            .rearrange("i p c -> p i c"))
                        for i in range(2):
                            hc = 2 * hp + i
                            for oi in range(4):
                                nc.tensor.matmul(
                                    pso[oi][:], wt[:, i, oi * 128:(oi + 1) * 128],
                                    qT[:, hc, t * TT:(t + 1) * TT],
                                    start=(hc == 0), stop=(hc == NH - 1),
                                )
                    for oi in range(4):
                        st = stp.tile([128, TT], bf, tag="stage")
                        nc.vector.tensor_copy(st[:], pso[oi][:])
                        nc.sync.dma_start(ar1h_in[t][:, og * 4 + oi, :], st[:])

            def allreduce1_half(t):
                nc.gpsimd.collective_compute(
                    "AllReduce", mybir.AluOpType.add,
                    replica_groups=REPLICA_GROUPS,
                    ins=[ar1h_in[t].opt()], outs=[ar1h_out[t].opt()],
                )

            def add1_half(t):
                # landing DMA rides the scalar HWDGE ring so its AR wait can't
                # head-of-line-block weight loads on the sync ring
                ts_ = slice(t * TT, (t + 1) * TT)
                nc.scalar.dma_start(hn[:, :, ts_], ar1h_out[t][:])
                for k in range(KC):
                    nc.vector.tensor_add(h[:, k, ts_], h[:, k, ts_], hn[:, k, ts_])

            def allreduce2_half(t):
                nc.gpsimd.collective_compute(
                    "AllReduce", mybir.AluOpType.add,
                    replica_groups=REPLICA_GROUPS,
                    ins=[ar2h_in[t].opt()], outs=[ar2h_out[t].opt()],
                )

            def add2_half(t):
                ts_ = slice(t * TT, (t + 1) * TT)
                nc.scalar.dma_start(hn[:, :, ts_], ar2h_out[t][:])
                for k in range(KC):
                    nc.vector.tensor_add(h[:, k, ts_], h[:, k, ts_], hn[:, k, ts_])

            def k_proj_half(wname, dst, t, tag="wqkv"):
                psq = [ps.tile([128, TT], f32, tag="psum", name=f"psqh_{hc}_{t}")
                       for hc in range(NH)]
                for kk in range(KC // 2):
                    wt = wp.tile([128, 2, DL], bf, tag=tag, name=f"wkh_{wname}_{kk}_{t}")
                    nc.sync.dma_start(wt[:], W[wname][2 * kk:2 * kk + 2].rearrange("i p c -> p i c"))
                    for i in range(2):
                        k = 2 * kk + i
                        for hc in range(NH):
                            nc.tensor.matmul(
                                psq[hc][:], wt[:, i, hc * HD:(hc + 1) * HD],
                                hn[:, k, t * TT:(t + 1) * TT],
                                start=(k == 0), stop=(k == KC - 1),
                            )
                for hc in range(NH):
                    rope_from_psum(psq[hc], dst, hc, t)

            def v_proj_half(l, t, tag="wqkv"):
                psv = [ps.tile([128, DL], f32, tag="psum", name=f"psvh_{c}_{t}")
                       for c in range(4)]
                for kk in range(KC // 2):
                    wt = wp.tile([128, 2, DL], bf, tag=tag, name=f"wvh_{l}_{kk}_{t}")
                    nc.sync.dma_start(wt[:], W[f"wv{l}"][2 * kk:2 * kk + 2].rearrange("i p c -> p i c"))
                    for i in range(2):
                        k = 2 * kk + i
                        for ci in range(4):
                            c = t * 4 + ci
                            nc.tensor.matmul(
                                psv[ci][:], hn[:, k, c * 128:(c + 1) * 128], wt[:, i, :],
                                start=(k == 0), stop=False,
                            )
                for ci in range(4):
                    c = t * 4 + ci
                    nc.tensor.matmul(
                        psv[ci][:], av_sb[:, c * 128:(c + 1) * 128], bv_sb[:],
                        start=False, stop=True,
                    )
                    nc.scalar.copy(vN[:, c, :], psv[ci][:])

            def mlp_gate_up_half(l, t):
                ts_ = slice(t * TT, (t + 1) * TT)
                for fc in range(FC):
                    psg = ps.tile([128, TT], f32, tag="psum", name=f"psg_{fc}_{t}")
                    psu = ps.tile([128, TT], f32, tag="psum", name=f"psu_{fc}_{t}")
                    for kh in range(2):
                        wgu = cwp.tile([128, KC // 2, 256], bf, tag="wgu",
                                       name=f"wgu_{fc}_{kh}_{t}")
                        nc.sync.dma_start(
                            wgu[:], W[f"wgu{l}"][fc][:, kh * (KC // 2):(kh + 1) * (KC // 2), :])
                        for ki in range(KC // 2):
                            k = kh * (KC // 2) + ki
                            nc.tensor.matmul(psg[:], wgu[:, ki, 0:128], hn[:, k, ts_],
                                             start=(k == 0), stop=(k == KC - 1))
                            nc.tensor.matmul(psu[:], wgu[:, ki, 128:256], hn[:, k, ts_],
                                             start=(k == 0), stop=(k == KC - 1))
                    sg = tp_.tile([128, TT], bf, tag="silu", bufs=1, name=f"sg_{fc}_{t}")
                    nc.scalar.activation(sg[:], psg[:], mybir.ActivationFunctionType.Silu)
                    nc.vector.tensor_mul(mT[:, fc, :], sg[:], psu[:])

            def mlp_down_half(l, t):
                for og in range(4):
                    pso = [ps.tile([128, TT], f32, tag="psum", name=f"psd_{og}_{oi}_{t}")
                           for oi in range(4)]
                    for kp in range((FC + 1) // 2):
                        nk = min(2, FC - 2 * kp)
                        wt = wp.tile([128, 2, TT], bf, tag="wqkv",
                                     name=f"wd_t_{og}_{kp}_{t}")
                        nc.sync.dma_start(
                            wt[:, 0:nk, :],
                            W[f"wd{l}"][2 * kp:2 * kp + nk,
                                        :, og * 512:(og + 1) * 512]
                            .rearrange("i p c -> p i c"))
                        for i in range(nk):
                            kc = 2 * kp + i
                            for oi in range(4):
                                nc.tensor.matmul(
                                    pso[oi][:], wt[:, i, oi * 128:(oi + 1) * 128],
                                    mT[:, kc, :],
                                    start=(kc == 0), stop=(kc == FC - 1),
                                )
                    for oi in range(4):
                        st = stp.tile([128, TT], bf, tag="stage")
                        nc.vector.tensor_copy(st[:], pso[oi][:])
                        nc.sync.dma_start(ar2h_in[t][:, og * 4 + oi, :], st[:])

            # ---------- tail (last layer): only the last token flows through
            # q/attention/Wo/MLP. Matvecs are transposed: the activation
            # column is the stationary operand, weights stream as rhs.

            def row_to_pm(row_sb, psum_pm, ncols):
                """psum_pm[:, c] = row_sb[0, c*128:(c+1)*128].T via K=1 matmuls."""
                for c in range(ncols):
                    nc.tensor.matmul(
                        psum_pm[:, c:c + 1],
                        row_sb[0:1, c * 128:(c + 1) * 128], one1_bf[:],
                        start=True, stop=True,
                    )

            def q_tail():
                psq = ps.tile([128, TT], f32, tag="psum", name="psq_tail")
                for kk in range(KC // 2):
                    wt = wp.tile([128, 2, DL], bf, tag="wqkv", name=f"wq1s_{kk}")
                    nc.sync.dma_start(wt[:], W["wq1"][2 * kk:2 * kk + 2].rearrange("i p c -> p i c"))
                    for i in range(2):
                        k = 2 * kk + i
                        nc.tensor.matmul(psq[0:1, :], hn[:, k, S - 1:S], wt[:, i, :],
                                         start=(k == 0), stop=False)
                nc.tensor.matmul(psq[0:1, :], aq_sb[:, S - 1:S], bq_sb[:],
                                 start=False, stop=True)
                qrow = tp_.tile([1, DL], bf, tag="qrow", bufs=1)
                nc.scalar.copy(qrow[:], psq[0:1, :])
                psqpm = ps.tile([128, NH], f32, tag="psum", name="psqpm")
                row_to_pm(qrow, psqpm, NH)
                # rope, all heads at once (per-partition cos/sin scalars)
                t2 = tp_.tile([128, NH], bf, tag="ropeS", bufs=2)
                t4 = tp_.tile([128, NH], bf, tag="ropeS", bufs=2)
                nc.vector.tensor_scalar_mul(q_last[0:64, :], psqpm[0:64, :],
                                            cos_last[0:64, :])
                nc.vector.tensor_scalar_mul(t2[0:64, :], psqpm[64:128, :],
                                            sin_last[0:64, :])
                nc.vector.tensor_sub(q_last[0:64, :], q_last[0:64, :], t2[0:64, :])
                nc.vector.tensor_scalar_mul(q_last[64:128, :], psqpm[64:128, :],
                                            cos_last[64:128, :])
                nc.vector.tensor_scalar_mul(t4[64:128, :], psqpm[0:64, :],
                                            sin_last[64:128, :])
                nc.vector.tensor_add(q_last[64:128, :], q_last[64:128, :], t4[64:128, :])

            def attention_tail():
                # scores for all heads/key-chunks in one psum tile
                pss = ps.tile([128, NH, TC], f32, tag="psum", name="pss_tail")
                for hh in range(NH):
                    for j in range(TC):
                        nc.tensor.matmul(
                            pss[:, hh, j:j + 1],
                            kT[:, hh, j * 128:(j + 1) * 128], q_last[:, hh:hh + 1],
                            start=True, stop=True,
                        )
                nc.scalar.activation(exp_tail[:], pss[:],
                                     mybir.ActivationFunctionType.Exp)
                for j in range(TC):
                    nc.vector.tensor_scalar_mul(exp_tail[:, :, j], exp_tail[:, :, j],
                                                m01_sb[:, j:j + 1])
                # denominators: all-partition sums then free-dim reduce per head
                psdb = ps.tile([128, NH, TC], f32, tag="psum", name="psdb_tail")
                nc.tensor.matmul(psdb[:], allones[:], exp_tail[:],
                                 start=True, stop=True)
                rdt = tp_.tile([128, NH], f32, tag="rdt")
                nc.vector.tensor_reduce(
                    rdt[:], psdb[:],
                    axis=mybir.AxisListType.X, op=mybir.AluOpType.add,
                )
                nc.vector.reciprocal_approx_fast(rdt[:], rdt[:])
                psc = ps.tile([128, NH], f32, tag="psum", name="psc_tail")
                for hh in range(NH):
                    for j in range(TC):
                        nc.tensor.matmul(
                            psc[:, hh:hh + 1], vN[:, j, hh * HD:(hh + 1) * HD],
                            exp_tail[:, hh, j:j + 1],
                            start=(j == 0), stop=(j == TC - 1),
                        )
                nc.vector.tensor_mul(ctx_n[:], psc[:], rdt[:])

            def wo_tail():
                pso = [ps.tile([128, TT], f32, tag="psum", name=f"psot_{c}")
                       for c in range(DC)]
                for hc in range(NH):
                    for dh in range(2):
                        wt = tsp.tile([128, 1024], bf, tag="wod1s",
                                      name=f"wo1s_{hc}_{dh}")
                        nc.sync.dma_start(wt[:], W["wo1"][hc][:, dh * 1024:(dh + 1) * 1024])
                        for ci in range(2):
                            c = dh * 2 + ci
                            nc.tensor.matmul(
                                pso[c][0:1, :], ctx_n[:, hc:hc + 1],
                                wt[:, ci * 512:(ci + 1) * 512],
                                start=(hc == 0), stop=(hc == NH - 1),
                            )
                orow = tp_.tile([1, D], bf, tag="orow", bufs=1)
                for c in range(DC):
                    nc.scalar.copy(orow[0:1, c * 512:(c + 1) * 512], pso[c][0:1, :])
                pst = ps.tile([128, KC], f32, tag="psum", name="pst_wo")
                row_to_pm(orow, pst, KC)
                stpm = tp_.tile([128, KC], bf, tag="stpm", bufs=2, name="stpm_wo")
                nc.vector.tensor_copy(stpm[:], pst[:])
                nc.sync.dma_start(ar_in_s[:, :, 0], stpm[:])

            def allreduce_and_add_tail():
                nc.gpsimd.collective_compute(
                    "AllReduce", mybir.AluOpType.add,
                    replica_groups=REPLICA_GROUPS,
                    ins=[ar_in_s.opt()], outs=[ar_out_s.opt()],
                )
                lb = tp_.tile([128, KC, 1], bf, tag="ar_land", bufs=2)
                nc.scalar.dma_start(lb[:], ar_out_s[:])
                nc.vector.tensor_add(h[:, :, S - 1:S], h[:, :, S - 1:S], lb[:])

            def norm_tail_to_hn_last():
                sqt = tp_.tile([128, KC, 1], bf, tag="sqlast")
                nc.scalar.activation(sqt[:], h[:, :, S - 1:S],
                                     mybir.ActivationFunctionType.Square)
                psl = ps.tile([128, 1], f32, tag="psum", name="psl_normt")
                for k in range(KC):
                    nc.tensor.matmul(psl[:], oneD128[:], sqt[:, k, :],
                                     start=(k == 0), stop=(k == KC - 1))
                rsb = tp_.tile([128, 1], f32, tag="rsb_tail", bufs=2)
                nc.scalar.activation(rsb[:], psl[:],
                                     mybir.ActivationFunctionType.Sqrt, bias=eps128[:])
                nc.vector.reciprocal_approx_fast(rsb[:], rsb[:])
                nc.vector.tensor_scalar_mul(hn_last[:], h[:, :, S - 1:S], rsb[:])

            def mlp_tail():
                norm_tail_to_hn_last()
                # gate/up: stream combined gate|up weights as rhs
                psg = [ps.tile([128, TT], f32, tag="psum", name=f"psgt_{c}")
                       for c in range(len(FLCH))]
                psu = [ps.tile([128, TT], f32, tag="psum", name=f"psut_{c}")
                       for c in range(len(FLCH))]
                for k in range(KC):
                    wgt = tsp.tile([128, FL], bf, tag="wg1s", name=f"wg1s_{k}")
                    wut = tsp.tile([128, FL], bf, tag="wu1s", name=f"wu1s_{k}")
                    nc.sync.dma_start(wgt[:], W["wgu1"][k][:, 0:FL])
                    nc.sync.dma_start(wut[:], W["wgu1"][k][:, FL:2 * FL])
                    for c, (off, sz) in enumerate(FLCH):
                        nc.tensor.matmul(psg[c][0:1, 0:sz], hn_last[:, k, :],
                                         wgt[:, off:off + sz],
                                         start=(k == 0), stop=(k == KC - 1))
                        nc.tensor.matmul(psu[c][0:1, 0:sz], hn_last[:, k, :],
                                         wut[:, off:off + sz],
                                         start=(k == 0), stop=(k == KC - 1))
                mrow = tp_.tile([1, FL], bf, tag="mrow", bufs=1)
                for c, (off, sz) in enumerate(FLCH):
                    grow = tp_.tile([1, TT], bf, tag="grow", bufs=1, name=f"grow_{c}")
                    nc.scalar.activation(grow[0:1, 0:sz], psg[c][0:1, 0:sz],
                                         mybir.ActivationFunctionType.Silu)
                    nc.vector.tensor_mul(mrow[0:1, off:off + sz], grow[0:1, 0:sz],
                                         psu[c][0:1, 0:sz])
                psm = ps.tile([128, FC], f32, tag="psum", name="psm_pm")
                row_to_pm(mrow, psm, FC)
                nc.vector.tensor_copy(m_pm[:], psm[:])
                # down
                psd = [ps.tile([128, TT], f32, tag="psum", name=f"psdt2_{c}")
                       for c in range(DC)]
                for fc in range(FC):
                    for dh in range(2):
                        wdt = tsp.tile([128, 1024], bf, tag="wod1s",
                                       name=f"wd1s_{fc}_{dh}")
                        nc.sync.dma_start(wdt[:],
                                          W["wd1"][fc][:, dh * 1024:(dh + 1) * 1024])
                        for ci in range(2):
                            c = dh * 2 + ci
                            nc.tensor.matmul(
                                psd[c][0:1, :], m_pm[:, fc:fc + 1],
                                wdt[:, ci * 512:(ci + 1) * 512],
                                start=(fc == 0), stop=(fc == FC - 1),
                            )
                orow = tp_.tile([1, D], bf, tag="orow", bufs=1)
                for c in range(DC):
                    nc.scalar.copy(orow[0:1, c * 512:(c + 1) * 512], psd[c][0:1, :])
                pst = ps.tile([128, KC], f32, tag="psum", name="pst_mlp")
                row_to_pm(orow, pst, KC)
                stpm = tp_.tile([128, KC], bf, tag="stpm", bufs=2, name="stpm_mlp")
                nc.vector.tensor_copy(stpm[:], pst[:])
                nc.sync.dma_start(ar_in_s[:, :, 0], stpm[:])

            # ================= layer 0 (all per token half, so the first
            # attn AllReduce fires as early as possible) =================
            for k in range(KC):
                nc.sync.dma_start(aqw[:, k, :], W["aq0"][k])
                nc.sync.dma_start(avw[:, k, :], W["av0"][k])
            nc.sync.dma_start(bq_sb[:], W["bq0"][:])
            nc.sync.dma_start(bv_sb[:], W["bv0"][:])
            for t in range(NT):
                norm_half(t)
                lora_down_half(avw, av_sb, t)
                lora_down_half(aqw, aq_sb, t)
                k_proj_half("wk0", kT, t)
                v_proj_half(0, t)
                q_proj_half("wq0", qT, t)
                attention_half(t)
                out_proj_half(0, t)
                allreduce1_half(t)
            for t in range(NT):
                add1_half(t)
                norm_half(t)
                mlp_gate_up_half(0, t)
                mlp_down_half(0, t)
                allreduce2_half(t)

            # ================= layer 1 (tail) =================
            for k in range(KC):
                nc.sync.dma_start(aqw[:, k, :], W["aq1"][k])
                nc.sync.dma_start(avw[:, k, :], W["av1"][k])
            nc.sync.dma_start(bq_sb[:], W["bq1"][:])
            nc.sync.dma_start(bv_sb[:], W["bv1"][:])
            for t in range(NT):
                add2_half(t)
                norm_half(t)
                lora_down_half(avw, av_sb, t)
                if t == NT - 1:
                    lora_down_half(aqw, aq_sb, t)
                k_proj_half("wk1", kT, t)
                v_proj_half(1, t)
            q_tail()
            attention_tail()
            wo_tail()
            allreduce_and_add_tail()
            mlp_tail()
            allreduce_and_add_tail()

            # ================= final norm + head (last token only) ========
            sq = tp_.tile([128, KC, 1], bf, tag="sqlast")
            nc.scalar.activation(sq[:], h[:, :, S - 1:S],
                                 mybir.ActivationFunctionType.Square)
            psl = ps.tile([128, 1], f32, tag="psum", name="psl_final")
            for k in range(KC):
                nc.tensor.matmul(psl[:], oneD128[:], sq[:, k, :],
                                 start=(k == 0), stop=(k == KC - 1))
            rsl = tp_.tile([128, 1], f32, tag="rsl_final")
            nc.scalar.activation(rsl[:], psl[:],
                                 mybir.ActivationFunctionType.Sqrt, bias=eps128[:])
            nc.vector.reciprocal_approx_fast(rsl[:], rsl[:])
            hl = tp_.tile([128, KC, 1], bf, tag="hlast")
            nc.vector.tensor_scalar_mul(hl[:], h[:, :, S - 1:S], rsl[:])
            pso = ps.tile([128, TT], f32, tag="psum", name="ps_head")
            for k in range(KC):
                nc.tensor.matmul(pso[0:OUT, 0:1], wreg_sb[:, k, :], hl[:, k, :],
                                 start=(k == 0), stop=(k == KC - 1))
            ot = tp_.tile([OUT, 1], f32, tag="outt")
            nc.vector.tensor_add(ot[:], pso[0:OUT, 0:1], breg_sb[:])
            nc.sync.dma_start(out_dram[:], ot[:])

    nc.finalize()
    return nc


_CACHED = {}


def _get_program():
    if "nc" not in _CACHED:
        _CACHED["nc"] = build_program()
    return _CACHED["nc"]


def _host_prepare(inputs):
    """Fold norms/scales into weights, gather embeddings, build the 8
    per-core input maps."""
    ids = np.asarray(inputs["input_ids"]).astype(np.int64)        # [B,S]
    amask = np.asarray(inputs["attention_mask"]).astype(np.int64)  # [B,S]
    embed = np.asarray(inputs["embed"], FP32)

    inv_sqrt_hd = FP32(1.0 / np.sqrt(HD))

    # rope tables (half: both halves identical)
    inv = 1.0 / (10000.0 ** (np.arange(0, HD, 2, dtype=np.float64) / HD))
    ang = (np.arange(S, dtype=np.float64)[:, None] * inv[None, :])  # [S,64]
    cos64 = np.cos(ang).T.astype(BF16)  # [64,S]
    sin64 = np.sin(ang).T.astype(BF16)
    cosT = np.concatenate([cos64, cos64], axis=0).copy()  # [128,S]
    sinT = np.concatenate([sin64, sin64], axis=0).copy()

    # causal strip [128, 896]: strip[p,u] = 1 if (u-384) >= p else 0
    u = np.arange(896)[None, :]
    p = np.arange(128)[:, None]
    mstrip = ((u - 384) >= p).astype(BF16)

    common = {}

    def fold(l):
        g1 = np.asarray(inputs["norm1"], FP32)[l][:, None]
        g2 = np.asarray(inputs["norm2"], FP32)[l][:, None]
        wq = np.asarray(inputs["Wq"], FP32)[l] * g1 * inv_sqrt_hd
        wk = np.asarray(inputs["Wk"], FP32)[l] * g1
        wv = np.asarray(inputs["Wv"], FP32)[l] * g1
        aq = np.asarray(inputs["Aq"], FP32)[l] * g1
        av = np.asarray(inputs["Av"], FP32)[l] * g1
        bq = np.asarray(inputs["Bq"], FP32)[l] * (SCALING * inv_sqrt_hd)
        bv = np.asarray(inputs["Bv"], FP32)[l] * SCALING
        wo = np.asarray(inputs["Wo"], FP32)[l]
        wg = np.asarray(inputs["Wgate"], FP32)[l] * g2
        wu = np.asarray(inputs["Wup"], FP32)[l] * g2
        wd = np.asarray(inputs["Wdown"], FP32)[l]
        return wq, wk, wv, aq, av, bq, bv, wo, wg, wu, wd

    folded = [fold(l) for l in range(L)]
    wregf = (np.asarray(inputs["Wreg"], FP32) * np.asarray(inputs["norm_f"], FP32)[:, None])
    common["wreg"] = wregf.reshape(KC, 128, OUT).astype(BF16)
    common["breg"] = np.asarray(inputs["breg"], FP32).reshape(OUT, 1)
    common["cosT"] = cosT
    common["sinT"] = sinT
    common["mstrip"] = mstrip

    in_maps = []
    for c in range(N_CORES):
        b = c // TP      # batch index (DP group)
        r = c % TP       # TP rank within group
        m = dict(common)
        # embedding gather, transposed, bf16: [D,S] -> [16,128,S] -> [128,16,S]
        xt = embed[ids[b]].T.reshape(KC, 128, S).transpose(1, 0, 2)
        m["xT"] = np.ascontiguousarray(xt).astype(BF16)
        # attention_mask bias [128, TC]: col j, part p -> key token 128j+p
        mb = np.where(amask[b] > 0, FP32(0), FP32(-1e9)).reshape(TC, 128).T
        m["maskbias"] = np.ascontiguousarray(mb)
        m["mask01"] = np.ascontiguousarray((amask[b] > 0).reshape(TC, 128).T).astype(FP32)
        for l in range(L):
            wq, wk, wv, aq, av, bq, bv, wo, wg, wu, wd = folded[l]
            dsl = slice(r * DL, (r + 1) * DL)
            fsl = slice(r * FL, (r + 1) * FL)
            m[f"wk{l}"] = np.ascontiguousarray(wk[:, dsl].reshape(KC, 128, DL)).astype(BF16)
            m[f"wv{l}"] = np.ascontiguousarray(wv[:, dsl].reshape(KC, 128, DL)).astype(BF16)
            m[f"aq{l}"] = np.ascontiguousarray(aq.reshape(KC, 128, R)).astype(BF16)
            m[f"av{l}"] = np.ascontiguousarray(av.reshape(KC, 128, R)).astype(BF16)
            m[f"bq{l}"] = np.ascontiguousarray(bq[:, dsl]).astype(BF16)
            m[f"bv{l}"] = np.ascontiguousarray(bv[:, dsl]).astype(BF16)
            m[f"wq{l}"] = np.ascontiguousarray(wq[:, dsl].reshape(KC, 128, DL)).astype(BF16)
            m[f"wo{l}"] = np.ascontiguousarray(wo[dsl].reshape(NH, 128, D)).astype(BF16)
            if l == 0:
                m["wd0"] = np.ascontiguousarray(wd[fsl].reshape(FC, 128, D)).astype(BF16)
                # gate|up interleaved, fc-major, contiguous per partition
                wg4 = wg[:, fsl].reshape(KC, 128, FC, 128).transpose(2, 1, 0, 3)
                wu4 = wu[:, fsl].reshape(KC, 128, FC, 128).transpose(2, 1, 0, 3)
                wgu = np.concatenate([wg4, wu4], axis=-1)  # [FC,128,KC,256]
                m["wgu0"] = np.ascontiguousarray(wgu).astype(BF16)
            else:
                wg_r = wg[:, fsl].reshape(KC, 128, FL)
                wu_r = wu[:, fsl].reshape(KC, 128, FL)
                m["wgu1"] = np.ascontiguousarray(
                    np.concatenate([wg_r, wu_r], axis=-1)).astype(BF16)
                m["wd1"] = np.ascontiguousarray(
                    wd[fsl].reshape(FC, 128, D)).astype(BF16)
        in_maps.append(m)
    return in_maps


def run_on_device(in_maps, trace=False):
    nc = _get_program()
    return bass_utils.run_bass_kernel_spmd(
        nc, in_maps, core_ids=list(range(N_CORES)), trace=trace,
    )


def kernel(**inputs):
    in_maps = _host_prepare(inputs)
    res = run_on_device(in_maps, trace=False)
    out = np.stack([
        res.results[0]["out"].reshape(OUT),
        res.results[TP]["out"].reshape(OUT),
    ]).astype(FP32)
    return out


# revision 50
# speedup vs baseline: 1.1116x; 1.0150x over previous
"""Trainium2 Bass kernel for nn_LlamaForSequenceRegression_14336600834254.

2-layer Llama (D=2048, H=16, HD=128, F=5632, LoRA r=16 on q/v) + regression
head, B=2, S=1024, fp32 reference.

Distribution (8 NeuronCores): DP2 x TP4.
  - cores 0-3 process batch 0, cores 4-7 batch 1 (data parallel).
  - within each group of 4: Megatron tensor parallel — Wq/Wk/Wv column
    shards (4 heads/core), Wo row shards, Wgate/Wup column shards
    (F/4=1408), Wdown row shards. AllReduce (bf16) after attn-out and
    after MLP-down, replica_groups=[[0,1,2,3],[4,5,6,7]].
  - embedding gather + norm-weight folding are done host-side; all
    device matmuls run in bf16 with fp32 PSUM accumulation; the
    residual stream / softmax / rmsnorm statistics are fp32.

Layout: activations are kept feature-major ("transposed"): h^T [D, T] as
SBUF tiles [128 part, 16 kchunk, 1024 tok] so every weight matmul uses the
natural [in, out] weight layout as lhsT and no transposes are needed.
Attention uses scores^T [Tk, Tq] so softmax needs no max-subtraction
(|scores| < ~6 with folded 1/sqrt(HD)) and probs feed the v-matmul
directly; the causal mask is an upload-once 0/1 strip multiplied into the
diagonal tiles, and the attention_mask rides the exp() per-partition bias.

Perf structure (v2):
  - attention / out_proj / AllReduce pipelined per token half so the attn
    AllReduce overlaps the other half's attention + the first MLP half.
  - partition reductions (rmsnorm sum-of-squares, softmax denominators)
    use an all-ones [128,128] stationary operand so the PSUM result is
    already broadcast across partitions: no M=1 matmuls, no
    single-partition reciprocals, no gpsimd broadcasts.
  - last layer runs in "tail" mode: only the last token flows through
    q/attention/Wo/MLP. Those matvecs are transposed — the activation
    vector is the stationary operand and the weights stream through the
    PE as the moving operand — so the tail is weight-DMA bound instead
    of LDWEIGHTS bound.
"""

import numpy as np
import ml_dtypes

import concourse.bacc as bacc
import concourse.tile as tile
from concourse import mybir
from concourse import bass_utils

BF16 = ml_dtypes.bfloat16
FP32 = np.float32

V, D, L, H, HD, F, R, ALPHA, B, S, OUT = 32000, 2048, 2, 16, 128, 5632, 16, 32, 2, 1024, 11
EPS = 1e-5
SCALING = ALPHA / R
N_CORES = 8
TP = 4
NH = H // TP          # 4 local heads
DL = NH * HD          # 512 local q/k/v cols
FL = F // TP          # 1408 local mlp cols
KC = D // 128         # 16 contraction chunks
FC = FL // 128        # 11 mlp chunks
TT = 512              # token tile (free dim per matmul)
NT = S // TT          # 2 token tiles
TC = S // 128         # 8 token chunks (128-wide)
DC = D // TT          # 4 output chunks of 512
FLCH = [(0, 512), (512, 512), (1024, 384)]  # FL split into <=512 chunks
REPLICA_GROUPS = [[0, 1, 2, 3], [4, 5, 6, 7]]

dt = mybir.dt


def build_program():
    """Build the SPMD Bass program (identical on all 8 cores; weights differ
    per core via the input maps)."""
    nc = bacc.Bacc(num_devices=N_CORES, debug=False)

    # ---- DRAM I/O ----
    xT = nc.dram_tensor("xT", [128, KC, S], dt.bfloat16, kind="ExternalInput")
    cosT = nc.dram_tensor("cosT", [128, S], dt.bfloat16, kind="ExternalInput")
    sinT = nc.dram_tensor("sinT", [128, S], dt.bfloat16, kind="ExternalInput")
    mstrip = nc.dram_tensor("mstrip", [128, 896], dt.bfloat16, kind="ExternalInput")
    maskbias = nc.dram_tensor("maskbias", [128, TC], dt.float32, kind="ExternalInput")
    mask01 = nc.dram_tensor("mask01", [128, TC], dt.float32, kind="ExternalInput")
    wreg = nc.dram_tensor("wreg", [KC, 128, OUT], dt.bfloat16, kind="ExternalInput")
    breg = nc.dram_tensor("breg", [OUT, 1], dt.float32, kind="ExternalInput")
    W = {}
    for l in range(L):
        W[f"wk{l}"] = nc.dram_tensor(f"wk{l}", [KC, 128, DL], dt.bfloat16, kind="ExternalInput")
        W[f"wv{l}"] = nc.dram_tensor(f"wv{l}", [KC, 128, DL], dt.bfloat16, kind="ExternalInput")
        W[f"aq{l}"] = nc.dram_tensor(f"aq{l}", [KC, 128, R], dt.bfloat16, kind="ExternalInput")
        W[f"av{l}"] = nc.dram_tensor(f"av{l}", [KC, 128, R], dt.bfloat16, kind="ExternalInput")
        W[f"bq{l}"] = nc.dram_tensor(f"bq{l}", [R, DL], dt.bfloat16, kind="ExternalInput")
        W[f"bv{l}"] = nc.dram_tensor(f"bv{l}", [R, DL], dt.bfloat16, kind="ExternalInput")
    # layer 0 (full-sequence Megatron TP layouts)
    W["wq0"] = nc.dram_tensor("wq0", [KC, 128, DL], dt.bfloat16, kind="ExternalInput")
    W["wo0"] = nc.dram_tensor("wo0", [NH, 128, D], dt.bfloat16, kind="ExternalInput")
    # gate|up interleaved per fc chunk: [fc][p][k*256 + (0:128 gate | 128:256 up)]
    W["wgu0"] = nc.dram_tensor("wgu0", [FC, 128, KC, 256], dt.bfloat16, kind="ExternalInput")
    W["wd0"] = nc.dram_tensor("wd0", [FC, 128, D], dt.bfloat16, kind="ExternalInput")
    # layer 1 (tail: weights stream as moving operand)
    W["wq1"] = nc.dram_tensor("wq1", [KC, 128, DL], dt.bfloat16, kind="ExternalInput")
    W["wo1"] = nc.dram_tensor("wo1", [NH, 128, D], dt.bfloat16, kind="ExternalInput")
    W["wgu1"] = nc.dram_tensor("wgu1", [KC, 128, 2 * FL], dt.bfloat16, kind="ExternalInput")
    W["wd1"] = nc.dram_tensor("wd1", [FC, 128, D], dt.bfloat16, kind="ExternalInput")
    out_dram = nc.dram_tensor("out", [OUT, 1], dt.float32, kind="ExternalOutput")

    with tile.TileContext(nc) as tc:
        with (
            tc.tile_pool(name="persist", bufs=1) as pp,
            tc.tile_pool(name="wts", bufs=5) as wp,
            tc.tile_pool(name="colw", bufs=8) as cwp,
            tc.tile_pool(name="tails", bufs=3) as tsp,
            tc.tile_pool(name="tmp", bufs=3) as tp_,
            tc.tile_pool(name="stage", bufs=3) as stp,
            tc.tile_pool(name="psum", bufs=8, space="PSUM") as ps,
            tc.tile_pool(name="dram", bufs=1, space="DRAM") as dram,
        ):
            f32, bf = dt.float32, dt.bfloat16
            # ---- persistent tiles ----
            h = pp.tile([128, KC, S], bf, tag="h")
            hn = pp.tile([128, KC, S], bf, tag="hn")
            cos_sb = pp.tile([128, S], bf, tag="cos")
            sin_sb = pp.tile([128, S], bf, tag="sin")
            mstrip_sb = pp.tile([128, 896], bf, tag="mstrip")
            mb_sb = pp.tile([128, TC], f32, tag="mb")
            m01_sb = pp.tile([128, TC], f32, tag="m01")
            cos_last = pp.tile([128, 1], f32, tag="cos_last")
            sin_last = pp.tile([128, 1], f32, tag="sin_last")
            ones_bf = pp.tile([128, 1], bf, tag="onesbf")
            allones = pp.tile([128, 128], bf, tag="allones")
            oneD128 = pp.tile([128, 128], bf, tag="oneD128")
            one1_bf = pp.tile([1, 1], bf, tag="one1")
            one64_bf = pp.tile([1, 1], bf, tag="one64")
            eps128 = pp.tile([128, 1], f32, tag="eps128")
            eps1 = pp.tile([1, 1], f32, tag="eps1")
            qT = pp.tile([128, NH, S], bf, tag="qT")     # q, then reused for ctx
            kT = pp.tile([128, NH, S], bf, tag="kT")
            vN = pp.tile([128, TC, DL], bf, tag="vN")
            expT = pp.tile([128, TC, TT], bf, tag="expT")
            mT = pp.tile([128, FC, TT], bf, tag="mT")    # per-half mlp act
            aqw = pp.tile([128, KC, R], bf, tag="aqw")
            avw = pp.tile([128, KC, R], bf, tag="avw")
            bq_sb = pp.tile([R, DL], bf, tag="bq")
            bv_sb = pp.tile([R, DL], bf, tag="bv")
            aq_sb = pp.tile([R, S], bf, tag="aq")
            av_sb = pp.tile([R, S], bf, tag="av")
            wreg_sb = pp.tile([128, KC, OUT], bf, tag="wreg")
            breg_sb = pp.tile([OUT, 1], f32, tag="breg")
            # tail smalls
            hn_last = pp.tile([128, KC, 1], bf, tag="hn_last")
            q_last = pp.tile([128, NH], bf, tag="q_last")
            ctx_n = pp.tile([128, NH], bf, tag="ctx_n")
            exp_tail = pp.tile([128, NH, TC], bf, tag="exp_tail")
            m_pm = pp.tile([128, FC], bf, tag="m_pm")

            # ---- constants in ----
            nc.vector.memset(ones_bf[:], 1.0)
            nc.vector.memset(allones[:], 1.0)
            nc.vector.memset(oneD128[:], 1.0 / D)
            nc.vector.memset(one1_bf[:], 1.0)
            nc.vector.memset(one64_bf[:], 1.0 / 64.0)
            nc.vector.memset(eps128[:], EPS)
            nc.vector.memset(eps1[:], EPS)
            nc.sync.dma_start(cos_sb[:], cosT[:])
            nc.sync.dma_start(sin_sb[:], sinT[:])
            nc.sync.dma_start(mstrip_sb[:], mstrip[:])
            nc.sync.dma_start(mb_sb[:], maskbias[:])
            nc.sync.dma_start(m01_sb[:], mask01[:])
            nc.scalar.copy(cos_last[:], cos_sb[:, S - 1:S])
            nc.scalar.copy(sin_last[:], sin_sb[:, S - 1:S])
            nc.sync.dma_start(breg_sb[:], breg[:])
            for k in range(KC):
                nc.sync.dma_start(wreg_sb[:, k, :], wreg[k])

            # ---- h init per half: bf16 upload -> fp32 residual ----
            for t in range(NT):
                ts_ = slice(t * TT, (t + 1) * TT)
                nc.sync.dma_start(h[:, :, ts_], xT[:, :, ts_])

            # DRAM bounce buffers for collectives
            ar1h_in = [dram.tile([128, KC, TT], bf, name=f"ar1hi_{t}") for t in range(NT)]
            ar1h_out = [dram.tile([128, KC, TT], bf, name=f"ar1ho_{t}") for t in range(NT)]
            ar2h_in = [dram.tile([128, KC, TT], bf, name=f"ar2hi_{t}") for t in range(NT)]
            ar2h_out = [dram.tile([128, KC, TT], bf, name=f"ar2ho_{t}") for t in range(NT)]
            ar_in_s = dram.tile([128, KC, 1], bf)
            ar_out_s = dram.tile([128, KC, 1], bf)

            def norm_half(t):
                """hn[:, :, half t] = h / sqrt(mean(h^2) + eps), bf16."""
                ts_ = slice(t * TT, (t + 1) * TT)
                psb = ps.tile([128, TT], f32, tag="psum", name=f"nps_{t}")
                for k in range(KC):
                    sq = tp_.tile([128, TT], bf, tag="sq", bufs=3, name=f"nsq_{k}_{t}")
                    nc.scalar.activation(sq[:], h[:, k, ts_],
                                         mybir.ActivationFunctionType.Square)
                    # all-ones/D stationary => result broadcast to all partitions
                    nc.tensor.matmul(psb[:], oneD128[:], sq[:],
                                     start=(k == 0), stop=(k == KC - 1))
                rs = tp_.tile([128, TT], f32, tag="rsbc", bufs=2, name=f"nrs_{t}")
                nc.scalar.activation(rs[:], psb[:],
                                     mybir.ActivationFunctionType.Sqrt, bias=eps128[:])
                nc.vector.reciprocal_approx_fast(rs[:], rs[:])
                for k in range(KC):
                    nc.vector.tensor_mul(hn[:, k, ts_], h[:, k, ts_], rs[:])

            def lora_down_half(aw, dst, t):
                psa = ps.tile([128, TT], f32, tag="psum", name=f"ldh_{id(aw)}_{t}")
                for k in range(KC):
                    nc.tensor.matmul(
                        psa[0:R, :], aw[:, k, :], hn[:, k, t * TT:(t + 1) * TT],
                        start=(k == 0), stop=(k == KC - 1),
                    )
                nc.scalar.copy(dst[:, t * TT:(t + 1) * TT], psa[0:R, :])

            def rope_from_psum(psq, dst, hc, t):
                """Apply RoPE to psum [128,TT] (one head, token tile t) and
                write bf16 into dst[:, hc, t*TT:...]."""
                ts_ = slice(t * TT, (t + 1) * TT)
                t2 = tp_.tile([128, TT], bf, tag="ropetB", bufs=1)
                t4 = tp_.tile([128, TT], bf, tag="ropetB", bufs=1)
                nc.vector.tensor_mul(dst[0:64, hc, ts_], psq[0:64, :], cos_sb[0:64, ts_])
                nc.vector.tensor_mul(t2[0:64, :], psq[64:128, :], sin_sb[0:64, ts_])
                nc.vector.tensor_sub(dst[0:64, hc, ts_], dst[0:64, hc, ts_], t2[0:64, :])
                nc.vector.tensor_mul(dst[64:128, hc, ts_], psq[64:128, :], cos_sb[64:128, ts_])
                nc.vector.tensor_mul(t4[64:128, :], psq[0:64, :], sin_sb[64:128, ts_])
                nc.vector.tensor_add(dst[64:128, hc, ts_], dst[64:128, hc, ts_], t4[64:128, :])

            def qk_proj(wname, dst, lora_bw, lora_act):
                """dst[:, hc, :] (bf16, roped) = rope(W.T @ hn [+ lora])."""
                psq = [[ps.tile([128, TT], f32, tag="psum", name=f"psq_{wname}_{hc}_{t}")
                        for t in range(NT)] for hc in range(NH)]
                for kk in range(KC // 2):
                    wt = wp.tile([128, 2, DL], bf, tag="wqkv", name=f"w_{wname}_{kk}")
                    nc.sync.dma_start(wt[:], W[wname][2 * kk:2 * kk + 2].rearrange("i p c -> p i c"))
                    for i in range(2):
                        k = 2 * kk + i
                        for hc in range(NH):
                            for t in range(NT):
                                nc.tensor.matmul(
                                    psq[hc][t][:], wt[:, i, hc * HD:(hc + 1) * HD],
                                    hn[:, k, t * TT:(t + 1) * TT],
                                    start=(k == 0),
                                    stop=(lora_bw is None and k == KC - 1),
                                )
                for hc in range(NH):
                    for t in range(NT):
                        if lora_bw is not None:
                            nc.tensor.matmul(
                                psq[hc][t][:], lora_bw[:, hc * HD:(hc + 1) * HD],
                                lora_act[:, t * TT:(t + 1) * TT],
                                start=False, stop=True,
                            )
                        rope_from_psum(psq[hc][t], dst, hc, t)

            def v_proj(l):
                """vN [128(tok), TC, DL] bf16 = hn.T @ Wv + lora."""
                psv = [ps.tile([128, DL], f32, tag="psum", name=f"psv_{c}")
                       for c in range(TC)]
                for kk in range(KC // 2):
                    wt = wp.tile([128, 2, DL], bf, tag="wqkv", name=f"wv_t_{kk}")
                    nc.sync.dma_start(wt[:], W[f"wv{l}"][2 * kk:2 * kk + 2].rearrange("i p c -> p i c"))
                    for i in range(2):
                        k = 2 * kk + i
                        for c in range(TC):
                            nc.tensor.matmul(
                                psv[c][:], hn[:, k, c * 128:(c + 1) * 128], wt[:, i, :],
                                start=(k == 0), stop=False,
                            )
                for c in range(TC):
                    nc.tensor.matmul(
                        psv[c][:], av_sb[:, c * 128:(c + 1) * 128], bv_sb[:],
                        start=False, stop=True,
                    )
                    nc.scalar.copy(vN[:, c, :], psv[c][:])

            def q_proj_half(wname, dst, t):
                """dst[:, hc, half t] = rope(Wq.T @ hn + lora)."""
                psq = [ps.tile([128, TT], f32, tag="psum", name=f"psqq_{hc}_{t}")
                       for hc in range(NH)]
                for kk in range(KC // 2):
                    wt = wp.tile([128, 2, DL], bf, tag="wqkv", name=f"wq_{kk}_{t}")
                    nc.sync.dma_start(wt[:], W[wname][2 * kk:2 * kk + 2].rearrange("i p c -> p i c"))
                    for i in range(2):
                        k = 2 * kk + i
                        for hc in range(NH):
                            nc.tensor.matmul(
                                psq[hc][:], wt[:, i, hc * HD:(hc + 1) * HD],
                                hn[:, k, t * TT:(t + 1) * TT],
                                start=(k == 0), stop=False,
                            )
                for hc in range(NH):
                    nc.tensor.matmul(
                        psq[hc][:], bq_sb[:, hc * HD:(hc + 1) * HD],
                        aq_sb[:, t * TT:(t + 1) * TT],
                        start=False, stop=True,
                    )
                    rope_from_psum(psq[hc], dst, hc, t)

            def attention_half(t):
                """qT,kT,vN -> ctx (written into qT) for token half t.

                The denominator/ctx accumulation matmuls for chunk j are
                emitted two chunks behind the QK+exp chain, so by the time
                the PE (strict in-order queue) reaches them their exp input
                has drained from the ACT engine and nothing head-of-line
                blocks."""
                ts_ = slice(t * TT, (t + 1) * TT)
                jmax = (t + 1) * (TT // 128)
                LAG = 2
                for hh in range(NH):
                    psb = ps.tile([128, TT], f32, tag="psum", name=f"psd_{hh}_{t}")
                    psc = ps.tile([128, TT], f32, tag="psum", name=f"psc_{hh}_{t}")

                    def emit_reduce_j(j):
                        nc.tensor.matmul(
                            psb[:], allones[:], expT[:, j, :],
                            start=(j == 0), stop=(j == jmax - 1),
                        )
                        nc.tensor.matmul(
                            psc[:], vN[:, j, hh * HD:(hh + 1) * HD],
                            expT[:, j, :],
                            start=(j == 0), stop=(j == jmax - 1),
                        )

                    for j in range(jmax):
                        pss = ps.tile([128, TT], f32, tag="psum",
                                      name=f"pss_{hh}_{t}_{j}")
                        nc.tensor.matmul(
                            pss[:], kT[:, hh, j * 128:(j + 1) * 128],
                            qT[:, hh, ts_], start=True, stop=True,
                        )
                        nc.scalar.activation(
                            expT[:, j, :], pss[:],
                            mybir.ActivationFunctionType.Exp,
                            bias=mb_sb[:, j:j + 1], scale=1.0,
                        )
                        off = t * TT - j * 128
                        if off < 128:
                            nc.vector.tensor_mul(
                                expT[:, j, :], expT[:, j, :],
                                mstrip_sb[:, 384 + off:896 + off],
                            )
                        if j >= LAG:
                            emit_reduce_j(j - LAG)
                    for j in range(max(0, jmax - LAG), jmax):
                        emit_reduce_j(j)
                    rden = tp_.tile([128, TT], f32, tag="rsbc", bufs=2,
                                    name=f"rden_{hh}_{t}")
                    nc.vector.reciprocal_approx_fast(rden[:], psb[:])
                    nc.vector.tensor_mul(qT[:, hh, ts_], psc[:], rden[:])

            def out_proj_half(l, t):
                """attn partial for token half t -> ar1h_in[t]."""
                for og in range(4):  # groups of 4 output chunks
                    pso = [ps.tile([128, TT], f32, tag="psum", name=f"pso_{og}_{oi}_{t}")
                           for oi in range(4)]
                    for hp in range(NH // 2):
                        wt = wp.tile([128, 2, TT], bf, tag="wqkv",
                                     name=f"wo_t_{og}_{hp}_{t}")
                        nc.sync.dma_start(
                            wt[:], W[f"wo{l}"][2 * hp:2 * hp + 2,
                                               :, og * 512:(og + 1) * 512]
                            .rearrange("i p c -> p i c"))
                        for i in range(2):
                            hc = 2 * hp + i
                            for oi in range(4):
                                nc.tensor.matmul(
                                    pso[oi][:], wt[:, i, oi * 128:(oi + 1) * 128],
                                    qT[:, hc, t * TT:(t + 1) * TT],
                                    start=(hc == 0), stop=(hc == NH - 1),
                                )
                    for oi in range(4):
                        st = stp.tile([128, TT], bf, tag="stage")
                        nc.vector.tensor_copy(st[:], pso[oi][:])
                        nc.sync.dma_start(ar1h_in[t][:, og * 4 + oi, :], st[:])

            def allreduce1_half(t):
                nc.gpsimd.collective_compute(
                    "AllReduce", mybir.AluOpType.add,
                    replica_groups=REPLICA_GROUPS,
                    ins=[ar1h_in[t].opt()], outs=[ar1h_out[t].opt()],
                )

            def add1_half(t):
                # landing DMA rides the scalar HWDGE ring so its AR wait can't
                # head-of-line-block weight loads on the sync ring
                ts_ = slice(t * TT, (t + 1) * TT)
                nc.scalar.dma_start(hn[:, :, ts_], ar1h_out[t][:])
                for k in range(KC):
                    nc.vector.tensor_add(h[:, k, ts_], h[:, k, ts_], hn[:, k, ts_])

            def allreduce2_half(t):
                nc.gpsimd.collective_compute(
                    "AllReduce", mybir.AluOpType.add,
                    replica_groups=REPLICA_GROUPS,
                    ins=[ar2h_in[t].opt()], outs=[ar2h_out[t].opt()],
                )

            def add2_half(t):
                ts_ = slice(t * TT, (t + 1) * TT)
                nc.scalar.dma_start(hn[:, :, ts_], ar2h_out[t][:])
                for k in range(KC):
                    nc.vector.tensor_add(h[:, k, ts_], h[:, k, ts_], hn[:, k, ts_])

            def k_proj_half(wname, dst, t, tag="wqkv"):
                psq = [ps.tile([128, TT], f32, tag="psum", name=f"psqh_{hc}_{t}")
                       for hc in range(NH)]
                for kk in range(KC // 2):
                    wt = wp.tile([128, 2, DL], bf, tag=tag, name=f"wkh_{wname}_{kk}_{t}")
                    nc.sync.dma_start(wt[:], W[wname][2 * kk:2 * kk + 2].rearrange("i p c -> p i c"))
                    for i in range(2):
                        k = 2 * kk + i
                        for hc in range(NH):
                            nc.tensor.matmul(
                                psq[hc][:], wt[:, i, hc * HD:(hc + 1) * HD],
                                hn[:, k, t * TT:(t + 1) * TT],
                                start=(k == 0), stop=(k == KC - 1),
                            )
                for hc in range(NH):
                    rope_from_psum(psq[hc], dst, hc, t)

            def v_proj_half(l, t, tag="wqkv"):
                psv = [ps.tile([128, DL], f32, tag="psum", name=f"psvh_{c}_{t}")
                       for c in range(4)]
                for kk in range(KC // 2):
                    wt = wp.tile([128, 2, DL], bf, tag=tag, name=f"wvh_{l}_{kk}_{t}")
                    nc.sync.dma_start(wt[:], W[f"wv{l}"][2 * kk:2 * kk + 2].rearrange("i p c -> p i c"))
                    for i in range(2):
                        k = 2 * kk + i
                        for ci in range(4):
                            c = t * 4 + ci
                            nc.tensor.matmul(
                                psv[ci][:], hn[:, k, c * 128:(c + 1) * 128], wt[:, i, :],
                                start=(k == 0), stop=False,
                            )
                for ci in range(4):
                    c = t * 4 + ci
                    nc.tensor.matmul(
                        psv[ci][:], av_sb[:, c * 128:(c + 1) * 128], bv_sb[:],
                        start=False, stop=True,
                    )
                    nc.scalar.copy(vN[:, c, :], psv[ci][:])

            def mlp_gate_up_half(l, t):
                ts_ = slice(t * TT, (t + 1) * TT)
                for fc in range(FC):
                    psg = ps.tile([128, TT], f32, tag="psum", name=f"psg_{fc}_{t}")
                    psu = ps.tile([128, TT], f32, tag="psum", name=f"psu_{fc}_{t}")
                    for kh in range(2):
                        wgu = cwp.tile([128, KC // 2, 256], bf, tag="wgu",
                                       name=f"wgu_{fc}_{kh}_{t}")
                        nc.sync.dma_start(
                            wgu[:], W[f"wgu{l}"][fc][:, kh * (KC // 2):(kh + 1) * (KC // 2), :])
                        for ki in range(KC // 2):
                            k = kh * (KC // 2) + ki
                            nc.tensor.matmul(psg[:], wgu[:, ki, 0:128], hn[:, k, ts_],
                                             start=(k == 0), stop=(k == KC - 1))
                            nc.tensor.matmul(psu[:], wgu[:, ki, 128:256], hn[:, k, ts_],
                                             start=(k == 0), stop=(k == KC - 1))
                    sg = tp_.tile([128, TT], bf, tag="silu", bufs=1, name=f"sg_{fc}_{t}")
                    nc.scalar.activation(sg[:], psg[:], mybir.ActivationFunctionType.Silu)
                    nc.vector.tensor_mul(mT[:, fc, :], sg[:], psu[:])

            def mlp_down_half(l, t):
                for og in range(4):
                    pso = [ps.tile([128, TT], f32, tag="psum", name=f"psd_{og}_{oi}_{t}")
                           for oi in range(4)]
                    for kp in range((FC + 1) // 2):
                        nk = min(2, FC - 2 * kp)
                        wt = wp.tile([128, 2, TT], bf, tag="wqkv",
                                     name=f"wd_t_{og}_{kp}_{t}")
                        nc.sync.dma_start(
                            wt[:, 0:nk, :],
                            W[f"wd{l}"][2 * kp:2 * kp + nk,
                                        :, og * 512:(og + 1) * 512]
                            .rearrange("i p c -> p i c"))
                        for i in range(nk):
                            kc = 2 * kp + i
                            for oi in range(4):
                                nc.tensor.matmul(
                                    pso[oi][:], wt[:, i, oi * 128:(oi + 1) * 128],
                                    mT[:, kc, :],
                                    start=(kc == 0), stop=(kc == FC - 1),
                                )
                    for oi in range(4):
                        st = stp.tile([128, TT], bf, tag="stage")
                        nc.vector.tensor_copy(st[:], pso[oi][:])
                        nc.sync.dma_start(ar2h_in[t][:, og * 4 + oi, :], st[:])

            # ---------- tail (last layer): only the last token flows through
            # q/attention/Wo/MLP. Matvecs are transposed: the activation
            # column is the stationary operand, weights stream as rhs.

            def row_to_pm(row_sb, psum_pm, ncols):
                """psum_pm[:, c] = row_sb[0, c*128:(c+1)*128].T via K=1 matmuls."""
                for c in range(ncols):
                    nc.tensor.matmul(
                        psum_pm[:, c:c + 1],
                        row_sb[0:1, c * 128:(c + 1) * 128], one1_bf[:],
                        start=True, stop=True,
                    )

            def q_tail():
                psq = ps.tile([128, TT], f32, tag="psum", name="psq_tail")
                for kk in range(KC // 2):
                    wt = wp.tile([128, 2, DL], bf, tag="wqkv", name=f"wq1s_{kk}")
                    nc.sync.dma_start(wt[:], W["wq1"][2 * kk:2 * kk + 2].rearrange("i p c -> p i c"))
                    for i in range(2):
                        k = 2 * kk + i
                        nc.tensor.matmul(psq[0:1, :], hn[:, k, S - 1:S], wt[:, i, :],
                                         start=(k == 0), stop=False)
                nc.tensor.matmul(psq[0:1, :], aq_sb[:, S - 1:S], bq_sb[:],
                                 start=False, stop=True)
                qrow = tp_.tile([1, DL], bf, tag="qrow", bufs=1)
                nc.scalar.copy(qrow[:], psq[0:1, :])
                psqpm = ps.tile([128, NH], f32, tag="psum", name="psqpm")
                row_to_pm(qrow, psqpm, NH)
                # rope, all heads at once (per-partition cos/sin scalars)
                t2 = tp_.tile([128, NH], bf, tag="ropeS", bufs=2)
                t4 = tp_.tile([128, NH], bf, tag="ropeS", bufs=2)
                nc.vector.tensor_scalar_mul(q_last[0:64, :], psqpm[0:64, :],
                                            cos_last[0:64, :])
                nc.vector.tensor_scalar_mul(t2[0:64, :], psqpm[64:128, :],
                                            sin_last[0:64, :])
                nc.vector.tensor_sub(q_last[0:64, :], q_last[0:64, :], t2[0:64, :])
                nc.vector.tensor_scalar_mul(q_last[64:128, :], psqpm[64:128, :],
                                            cos_last[64:128, :])
                nc.vector.tensor_scalar_mul(t4[64:128, :], psqpm[0:64, :],
                                            sin_last[64:128, :])
                nc.vector.tensor_add(q_last[64:128, :], q_last[64:128, :], t4[64:128, :])

            def attention_tail():
                # scores for all heads/key-chunks in one psum tile
                pss = ps.tile([128, NH, TC], f32, tag="psum", name="pss_tail")
                for hh in range(NH):
                    for j in range(TC):
                        nc.tensor.matmul(
                            pss[:, hh, j:j + 1],
                            kT[:, hh, j * 128:(j + 1) * 128], q_last[:, hh:hh + 1],
                            start=True, stop=True,
                        )
                nc.scalar.activation(exp_tail[:], pss[:],
                                     mybir.ActivationFunctionType.Exp)
                for j in range(TC):
                    nc.vector.tensor_scalar_mul(exp_tail[:, :, j], exp_tail[:, :, j],
                                                m01_sb[:, j:j + 1])
                # denominators: all-partition sums then free-dim reduce per head
                psdb = ps.tile([128, NH, TC], f32, tag="psum", name="psdb_tail")
                nc.tensor.matmul(psdb[:], allones[:], exp_tail[:],
                                 start=True, stop=True)
                rdt = tp_.tile([128, NH], f32, tag="rdt")
                nc.vector.tensor_reduce(
                    rdt[:], psdb[:],
                    axis=mybir.AxisListType.X, op=mybir.AluOpType.add,
                )
                nc.vector.reciprocal_approx_fast(rdt[:], rdt[:])
                psc = ps.tile([128, NH], f32, tag="psum", name="psc_tail")
                for hh in range(NH):
                    for j in range(TC):
                        nc.tensor.matmul(
                            psc[:, hh:hh + 1], vN[:, j, hh * HD:(hh + 1) * HD],
                            exp_tail[:, hh, j:j + 1],
                            start=(j == 0), stop=(j == TC - 1),
                        )
                nc.vector.tensor_mul(ctx_n[:], psc[:], rdt[:])

            def wo_tail():
                pso = [ps.tile([128, TT], f32, tag="psum", name=f"psot_{c}")
                       for c in range(DC)]
                for hc in range(NH):
                    for dh in range(2):
                        wt = tsp.tile([128, 1024], bf, tag="wod1s",
                                      name=f"wo1s_{hc}_{dh}")
                        nc.sync.dma_start(wt[:], W["wo1"][hc][:, dh * 1024:(dh + 1) * 1024])
                        for ci in range(2):
                            c = dh * 2 + ci
                            nc.tensor.matmul(
                                pso[c][0:1, :], ctx_n[:, hc:hc + 1],
                                wt[:, ci * 512:(ci + 1) * 512],
                                start=(hc == 0), stop=(hc == NH - 1),
                            )
                orow = tp_.tile([1, D], bf, tag="orow", bufs=1)
                for c in range(DC):
                    nc.scalar.copy(orow[0:1, c * 512:(c + 1) * 512], pso[c][0:1, :])
                pst = ps.tile([128, KC], f32, tag="psum", name="pst_wo")
                row_to_pm(orow, pst, KC)
                stpm = tp_.tile([128, KC], bf, tag="stpm", bufs=2, name="stpm_wo")
                nc.vector.tensor_copy(stpm[:], pst[:])
                nc.sync.dma_start(ar_in_s[:, :, 0], stpm[:])

            def allreduce_and_add_tail():
                nc.gpsimd.collective_compute(
                    "AllReduce", mybir.AluOpType.add,
                    replica_groups=REPLICA_GROUPS,
                    ins=[ar_in_s.opt()], outs=[ar_out_s.opt()],
                )
                lb = tp_.tile([128, KC, 1], bf, tag="ar_land", bufs=2)
                nc.scalar.dma_start(lb[:], ar_out_s[:])
                nc.vector.tensor_add(h[:, :, S - 1:S], h[:, :, S - 1:S], lb[:])

            def norm_tail_to_hn_last():
                sqt = tp_.tile([128, KC, 1], bf, tag="sqlast")
                nc.scalar.activation(sqt[:], h[:, :, S - 1:S],
                                     mybir.ActivationFunctionType.Square)
                psl = ps.tile([128, 1], f32, tag="psum", name="psl_normt")
                for k in range(KC):
                    nc.tensor.matmul(psl[:], oneD128[:], sqt[:, k, :],
                                     start=(k == 0), stop=(k == KC - 1))
                rsb = tp_.tile([128, 1], f32, tag="rsb_tail", bufs=2)
                nc.scalar.activation(rsb[:], psl[:],
                                     mybir.ActivationFunctionType.Sqrt, bias=eps128[:])
                nc.vector.reciprocal_approx_fast(rsb[:], rsb[:])
                nc.vector.tensor_scalar_mul(hn_last[:], h[:, :, S - 1:S], rsb[:])

            def mlp_tail():
                norm_tail_to_hn_last()
                # gate/up: stream combined gate|up weights as rhs
                psg = [ps.tile([128, TT], f32, tag="psum", name=f"psgt_{c}")
                       for c in range(len(FLCH))]
                psu = [ps.tile([128, TT], f32, tag="psum", name=f"psut_{c}")
                       for c in range(len(FLCH))]
                for k in range(KC):
                    wgt = tsp.tile([128, FL], bf, tag="wg1s", name=f"wg1s_{k}")
                    wut = tsp.tile([128, FL], bf, tag="wu1s", name=f"wu1s_{k}")
                    nc.sync.dma_start(wgt[:], W["wgu1"][k][:, 0:FL])
                    nc.sync.dma_start(wut[:], W["wgu1"][k][:, FL:2 * FL])
                    for c, (off, sz) in enumerate(FLCH):
                        nc.tensor.matmul(psg[c][0:1, 0:sz], hn_last[:, k, :],
                                         wgt[:, off:off + sz],
                                         start=(k == 0), stop=(k == KC - 1))
                        nc.tensor.matmul(psu[c][0:1, 0:sz], hn_last[:, k, :],
                                         wut[:, off:off + sz],
                                         start=(k == 0), stop=(k == KC - 1))
                mrow = tp_.tile([1, FL], bf, tag="mrow", bufs=1)
                for c, (off, sz) in enumerate(FLCH):
                    grow = tp_.tile([1, TT], bf, tag="grow", bufs=1, name=f"grow_{c}")
                    nc.scalar.activation(grow[0:1, 0:sz], psg[c][0:1, 0:sz],
                                         mybir.ActivationFunctionType.Silu)
                    nc.vector.tensor_mul(mrow[0:1, off:off + sz], grow[0:1, 0:sz],
                                         psu[c][0:1, 0:sz])
                psm = ps.tile([128, FC], f32, tag="psum", name="psm_pm")
                row_to_pm(mrow, psm, FC)
                nc.vector.tensor_copy(m_pm[:], psm[:])
                # down
                psd = [ps.tile([128, TT], f32, tag="psum", name=f"psdt2_{c}")
                       for c in range(DC)]
                for fc in range(FC):
                    for dh in range(2):
                        wdt = tsp.tile([128, 1024], bf, tag="wod1s",
                                       name=f"wd1s_{fc}_{dh}")
                        nc.sync.dma_start(wdt[:],
                                          W["wd1"][fc][:, dh * 1024:(dh + 1) * 1024])
                        for ci in range(2):
                            c = dh * 2 + ci
                            nc.tensor.matmul(
                                psd[c][0:1, :], m_pm[:, fc:fc + 1],
                                wdt[:, ci * 512:(ci + 1) * 512],
                                start=(fc == 0), stop=(fc == FC - 1),
                            )
                orow = tp_.tile([1, D], bf, tag="orow", bufs=1)
                for c in range(DC):
                    nc.scalar.copy(orow[0:1, c * 512:(c + 1) * 512], psd[c][0:1, :])
                pst = ps.tile([128, KC], f32, tag="psum", name="pst_mlp")
                row_to_pm(orow, pst, KC)
                stpm = tp_.tile([128, KC], bf, tag="stpm", bufs=2, name="stpm_mlp")
                nc.vector.tensor_copy(stpm[:], pst[:])
                nc.sync.dma_start(ar_in_s[:, :, 0], stpm[:])

            # ================= layer 0 (all per token half, so the first
            # attn AllReduce fires as early as possible) =================
            for k in range(KC):
                nc.sync.dma_start(aqw[:, k, :], W["aq0"][k])
                nc.sync.dma_start(avw[:, k, :], W["av0"][k])
            nc.sync.dma_start(bq_sb[:], W["bq0"][:])
            nc.sync.dma_start(bv_sb[:], W["bv0"][:])
            for t in range(NT):
                norm_half(t)
                lora_down_half(avw, av_sb, t)
                lora_down_half(aqw, aq_sb, t)
                k_proj_half("wk0", kT, t)
                v_proj_half(0, t)
                q_proj_half("wq0", qT, t)
                attention_half(t)
                out_proj_half(0, t)
                allreduce1_half(t)
            for t in range(NT):
                add1_half(t)
                norm_half(t)
                mlp_gate_up_half(0, t)
                mlp_down_half(0, t)
                allreduce2_half(t)

            # ================= layer 1 (tail) =================
            for k in range(KC):
                nc.sync.dma_start(aqw[:, k, :], W["aq1"][k])
                nc.sync.dma_start(avw[:, k, :], W["av1"][k])
            nc.sync.dma_start(bq_sb[:], W["bq1"][:])
            nc.sync.dma_start(bv_sb[:], W["bv1"][:])
            for t in range(NT):
                add2_half(t)
                norm_half(t)
                lora_down_half(avw, av_sb, t)
                if t == NT - 1:
                    lora_down_half(aqw, aq_sb, t)
                k_proj_half("wk1", kT, t)
                v_proj_half(1, t)
            q_tail()
            attention_tail()
            wo_tail()
            allreduce_and_add_tail()
            mlp_tail()
            allreduce_and_add_tail()

            # ================= final norm + head (last token only) ========
            sq = tp_.tile([128, KC, 1], bf, tag="sqlast")
            nc.scalar.activation(sq[:], h[:, :, S - 1:S],
                                 mybir.ActivationFunctionType.Square)
            psl = ps.tile([128, 1], f32, tag="psum", name="psl_final")
            for k in range(KC):
                nc.tensor.matmul(psl[:], oneD128[:], sq[:, k, :],
                                 start=(k == 0), stop=(k == KC - 1))
            rsl = tp_.tile([128, 1], f32, tag="rsl_final")
            nc.scalar.activation(rsl[:], psl[:],
                                 mybir.ActivationFunctionType.Sqrt, bias=eps128[:])
            nc.vector.reciprocal_approx_fast(rsl[:], rsl[:])
            hl = tp_.tile([128, KC, 1], bf, tag="hlast")
            nc.vector.tensor_scalar_mul(hl[:], h[:, :, S - 1:S], rsl[:])
            pso = ps.tile([128, TT], f32, tag="psum", name="ps_head")
            for k in range(KC):
                nc.tensor.matmul(pso[0:OUT, 0:1], wreg_sb[:, k, :], hl[:, k, :],
                                 start=(k == 0), stop=(k == KC - 1))
            ot = tp_.tile([OUT, 1], f32, tag="outt")
            nc.vector.tensor_add(ot[:], pso[0:OUT, 0:1], breg_sb[:])
            nc.sync.dma_start(out_dram[:], ot[:])

    nc.finalize()
    return nc


_CACHED = {}


def _get_program():
    if "nc" not in _CACHED:
        _CACHED["nc"] = build_program()
    return _CACHED["nc"]


def _host_prepare(inputs):
    """Fold norms/scales into weights, gather embeddings, build the 8
    per-core input maps."""
    ids = np.asarray(inputs["input_ids"]).astype(np.int64)        # [B,S]
    amask = np.asarray(inputs["attention_mask"]).astype(np.int64)  # [B,S]
    embed = np.asarray(inputs["embed"], FP32)

    inv_sqrt_hd = FP32(1.0 / np.sqrt(HD))

    # rope tables (half: both halves identical)
    inv = 1.0 / (10000.0 ** (np.arange(0, HD, 2, dtype=np.float64) / HD))
    ang = (np.arange(S, dtype=np.float64)[:, None] * inv[None, :])  # [S,64]
    cos64 = np.cos(ang).T.astype(BF16)  # [64,S]
    sin64 = np.sin(ang).T.astype(BF16)
    cosT = np.concatenate([cos64, cos64], axis=0).copy()  # [128,S]
    sinT = np.concatenate([sin64, sin64], axis=0).copy()

    # causal strip [128, 896]: strip[p,u] = 1 if (u-384) >= p else 0
    u = np.arange(896)[None, :]
    p = np.arange(128)[:, None]
    mstrip = ((u - 384) >= p).astype(BF16)

    common = {}

    def fold(l):
        g1 = np.asarray(inputs["norm1"], FP32)[l][:, None]
        g2 = np.asarray(inputs["norm2"], FP32)[l][:, None]
        wq = np.asarray(inputs["Wq"], FP32)[l] * g1 * inv_sqrt_hd
        wk = np.asarray(inputs["Wk"], FP32)[l] * g1
        wv = np.asarray(inputs["Wv"], FP32)[l] * g1
        aq = np.asarray(inputs["Aq"], FP32)[l] * g1
        av = np.asarray(inputs["Av"], FP32)[l] * g1
        bq = np.asarray(inputs["Bq"], FP32)[l] * (SCALING * inv_sqrt_hd)
        bv = np.asarray(inputs["Bv"], FP32)[l] * SCALING
        wo = np.asarray(inputs["Wo"], FP32)[l]
        wg = np.asarray(inputs["Wgate"], FP32)[l] * g2
        wu = np.asarray(inputs["Wup"], FP32)[l] * g2
        wd = np.asarray(inputs["Wdown"], FP32)[l]
        return wq, wk, wv, aq, av, bq, bv, wo, wg, wu, wd

    folded = [fold(l) for l in range(L)]
    wregf = (np.asarray(inputs["Wreg"], FP32) * np.asarray(inputs["norm_f"], FP32)[:, None])
    common["wreg"] = wregf.reshape(KC, 128, OUT).astype(BF16)
    common["breg"] = np.asarray(inputs["breg"], FP32).reshape(OUT, 1)
    common["cosT"] = cosT
    common["sinT"] = sinT
    common["mstrip"] = mstrip

    in_maps = []
    for c in range(N_CORES):
        b = c // TP      # batch index (DP group)
        r = c % TP       # TP rank within group
        m = dict(common)
        # embedding gather, transposed, bf16: [D,S] -> [16,128,S] -> [128,16,S]
        xt = embed[ids[b]].T.reshape(KC, 128, S).transpose(1, 0, 2)
        m["xT"] = np.ascontiguousarray(xt).astype(BF16)
        # attention_mask bias [128, TC]: col j, part p -> key token 128j+p
        mb = np.where(amask[b] > 0, FP32(0), FP32(-1e9)).reshape(TC, 128).T
        m["maskbias"] = np.ascontiguousarray(mb)
        m["mask01"] = np.ascontiguousarray((amask[b] > 0).reshape(TC, 128).T).astype(FP32)
        for l in range(L):
            wq, wk, wv, aq, av, bq, bv, wo, wg, wu, wd = folded[l]
            dsl = slice(r * DL, (r + 1) * DL)
            fsl = slice(r * FL, (r + 1) * FL)
            m[f"wk{l}"] = np.ascontiguousarray(wk[:, dsl].reshape(KC, 128, DL)).astype(BF16)
            m[f"wv{l}"] = np.ascontiguousarray(wv[:, dsl].reshape(KC, 128, DL)).astype(BF16)
            m[f"aq{l}"] = np.ascontiguousarray(aq.reshape(KC, 128, R)).astype(BF16)
            m[f"av{l}"] = np.ascontiguousarray(av.reshape(KC, 128, R)).astype(BF16)
            m[f"bq{l}"] = np.ascontiguousarray(bq[:, dsl]).astype(BF16)
            m[f"bv{l}"] = np.ascontiguousarray(bv[:, dsl]).astype(BF16)
            m[f"wq{l}"] = np.ascontiguousarray(wq[:, dsl].reshape(KC, 128, DL)).astype(BF16)
            m[f"wo{l}"] = np.ascontiguousarray(wo[dsl].reshape(NH, 128, D)).astype(BF16)
            if l == 0:
                m["wd0"] = np.ascontiguousarray(wd[fsl].reshape(FC, 128, D)).astype(BF16)
                # gate|up interleaved, fc-major, contiguous per partition
                wg4 = wg[:, fsl].reshape(KC, 128, FC, 128).transpose(2, 1, 0, 3)
                wu4 = wu[:, fsl].reshape(KC, 128, FC, 128).transpose(2, 1, 0, 3)
                wgu = np.concatenate([wg4, wu4], axis=-1)  # [FC,128,KC,256]
                m["wgu0"] = np.ascontiguousarray(wgu).astype(BF16)
            else:
                wg_r = wg[:, fsl].reshape(KC, 128, FL)
                wu_r = wu[:, fsl].reshape(KC, 128, FL)
                m["wgu1"] = np.ascontiguousarray(
                    np.concatenate([wg_r, wu_r], axis=-1)).astype(BF16)
                m["wd1"] = np.ascontiguousarray(
                    wd[fsl].reshape(FC, 128, D)).astype(BF16)
        in_maps.append(m)
    return in_maps


def run_on_device(in_maps, trace=False):
    nc = _get_program()
    return bass_utils.run_bass_kernel_spmd(
        nc, in_maps, core_ids=list(range(N_CORES)), trace=trace,
    )


def kernel(**inputs):
    in_maps = _host_prepare(inputs)
    res = run_on_device(in_maps, trace=False)
    out = np.stack([
        res.results[0]["out"].reshape(OUT),
        res.results[TP]["out"].reshape(OUT),
    ]).astype(FP32)
    return out
